# revision 12
# baseline (speedup 1.0000x reference)
"""Trainium2 Bass kernel for spatial attention (GroupNorm + QKV + softmax attention
+ output projection + residual), distributed over 8 NeuronCores.

Sharding: core = 2*b + hp handles image b (of 4) and head pair hp (heads 2hp, 2hp+1).
Each core computes GroupNorm(x[b]), its heads' q/k/v, full spatial attention for its
two heads, and a partial output projection (out_w columns for its heads). Core pairs'
partial outputs are summed on the host (hp==0 core carries the residual + bias).

Perf notes (v2):
- Scores stay bf16, row-tiled so both heads' S^T matmuls run concurrently.
- softmax exp is computed 2-of-3 on the DVE via a Schraudolph bit-trick directly
  into fp8e4m3 bits (uint8 = trunc(s*A + B) reinterpreted as fp8), 1-of-3 on ACT
  (fp8 output) — splitting the 10.6M-element exp load across engines.
- PV runs as fp8 DoubleRow matmuls (two key-chunks = K=256 per instruction),
  halving PV stream time; v^T keeps a 65th all-ones column so the softmax
  denominator accumulates in the same matmul.
- Output projection + residual + store are pipelined per-iblk (lag 1) so the PE
  stays dense through the end of the kernel (HAM stays warm).
"""

import math

import numpy as np

import concourse.bass as bass
import concourse.bacc as bacc
import concourse.tile as tile
from concourse import mybir
from concourse import bass_utils
from concourse.alu_op_type import AluOpType

B, C, H, W = 4, 256, 48, 48
HW = H * W  # 2304
NH, HD = 4, 64
G, GC = 16, 16  # 16 groups x 16 channels
EPS = 1e-5
NCORES = 8
JC = 128  # j (key spatial) chunk
NJ = HW // JC  # 18
NJP = NJ // 2  # 9 key-chunk pairs (DoubleRow K=256)
IBLKS = [(0, 512), (512, 1024), (1024, 1536), (1536, 2048), (2048, 2304)]
HALF = HW // 2  # 1152
PVM = HD + 1  # 65: 64 v channels + denominator ones row
VST = 80  # fp8 v^T subtile stride (16-byte aligned, >= PVM)
SALIGN = 512

F32 = mybir.dt.float32
BF16 = mybir.dt.bfloat16
F8 = mybir.dt.float8e4
U8 = mybir.dt.uint8
AX = mybir.AxisListType.X
AF = mybir.ActivationFunctionType
OP = AluOpType
PM = mybir.MatmulPerfMode

# Schraudolph exp into fp8e4m3 bit space: bits = trunc(s*EXPA + EXPB),
# value(bits) ~= exp(s/16).  EXPB tuned numerically for minimax rel err (~7%)
# assuming truncation on the DVE float->uint8 convert.
EXPA = 8.0 * math.log2(math.e) / 16.0
EXPB = 56.13
# exp engine split: ACT is slightly faster per unit and has less other work,
# so it takes 5 of every 9 key chunks; the DVE Schraudolph path takes 4.
ACT_JC = (0, 2, 4, 6, 8)


def _nchunks(size, step=512):
    # PSUM-bank-aligned chunks: a matmul output may not cross a 512-fp32 bank boundary
    return [(a, min(a + step, size)) for a in range(0, size, step)]


def _build(mm_dt=BF16):
    nc = bacc.Bacc("TRN2", target_bir_lowering=False, debug=False, enable_asserts=False)

    x_d = nc.dram_tensor("x", [C, HW], F32, kind="ExternalInput").ap()
    res_d = nc.dram_tensor("res", [C, HW], F32, kind="ExternalInput").ap()
    wq_d = nc.dram_tensor("wq", [C, 2 * HD], F32, kind="ExternalInput").ap()
    wk_d = nc.dram_tensor("wk", [C, 2 * HD], F32, kind="ExternalInput").ap()
    wv_d = nc.dram_tensor("wv", [C, 2 * HD], F32, kind="ExternalInput").ap()
    wo_d = nc.dram_tensor("wo", [2 * HD, C], F32, kind="ExternalInput").ap()
    gnp_d = nc.dram_tensor("gnp", [C, 2], F32, kind="ExternalInput").ap()
    gind_d = nc.dram_tensor("gind", [128, 32], F32, kind="ExternalInput").ap()
    gbc_d = nc.dram_tensor("gbc", [16, C], F32, kind="ExternalInput").ap()
    y_d = nc.dram_tensor("y", [C, HW], F32, kind="ExternalOutput").ap()

    with tile.TileContext(nc) as tc:
        with (
            tc.tile_pool(name="consts", bufs=1) as consts,
            tc.tile_pool(name="big", bufs=1) as big,
            tc.tile_pool(name="small", bufs=4) as small,
            tc.tile_pool(name="pt", bufs=3) as ptp,
        ):
            # ---- input x first (GN stats are the critical path) ----
            x_sb, xn_sb, res_sb = [], [], []
            for ct in range(2):
                t = big.tile([128, HW], F32, tag=f"x{ct}", name=f"x{ct}")
                for a, b_ in _nchunks(HW, 1152):
                    nc.sync.dma_start(t[:, a:b_], x_d[ct * 128 : (ct + 1) * 128, a:b_])
                x_sb.append(t)
                xn_sb.append(big.tile([128, HW], mm_dt, tag=f"xn{ct}", name=f"xn{ct}"))

            # ---- constant / weight loads ----
            gind_sb = consts.tile([128, 32], F32, tag="gind", name="gind")
            nc.sync.dma_start(gind_sb[:], gind_d[:])
            gbc_sb = consts.tile([16, C], F32, tag="gbc", name="gbc")
            nc.sync.dma_start(gbc_sb[:], gbc_d[:])
            gnp_sb = []
            for ct in range(2):
                t = consts.tile([128, 2], F32, tag=f"gnp{ct}", name=f"gnp{ct}")
                nc.sync.dma_start(t[:], gnp_d[ct * 128 : (ct + 1) * 128, :])
                gnp_sb.append(t)
            w_sb = {}
            for name, d in (("wq", wq_d), ("wk", wk_d), ("wv", wv_d)):
                for kc in range(2):
                    tf = consts.tile([128, 2 * HD], F32, tag=f"{name}{kc}f", name=f"{name}{kc}f")
                    nc.sync.dma_start(tf[:], d[kc * 128 : (kc + 1) * 128, :])
                    t = consts.tile([128, 2 * HD], mm_dt, tag=f"{name}{kc}", name=f"{name}{kc}")
                    nc.vector.tensor_copy(t[:], tf[:])
                    w_sb[name, kc] = t
            wof = consts.tile([128, C], F32, tag="wof", name="wof")
            nc.sync.dma_start(wof[:], wo_d[:])
            wo_sb = consts.tile([128, C], mm_dt, tag="wo", name="wo")
            nc.vector.tensor_copy(wo_sb[:], wof[:])
            for ct in range(2):
                r = big.tile([128, HW], F32, tag=f"res{ct}", name=f"res{ct}")
                nc.sync.dma_start(r[:], res_d[ct * 128 : (ct + 1) * 128, :])
                res_sb.append(r)

            # ---- GroupNorm ----
            # per-channel sums / sum-of-squares -> stats[:, (s0,q0,s1,q1)]
            # computed on ACT (activation accumulate) to keep the DVE free;
            # the activation outputs go to xn_sb as scratch (overwritten below)
            stats = small.tile([128, 4], F32, tag="stats", name="stats")
            for ct in range(2):
                nc.scalar.activation(
                    xn_sb[ct][:], x_sb[ct][:], AF.Copy,
                    accum_out=stats[:, 2 * ct : 2 * ct + 1],
                )
                nc.scalar.activation(
                    xn_sb[ct][:], x_sb[ct][:], AF.Square,
                    accum_out=stats[:, 2 * ct + 1 : 2 * ct + 2],
                )
            with tc.tile_pool(name="ps_gn", bufs=2, space=bass.MemorySpace.PSUM) as ps_gn:
                # accumulate both channel tiles' per-group (sum, sumsq) into [16, 2]
                g_ps = ps_gn.tile([16, 2], F32, tag="g", name="g")
                nc.tensor.matmul(g_ps[:], gind_sb[:, 0:16], stats[:, 0:2], start=True, stop=False)
                nc.tensor.matmul(g_ps[:], gind_sb[:, 16:32], stats[:, 2:4], start=False, stop=True)
                mall = small.tile([16, 2], F32, tag="mall", name="mall")
                nc.vector.tensor_scalar_mul(mall[:], g_ps[:], 1.0 / (GC * HW))
                msq = small.tile([16, 1], F32, tag="msq", name="msq")
                nc.vector.tensor_tensor(msq[:], mall[:, 0:1], mall[:, 0:1], op=OP.mult)
                ve = small.tile([16, 1], F32, tag="ve", name="ve")
                nc.vector.tensor_tensor(ve[:], mall[:, 1:2], msq[:], op=OP.subtract)
                ve2 = small.tile([16, 1], F32, tag="ve2", name="ve2")
                nc.vector.tensor_scalar_add(ve2[:], ve[:], EPS)
                # rstd = exp(-0.5 * ln(v)) — keeps ACT in the natural_log_exp
                # table set (shared with attention's Exp: one table load)
                lg = small.tile([16, 1], F32, tag="lg", name="lg")
                nc.scalar.activation(lg[:], ve2[:], AF.Ln)
                # gvals [16, 2] = per-group (mean, rstd)
                gvals = small.tile([16, 2], F32, tag="gvals", name="gvals")
                nc.vector.tensor_copy(gvals[:, 0:1], mall[:, 0:1])
                nc.scalar.activation(gvals[:, 1:2], lg[:], AF.Exp, scale=-0.5)
                for ct in range(2):
                    cv = ps_gn.tile([128, 2], F32, tag="cv", name="cv")
                    nc.tensor.matmul(
                        cv[:], gbc_sb[:, ct * 128 : (ct + 1) * 128], gvals[:],
                        start=True, stop=True,
                    )
                    scale_t = small.tile([128, 1], F32, tag="scale", name="scale")
                    nc.vector.tensor_tensor(scale_t[:], gnp_sb[ct][:, 0:1], cv[:, 1:2], op=OP.mult)
                    tb = small.tile([128, 1], F32, tag="tb", name="tb")
                    nc.vector.tensor_tensor(tb[:], cv[:, 0:1], scale_t[:], op=OP.mult)
                    bias_t = small.tile([128, 1], F32, tag="bias", name="bias")
                    nc.vector.tensor_tensor(bias_t[:], gnp_sb[ct][:, 1:2], tb[:], op=OP.subtract)
                    # one normalize-apply per engine so they run concurrently
                    if ct == 0:
                        nc.vector.tensor_scalar(
                            xn_sb[ct][:], x_sb[ct][:], scale_t[:], bias_t[:],
                            op0=OP.mult, op1=OP.add,
                        )
                    else:
                        nc.scalar.activation(
                            xn_sb[ct][:], x_sb[ct][:], AF.Identity,
                            bias=bias_t[:], scale=scale_t[:],
                        )

            # ---- QKV projections ----
            q_sb = big.tile([128, HW], mm_dt, tag="q", name="q")
            k_sb = big.tile([128, HW], mm_dt, tag="k", name="k")
            # v^T in fp8, laid out [128 spatial, NJ subtiles of VST]: subtile jc
            # holds chunk jc's [64 v-channels + ones column(s)].  Pair 2p,2p+1
            # forms the DoubleRow K=256 stationary operand.
            vt_sb, vt_v = [], []
            for h in range(2):
                t = big.tile([128, NJ * VST], F8, tag=f"vt{h}", name=f"vt{h}")
                nc.gpsimd.memset(t[:], 1.0)  # ones columns (and padding)
                vt_sb.append(t)
                vt_v.append(t[:].rearrange("p (j c) -> p j c", c=VST))

            with tc.tile_pool(name="ps_qkv", bufs=2, space=bass.MemorySpace.PSUM) as ps_qkv:
                # v^T first: its DVE casts then overlap the q/k matmuls, and
                # q/k (which attention needs first) are ready right at the boundary
                for half in range(2):
                    vps = ps_qkv.tile([128, HALF], F32, tag="qkv", name="qkv")
                    for j9 in range(9):
                        jc = half * 9 + j9
                        for kc in range(2):
                            nc.tensor.matmul(
                                vps[:, j9 * 128 : (j9 + 1) * 128],
                                xn_sb[kc][:, jc * JC : (jc + 1) * JC],
                                w_sb["wv", kc][:],
                                start=(kc == 0), stop=(kc == 1),
                            )
                    vps3 = vps[:].rearrange("p (j c) -> p j c", c=128)
                    for h in range(2):
                        nc.vector.tensor_copy(
                            vt_v[h][:, half * 9 : (half + 1) * 9, 0:HD],
                            vps3[:, :, h * HD : (h + 1) * HD],
                        )
                for dst, wname in ((k_sb, "wk"), (q_sb, "wq")):
                    for half in range(2):
                        ps = ps_qkv.tile([128, HALF], F32, tag="qkv", name="qkv")
                        for kc in range(2):
                            for n0, n1 in _nchunks(HALF):
                                nc.tensor.matmul(
                                    ps[:, n0:n1],
                                    w_sb[wname, kc][:],
                                    xn_sb[kc][:, half * HALF + n0 : half * HALF + n1],
                                    start=(kc == 0), stop=(kc == 1),
                                )
                        nc.vector.tensor_copy(dst[:, half * HALF : (half + 1) * HALF], ps[:])

            # ---- attention + pipelined output projection ----
            with (
                tc.tile_pool(name="ps_att", bufs=1, space=bass.MemorySpace.PSUM) as ps_att,
                tc.tile_pool(name="ps_out", bufs=2, space=bass.MemorySpace.PSUM) as ps_out,
            ):
                pending = None  # closure emitting previous iblk's output projection

                def emit_outproj(i0, i1, ho):
                    blk = i1 - i0
                    for mt in range(2):
                        yp = ps_out.tile([128, blk], F32, tag="yp", name="yp")
                        nc.tensor.matmul(
                            yp[:], wo_sb[:, mt * 128 : (mt + 1) * 128], ho[:],
                            start=True, stop=True,
                        )
                        yo = small.tile([128, blk], F32, tag="yo", name="yo")
                        nc.vector.tensor_tensor(
                            yo[:], yp[:], res_sb[mt][:, i0:i1], op=OP.add,
                        )
                        nc.sync.dma_start(y_d[mt * 128 : (mt + 1) * 128, i0:i1], yo[:])

                for i0, i1 in IBLKS:
                    blk = i1 - i0
                    # u[h]: [65, blk] accumulator (64 channels + denominator row)
                    u = [
                        ps_att.tile([PVM, SALIGN], F32, tag=f"u{h}", name=f"u{h}", bufs=1)
                        for h in range(2)
                    ]

                    def emit_s(jc):
                        # S^T chunk for both heads, row-tiled (concurrent on PE).
                        # h0/h1 outputs land in different psum banks.
                        st = ps_att.tile([128, 2 * SALIGN], F32, tag="s", name="s", bufs=2)
                        for h in range(2):
                            nc.tensor.matmul(
                                st[:, h * SALIGN : h * SALIGN + blk],
                                k_sb[h * HD : (h + 1) * HD, jc * JC : (jc + 1) * JC],
                                q_sb[h * HD : (h + 1) * HD, i0:i1],
                                start=True, stop=True,
                            )
                        return st

                    def emit_exp(jc, st, pair_v, s):
                        # pt8[:, h, s, :] = exp(st[:, h, :]/16) as fp8e4m3
                        src = st[:].rearrange("p (h x) -> p h x", h=2)[:, :, 0:blk]
                        dst = pair_v[:, :, s, 0:blk]
                        if jc % 9 in ACT_JC:
                            nc.scalar.activation(dst, src, AF.Exp, scale=1.0 / 16.0)
                        else:
                            nc.vector.tensor_scalar(
                                dst.bitcast(U8), src, EXPA, EXPB,
                                op0=OP.mult, op1=OP.add,
                            )

                    def emit_pv(pp, pair_v):
                        # DoubleRow fp8: contracts both chunks of the pair (K=256)
                        for h in range(2):
                            nc.tensor.matmul(
                                u[h][:, 0:blk],
                                vt_v[h][:, 2 * pp : 2 * pp + 2, 0:PVM],
                                pair_v[:, h, :, 0:blk],
                                start=(pp == 0), stop=(pp == NJP - 1),
                                perf_mode=PM.DoubleRow,
                            )

                    prev_pair = None
                    for pp in range(NJP):
                        pt = ptp.tile([128, 4 * blk], F8, tag="pt8", name="pt8")
                        pair_v = pt[:].rearrange("p (h s x) -> p h s x", h=2, s=2)
                        for s in range(2):
                            jc = 2 * pp + s
                            st = emit_s(jc)
                            emit_exp(jc, st, pair_v, s)
                        if pp == 1 and pending is not None:
                            pending()
                            pending = None
                        if pp > 0:
                            emit_pv(pp - 1, prev_pair)
                        prev_pair = pair_v
                    emit_pv(NJP - 1, prev_pair)

                    ho = ptp.tile([128, blk], mm_dt, tag="ho", name="ho", bufs=2)
                    for h in range(2):
                        # plain reciprocal (no Newton refine): softmax denominator
                        # tolerance is loose relative to the fp8 weights
                        rcp = small.tile([1, blk], F32, tag="rcp", name="rcp")
                        nc.vector.reciprocal(rcp[:], u[h][HD : HD + 1, 0:blk])
                        rb = small.tile([HD, blk], F32, tag="rb", name="rb")
                        nc.gpsimd.partition_broadcast(rb[:], rcp[:])
                        nc.vector.tensor_tensor(
                            ho[h * HD : (h + 1) * HD, :],
                            u[h][0:HD, 0:blk], rb[:], op=OP.mult,
                        )
                    pending = (lambda a=i0, b_=i1, t=ho: emit_outproj(a, b_, t))
                pending()

    nc.compile()
    return nc


def _consts():
    # gind[:, 0:16]: tile-0 channel -> group one-hot; [:, 16:32]: tile-1 channel -> group
    gind = np.zeros((128, 32), np.float32)
    for c in range(128):
        gind[c, c // GC] = 1.0
        gind[c, 16 + 8 + c // GC] = 1.0
    gbc = np.zeros((16, C), np.float32)
    for c in range(C):
        gbc[c // GC, c] = 1.0
    return gind, gbc


def make_in_maps(x, gn_weight, gn_bias, qkv_w, out_w, out_b):
    x = np.asarray(x, np.float32)
    qkv_w = np.asarray(qkv_w, np.float32)
    out_w = np.asarray(out_w, np.float32)
    out_b = np.asarray(out_b, np.float32)
    gn_weight = np.asarray(gn_weight, np.float32)
    gn_bias = np.asarray(gn_bias, np.float32)
    xr = np.ascontiguousarray(x.reshape(B, C, HW))
    gind, gbc = _consts()
    gnp = np.ascontiguousarray(np.stack([gn_weight, gn_bias], axis=1))
    in_maps = []
    for core in range(NCORES):
        b, hp = divmod(core, 2)
        heads = (2 * hp, 2 * hp + 1)
        qs = np.concatenate([qkv_w[n * 192 : n * 192 + 64] for n in heads], 0)
        ks = np.concatenate([qkv_w[n * 192 + 64 : n * 192 + 128] for n in heads], 0)
        vs = np.concatenate([qkv_w[n * 192 + 128 : n * 192 + 192] for n in heads], 0)
        res = xr[b] + out_b[:, None] if hp == 0 else np.zeros_like(xr[b])
        in_maps.append({
            "x": xr[b],
            "res": np.ascontiguousarray(res, np.float32),
            "wq": np.ascontiguousarray(qs.T),
            "wk": np.ascontiguousarray(ks.T),
            "wv": np.ascontiguousarray(vs.T),
            "wo": np.ascontiguousarray(out_w[:, hp * 128 : (hp + 1) * 128].T),
            "gnp": gnp,
            "gind": gind,
            "gbc": gbc,
        })
    return in_maps


_NC_CACHE = {}


def get_nc(mm_dt=BF16):
    key = str(mm_dt)
    if key not in _NC_CACHE:
        _NC_CACHE[key] = _build(mm_dt)
    return _NC_CACHE[key]


def kernel(x, gn_weight, gn_bias, qkv_w, out_w, out_b):
    nc = get_nc(BF16)
    in_maps = make_in_maps(x, gn_weight, gn_bias, qkv_w, out_w, out_b)
    res = bass_utils.run_bass_kernel_spmd(nc, in_maps, core_ids=list(range(NCORES)))
    y = np.empty((B, C, HW), np.float32)
    for b in range(B):
        y[b] = res.results[2 * b]["y"] + res.results[2 * b + 1]["y"]
    return y.reshape(B, C, H, W)


# revision 13
# speedup vs baseline: 1.3531x; 1.3531x over previous
"""Trainium2 Bass kernel for spatial attention (GroupNorm + QKV + softmax attention
+ output projection + residual), distributed over 8 NeuronCores.

Sharding: core = 2*b + hp handles image b (of 4) and head pair hp (heads 2hp, 2hp+1).
Each core computes GroupNorm(x[b]), its heads' q/k/v, full spatial attention for its
two heads, and per-head UNNORMALIZED partial output projections.  The softmax
denominators ship back with the partials; the host divides, sums the four partials
per image, and adds the residual + bias (cheap [C, HW] numpy ops, off the device
critical path).

Perf notes (v4):
- Scores bf16, row-tiled: both heads' S^T matmuls run concurrently on the PE.
- softmax exp split across engines: 5 of 9 key chunks on ACT (fp8 output),
  4 of 9 on the DVE via a Schraudolph bit-trick (uint8 = trunc(s*A+B)
  reinterpreted as fp8e4m3) — the 10.6M-element exp load is the bottleneck.
- PV runs as fp8 DoubleRow matmuls (two key chunks = K=256 per instruction),
  halving PV stream time; v^T carries a 65th all-ones column so the softmax
  denominator accumulates in the same matmul.
- No on-device normalize/residual: u (incl. denominator row) is copied psum->sbuf
  bf16, projected per head, and DMA'd out; host does the divides.
- v and q-half1 projections are interleaved INTO the first two attention iblks
  (sharing the ps_out psum tag) so the PE never idles while exp catches up.
"""

import math

import numpy as np

import concourse.bass as bass
import concourse.bacc as bacc
import concourse.tile as tile
from concourse import mybir
from concourse import bass_utils
from concourse.alu_op_type import AluOpType

B, C, H, W = 4, 256, 48, 48
HW = H * W  # 2304
NH, HD = 4, 64
G, GC = 16, 16  # 16 groups x 16 channels
EPS = 1e-5
NCORES = 8
JC = 128  # j (key spatial) chunk
NJ = HW // JC  # 18
NJP = NJ // 2  # 9 key-chunk pairs (DoubleRow K=256)
IBLKS = [(0, 512), (512, 1024), (1024, 1536), (1536, 2048), (2048, 2304)]
HALF = HW // 2  # 1152
QSPLIT = 1024  # q_sb split point (iblk-aligned)
PVM = HD + 1  # 65: 64 v channels + denominator ones row
VST = 80  # fp8 v^T subtile stride (16-byte aligned, >= PVM)
SALIGN = 512

F32 = mybir.dt.float32
BF16 = mybir.dt.bfloat16
F8 = mybir.dt.float8e4
U8 = mybir.dt.uint8
AX = mybir.AxisListType.X
AF = mybir.ActivationFunctionType
OP = AluOpType
PM = mybir.MatmulPerfMode

# Schraudolph exp into fp8e4m3 bit space: bits = trunc(s*EXPA + EXPB),
# value(bits) ~= exp(s/16).  EXPB tuned numerically for minimax rel err (~7%)
# assuming truncation on the DVE float->uint8 convert.
EXPA = 8.0 * math.log2(math.e) / 16.0
EXPB = 56.13
ACT_JC = (0, 2, 4, 6, 8)  # key chunks (mod 9) whose exp runs on ACT


def _nchunks(size, step=512):
    # PSUM-bank-aligned chunks: a matmul output may not cross a 512-fp32 bank boundary
    return [(a, min(a + step, size)) for a in range(0, size, step)]


def _build(mm_dt=BF16):
    nc = bacc.Bacc("TRN2", target_bir_lowering=False, debug=False, enable_asserts=False)

    x_d = nc.dram_tensor("x", [C, HW], F32, kind="ExternalInput").ap()
    wq_d = nc.dram_tensor("wq", [C, 2 * HD], F32, kind="ExternalInput").ap()
    wk_d = nc.dram_tensor("wk", [C, 2 * HD], F32, kind="ExternalInput").ap()
    wv_d = nc.dram_tensor("wv", [C, 2 * HD], F32, kind="ExternalInput").ap()
    wo_d = nc.dram_tensor("wo", [2 * HD, C], F32, kind="ExternalInput").ap()
    gnp_d = nc.dram_tensor("gnp", [C, 2], F32, kind="ExternalInput").ap()
    gind_d = nc.dram_tensor("gind", [128, 32], F32, kind="ExternalInput").ap()
    gbc_d = nc.dram_tensor("gbc", [16, C], F32, kind="ExternalInput").ap()
    y_d = [
        nc.dram_tensor(f"y{h}", [C, HW], F32, kind="ExternalOutput").ap()
        for h in range(2)
    ]
    dn_d = nc.dram_tensor("dns", [2, HW], BF16, kind="ExternalOutput").ap()

    with tile.TileContext(nc) as tc:
        with (
            tc.tile_pool(name="consts", bufs=1) as consts,
            tc.tile_pool(name="big", bufs=1) as big,
            tc.tile_pool(name="small", bufs=4) as small,
            tc.tile_pool(name="pt", bufs=3) as ptp,
        ):
            # ---- input x first (GN stats are the critical path) ----
            x_sb, xn_sb = [], []
            for ct in range(2):
                t = big.tile([128, HW], F32, tag=f"x{ct}", name=f"x{ct}")
                nc.sync.dma_start(t[:], x_d[ct * 128 : (ct + 1) * 128, :])
                x_sb.append(t)
                xn_sb.append(big.tile([128, HW], mm_dt, tag=f"xn{ct}", name=f"xn{ct}"))

            # ---- constant / weight loads ----
            gind_sb = consts.tile([128, 32], F32, tag="gind", name="gind")
            nc.sync.dma_start(gind_sb[:], gind_d[:])
            gbc_sb = consts.tile([16, C], F32, tag="gbc", name="gbc")
            nc.sync.dma_start(gbc_sb[:], gbc_d[:])
            gnp_sb = []
            for ct in range(2):
                t = consts.tile([128, 2], F32, tag=f"gnp{ct}", name=f"gnp{ct}")
                nc.sync.dma_start(t[:], gnp_d[ct * 128 : (ct + 1) * 128, :])
                gnp_sb.append(t)
            w_sb = {}
            for name, d in (("wk", wk_d), ("wq", wq_d), ("wv", wv_d)):
                for kc in range(2):
                    tf = consts.tile([128, 2 * HD], F32, tag=f"{name}{kc}f", name=f"{name}{kc}f")
                    nc.sync.dma_start(tf[:], d[kc * 128 : (kc + 1) * 128, :])
                    t = consts.tile([128, 2 * HD], mm_dt, tag=f"{name}{kc}", name=f"{name}{kc}")
                    nc.vector.tensor_copy(t[:], tf[:])
                    w_sb[name, kc] = t
            # wo rows 0:64 (head 0) in place; rows 64:128 (head 1) also loaded at
            # base partition 0 so both heads' K=64 projections can stream from
            # partitions 0-63 (rhs = ho tile lives there).
            wof = consts.tile([128, C], F32, tag="wof", name="wof")
            nc.sync.dma_start(wof[:], wo_d[:])
            wo2f = consts.tile([64, C], F32, tag="wo2f", name="wo2f")
            nc.sync.dma_start(wo2f[:], wo_d[64:128, :])
            wo_sb = consts.tile([128, C], mm_dt, tag="wo", name="wo")
            nc.vector.tensor_copy(wo_sb[:], wof[:])
            wo2_sb = consts.tile([64, C], mm_dt, tag="wo2", name="wo2")
            nc.vector.tensor_copy(wo2_sb[:], wo2f[:])
            wo_h = {0: wo_sb, 1: wo2_sb}

            # ---- GroupNorm ----
            # per-channel sums on ACT (activation accumulate), sum-of-squares on
            # DVE (scalar_tensor_tensor accumulate) -> run concurrently.
            # activation output goes to xn_sb as scratch (overwritten below).
            stats = small.tile([128, 4], F32, tag="stats", name="stats")
            for ct in range(2):
                nc.scalar.activation(
                    xn_sb[ct][:], x_sb[ct][:], AF.Copy,
                    accum_out=stats[:, 2 * ct : 2 * ct + 1],
                )
                nc.vector.scalar_tensor_tensor(
                    xn_sb[ct][:], x_sb[ct][:], 1.0, x_sb[ct][:],
                    op0=OP.mult, op1=OP.mult,
                    accum_out=stats[:, 2 * ct + 1 : 2 * ct + 2],
                )
            with tc.tile_pool(name="ps_gn", bufs=2, space=bass.MemorySpace.PSUM) as ps_gn:
                # accumulate both channel tiles' per-group (sum, sumsq) into [16, 2]
                g_ps = ps_gn.tile([16, 2], F32, tag="g", name="g")
                nc.tensor.matmul(g_ps[:], gind_sb[:, 0:16], stats[:, 0:2], start=True, stop=False)
                nc.tensor.matmul(g_ps[:], gind_sb[:, 16:32], stats[:, 2:4], start=False, stop=True)
                mall = small.tile([16, 2], F32, tag="mall", name="mall")
                nc.vector.tensor_scalar_mul(mall[:], g_ps[:], 1.0 / (GC * HW))
                msq = small.tile([16, 1], F32, tag="msq", name="msq")
                nc.vector.tensor_tensor(msq[:], mall[:, 0:1], mall[:, 0:1], op=OP.mult)
                ve = small.tile([16, 1], F32, tag="ve", name="ve")
                nc.vector.tensor_tensor(ve[:], mall[:, 1:2], msq[:], op=OP.subtract)
                ve2 = small.tile([16, 1], F32, tag="ve2", name="ve2")
                nc.vector.tensor_scalar_add(ve2[:], ve[:], EPS)
                # rstd = exp(-0.5 * ln(v)) — keeps ACT in the natural_log_exp
                # table set (shared with attention's Exp: one table load)
                lg = small.tile([16, 1], F32, tag="lg", name="lg")
                nc.scalar.activation(lg[:], ve2[:], AF.Ln)
                # gvals [16, 2] = per-group (mean, rstd)
                gvals = small.tile([16, 2], F32, tag="gvals", name="gvals")
                nc.vector.tensor_copy(gvals[:, 0:1], mall[:, 0:1])
                nc.scalar.activation(gvals[:, 1:2], lg[:], AF.Exp, scale=-0.5)
                for ct in range(2):
                    cv = ps_gn.tile([128, 2], F32, tag="cv", name="cv")
                    nc.tensor.matmul(
                        cv[:], gbc_sb[:, ct * 128 : (ct + 1) * 128], gvals[:],
                        start=True, stop=True,
                    )
                    scale_t = small.tile([128, 1], F32, tag="scale", name="scale")
                    nc.vector.tensor_tensor(scale_t[:], gnp_sb[ct][:, 0:1], cv[:, 1:2], op=OP.mult)
                    tb = small.tile([128, 1], F32, tag="tb", name="tb")
                    nc.vector.tensor_tensor(tb[:], cv[:, 0:1], scale_t[:], op=OP.mult)
                    bias_t = small.tile([128, 1], F32, tag="bias", name="bias")
                    nc.vector.tensor_tensor(bias_t[:], gnp_sb[ct][:, 1:2], tb[:], op=OP.subtract)
                    # one normalize-apply per engine so they run concurrently
                    if ct == 0:
                        nc.vector.tensor_scalar(
                            xn_sb[ct][:], x_sb[ct][:], scale_t[:], bias_t[:],
                            op0=OP.mult, op1=OP.add,
                        )
                    else:
                        nc.scalar.activation(
                            xn_sb[ct][:], x_sb[ct][:], AF.Identity,
                            bias=bias_t[:], scale=scale_t[:],
                        )

            # ---- k and q[0:QSPLIT] projections (needed before attention) ----
            # q is split into two tiles at QSPLIT so the q-half1 projection can
            # be interleaved into the attention loop without false WAR hazards.
            k_sb = big.tile([128, HW], mm_dt, tag="k", name="k")
            q_a = big.tile([128, QSPLIT], mm_dt, tag="qa", name="qa")
            q_b = big.tile([128, HW - QSPLIT], mm_dt, tag="qb", name="qb")

            def q_ap(i0, i1):
                if i1 <= QSPLIT:
                    return q_a[:, i0:i1]
                assert i0 >= QSPLIT
                return q_b[:, i0 - QSPLIT : i1 - QSPLIT]

            # v^T in fp8, laid out [128 spatial, NJ subtiles of VST]: subtile jc
            # holds chunk jc's [64 v-channels + ones column(s)].  Pair 2p,2p+1
            # forms the DoubleRow K=256 stationary operand.
            vt_sb, vt_v = [], []
            for h in range(2):
                t = big.tile([128, NJ * VST], F8, tag=f"vt{h}", name=f"vt{h}")
                nc.gpsimd.memset(t[:], 1.0)  # ones columns (and padding)
                vt_sb.append(t)
                vt_v.append(t[:].rearrange("p (j c) -> p j c", c=VST))

            with tc.tile_pool(name="ps_qkv", bufs=2, space=bass.MemorySpace.PSUM) as ps_qkv:
                for half in range(2):
                    ps = ps_qkv.tile([128, HALF], F32, tag="qkv", name="qkv")
                    for kc in range(2):
                        for n0, n1 in _nchunks(HALF):
                            nc.tensor.matmul(
                                ps[:, n0:n1],
                                w_sb["wk", kc][:],
                                xn_sb[kc][:, half * HALF + n0 : half * HALF + n1],
                                start=(kc == 0), stop=(kc == 1),
                            )
                    nc.vector.tensor_copy(k_sb[:, half * HALF : (half + 1) * HALF], ps[:])
                # q half 0 -> q_a[0:1024] + q_b[0:128]
                ps = ps_qkv.tile([128, HALF], F32, tag="qkv", name="qkv")
                for kc in range(2):
                    for n0, n1 in _nchunks(HALF):
                        nc.tensor.matmul(
                            ps[:, n0:n1],
                            w_sb["wq", kc][:],
                            xn_sb[kc][:, n0:n1],
                            start=(kc == 0), stop=(kc == 1),
                        )
                nc.vector.tensor_copy(q_a[:], ps[:, 0:QSPLIT])
                nc.vector.tensor_copy(q_b[:, 0 : HALF - QSPLIT], ps[:, QSPLIT:HALF])

            # ---- attention, with v / q-half1 / output projection woven in ----
            with (
                tc.tile_pool(name="ps_att", bufs=1, space=bass.MemorySpace.PSUM) as ps_att,
                tc.tile_pool(name="ps_out", bufs=2, space=bass.MemorySpace.PSUM) as ps_out,
            ):
                def vjob(g):
                    # project v for chunks 4g..4g+3 (last group: 2 chunks) and
                    # cast into the fp8 v^T tiles
                    chunks = list(range(4 * g, min(4 * g + 4, NJ)))
                    w = len(chunks) * JC
                    vps = ps_out.tile([128, 512], F32, tag="yp", name="vps")
                    for ci, jc in enumerate(chunks):
                        for kc in range(2):
                            nc.tensor.matmul(
                                vps[:, ci * JC : (ci + 1) * JC],
                                xn_sb[kc][:, jc * JC : (jc + 1) * JC],
                                w_sb["wv", kc][:],
                                start=(kc == 0), stop=(kc == 1),
                            )
                    vps3 = vps[:, 0:w].rearrange("p (j c) -> p j c", c=128)
                    for h in range(2):
                        nc.vector.tensor_copy(
                            vt_v[h][:, chunks[0] : chunks[0] + len(chunks), 0:HD],
                            vps3[:, :, h * HD : (h + 1) * HD],
                        )

                def qjob(n0, n1):
                    # q half 1 chunk [HALF+n0, HALF+n1) -> q_b
                    ps = ps_out.tile([128, 512], F32, tag="yp", name="qps")
                    for kc in range(2):
                        nc.tensor.matmul(
                            ps[:, 0 : n1 - n0],
                            w_sb["wq", kc][:],
                            xn_sb[kc][:, HALF + n0 : HALF + n1],
                            start=(kc == 0), stop=(kc == 1),
                        )
                    nc.vector.tensor_copy(
                        q_b[:, HALF - QSPLIT + n0 : HALF - QSPLIT + n1],
                        ps[:, 0 : n1 - n0],
                    )

                pending = None  # closure emitting previous iblk's epilogue

                def emit_epilogue(i0, i1, u):
                    blk = i1 - i0
                    for h in range(2):
                        # u (64 channels + denominator row) psum -> sbuf bf16
                        ho = ptp.tile([PVM, blk], mm_dt, tag=f"ho{h}", name=f"ho{h}", bufs=2)
                        nc.vector.tensor_copy(ho[:], u[h][:, 0:blk])
                        nc.sync.dma_start(dn_d[h : h + 1, i0:i1], ho[HD : HD + 1, :])
                        for mt in range(2):
                            yp = ps_out.tile([128, blk], F32, tag="yp", name="yp")
                            nc.tensor.matmul(
                                yp[:],
                                wo_h[h][0:64, mt * 128 : (mt + 1) * 128],
                                ho[0:HD, :],
                                start=True, stop=True,
                            )
                            yo = small.tile([128, blk], F32, tag="yo", name="yo")
                            if (h + mt) % 2 == 0:
                                nc.vector.tensor_copy(yo[:], yp[:])
                            else:
                                nc.scalar.copy(yo[:], yp[:])
                            nc.sync.dma_start(
                                y_d[h][mt * 128 : (mt + 1) * 128, i0:i1], yo[:],
                            )

                for ib, (i0, i1) in enumerate(IBLKS):
                    blk = i1 - i0
                    # u[h]: [65, blk] accumulator (64 channels + denominator row)
                    u = [
                        ps_att.tile([PVM, SALIGN], F32, tag=f"u{h}", name=f"u{h}", bufs=1)
                        for h in range(2)
                    ]

                    def emit_s(jc):
                        # S^T chunk for both heads, row-tiled (concurrent on PE).
                        # h0/h1 outputs land in different psum banks.
                        st = ps_att.tile([128, 2 * SALIGN], F32, tag="s", name="s", bufs=2)
                        for h in range(2):
                            nc.tensor.matmul(
                                st[:, h * SALIGN : h * SALIGN + blk],
                                k_sb[h * HD : (h + 1) * HD, jc * JC : (jc + 1) * JC],
                                q_ap(i0, i1)[h * HD : (h + 1) * HD, :],
                                start=True, stop=True,
                            )
                        return st

                    def emit_exp(jc, st, pair_v, s):
                        # pt8[:, h, s, :] = exp(st[:, h, :]/16) as fp8e4m3
                        src = st[:].rearrange("p (h x) -> p h x", h=2)[:, :, 0:blk]
                        dst = pair_v[:, :, s, 0:blk]
                        if jc % 9 in ACT_JC:
                            nc.scalar.activation(dst, src, AF.Exp, scale=1.0 / 16.0)
                        else:
                            nc.vector.tensor_scalar(
                                dst.bitcast(U8), src, EXPA, EXPB,
                                op0=OP.mult, op1=OP.add,
                            )

                    def emit_pv(pp, pair_v):
                        # DoubleRow fp8: contracts both chunks of the pair (K=256)
                        for h in range(2):
                            nc.tensor.matmul(
                                u[h][:, 0:blk],
                                vt_v[h][:, 2 * pp : 2 * pp + 2, 0:PVM],
                                pair_v[:, h, :, 0:blk],
                                start=(pp == 0), stop=(pp == NJP - 1),
                                perf_mode=PM.DoubleRow,
                            )

                    prev_pair = None
                    for pp in range(NJP):
                        pt = ptp.tile([128, 4 * blk], F8, tag="pt8", name="pt8")
                        pair_v = pt[:].rearrange("p (h s x) -> p h s x", h=2, s=2)
                        for s in range(2):
                            jc = 2 * pp + s
                            st = emit_s(jc)
                            emit_exp(jc, st, pair_v, s)
                        if ib == 0 and pp < 5:
                            vjob(pp)
                        if ib == 1 and pp < 3:
                            qjob(*(_nchunks(HALF)[pp]))
                        if pp == 1 and pending is not None:
                            pending()
                            pending = None
                        if pp > 0:
                            emit_pv(pp - 1, prev_pair)
                        prev_pair = pair_v
                    emit_pv(NJP - 1, prev_pair)
                    pending = (lambda a=i0, b_=i1, t=u: emit_epilogue(a, b_, t))
                pending()

    nc.compile()
    return nc


def _consts():
    # gind[:, 0:16]: tile-0 channel -> group one-hot; [:, 16:32]: tile-1 channel -> group
    gind = np.zeros((128, 32), np.float32)
    for c in range(128):
        gind[c, c // GC] = 1.0
        gind[c, 16 + 8 + c // GC] = 1.0
    gbc = np.zeros((16, C), np.float32)
    for c in range(C):
        gbc[c // GC, c] = 1.0
    return gind, gbc


def make_in_maps(x, gn_weight, gn_bias, qkv_w, out_w, out_b):
    x = np.asarray(x, np.float32)
    qkv_w = np.asarray(qkv_w, np.float32)
    out_w = np.asarray(out_w, np.float32)
    gn_weight = np.asarray(gn_weight, np.float32)
    gn_bias = np.asarray(gn_bias, np.float32)
    xr = np.ascontiguousarray(x.reshape(B, C, HW))
    gind, gbc = _consts()
    gnp = np.ascontiguousarray(np.stack([gn_weight, gn_bias], axis=1))
    in_maps = []
    for core in range(NCORES):
        b, hp = divmod(core, 2)
        heads = (2 * hp, 2 * hp + 1)
        qs = np.concatenate([qkv_w[n * 192 : n * 192 + 64] for n in heads], 0)
        ks = np.concatenate([qkv_w[n * 192 + 64 : n * 192 + 128] for n in heads], 0)
        vs = np.concatenate([qkv_w[n * 192 + 128 : n * 192 + 192] for n in heads], 0)
        in_maps.append({
            "x": xr[b],
            "wq": np.ascontiguousarray(qs.T),
            "wk": np.ascontiguousarray(ks.T),
            "wv": np.ascontiguousarray(vs.T),
            "wo": np.ascontiguousarray(out_w[:, hp * 128 : (hp + 1) * 128].T),
            "gnp": gnp,
            "gind": gind,
            "gbc": gbc,
        })
    return in_maps


def gather(results, x, out_b):
    """Host-side: divide per-head partials by softmax denominators, sum, add
    residual + bias."""
    x = np.asarray(x, np.float32)
    out_b = np.asarray(out_b, np.float32)
    xr = x.reshape(B, C, HW)
    y = np.empty((B, C, HW), np.float32)
    for b in range(B):
        acc = xr[b] + out_b[:, None]
        for hp in range(2):
            r = results[2 * b + hp]
            dns = np.asarray(r["dns"], np.float32)
            acc = acc + np.asarray(r["y0"], np.float32) / dns[0][None, :]
            acc = acc + np.asarray(r["y1"], np.float32) / dns[1][None, :]
        y[b] = acc
    return y.reshape(B, C, H, W)


_NC_CACHE = {}


def get_nc(mm_dt=BF16):
    key = str(mm_dt)
    if key not in _NC_CACHE:
        _NC_CACHE[key] = _build(mm_dt)
    return _NC_CACHE[key]


def kernel(x, gn_weight, gn_bias, qkv_w, out_w, out_b):
    nc = get_nc(BF16)
    in_maps = make_in_maps(x, gn_weight, gn_bias, qkv_w, out_w, out_b)
    res = bass_utils.run_bass_kernel_spmd(nc, in_maps, core_ids=list(range(NCORES)))
    return gather(res.results, x, out_b)


# revision 21
# speedup vs baseline: 1.3932x; 1.0297x over previous
"""Trainium2 Bass kernel for spatial attention (GroupNorm + QKV + softmax attention
+ output projection + residual), distributed over 8 NeuronCores.

Sharding: core = 2*b + hp handles image b (of 4) and head pair hp (heads 2hp, 2hp+1).
Each core computes GroupNorm(x[b]), its heads' q/k/v, full spatial attention for its
two heads, and per-head UNNORMALIZED partial output projections.  The softmax
denominators ship back with the partials; the host divides, sums the four partials
per image, and adds the residual + bias (cheap [C, HW] numpy ops, off the device
critical path).

Perf notes (v4):
- Scores bf16, row-tiled: both heads' S^T matmuls run concurrently on the PE.
- softmax exp split across engines: 5 of 9 key chunks on ACT (fp8 output),
  4 of 9 on the DVE via a Schraudolph bit-trick (uint8 = trunc(s*A+B)
  reinterpreted as fp8e4m3) — the 10.6M-element exp load is the bottleneck.
- PV runs as fp8 DoubleRow matmuls (two key chunks = K=256 per instruction),
  halving PV stream time; v^T carries a 65th all-ones column so the softmax
  denominator accumulates in the same matmul.
- No on-device normalize/residual: u (incl. denominator row) is copied psum->sbuf
  bf16, projected per head, and DMA'd out; host does the divides.
- v and q-half1 projections are interleaved INTO the first two attention iblks
  (sharing the ps_out psum tag) so the PE never idles while exp catches up.
"""

import math

import numpy as np

import concourse.bass as bass
import concourse.bacc as bacc
import concourse.tile as tile
from concourse import mybir
from concourse import bass_utils
from concourse.alu_op_type import AluOpType

B, C, H, W = 4, 256, 48, 48
HW = H * W  # 2304
NH, HD = 4, 64
G, GC = 16, 16  # 16 groups x 16 channels
EPS = 1e-5
NCORES = 8
JC = 128  # j (key spatial) chunk
NJ = HW // JC  # 18
NJP = NJ // 2  # 9 key-chunk pairs (DoubleRow K=256)
IBLKS = [(0, 512), (512, 1024), (1024, 1536), (1536, 2048), (2048, 2304)]
HALF = HW // 2  # 1152
QSPLIT = 1024  # q_sb split point (iblk-aligned)
PVM = HD + 1  # 65: 64 v channels + denominator ones row
VST = 80  # fp8 v^T subtile stride (16-byte aligned, >= PVM)
SALIGN = 512

F32 = mybir.dt.float32
BF16 = mybir.dt.bfloat16
F8 = mybir.dt.float8e4
U8 = mybir.dt.uint8
AX = mybir.AxisListType.X
AF = mybir.ActivationFunctionType
OP = AluOpType
PM = mybir.MatmulPerfMode

# Schraudolph exp into fp8e4m3 bit space: bits = trunc(s*EXPA + EXPB),
# value(bits) ~= exp(s/16).  EXPB tuned numerically for minimax rel err (~7%)
# assuming truncation on the DVE float->uint8 convert.
EXPA = 8.0 * math.log2(math.e) / 16.0
EXPB = 56.13


def _nchunks(size, step=512):
    # PSUM-bank-aligned chunks: a matmul output may not cross a 512-fp32 bank boundary
    return [(a, min(a + step, size)) for a in range(0, size, step)]


def _build(mm_dt=BF16):
    nc = bacc.Bacc("TRN2", target_bir_lowering=False, debug=False, enable_asserts=False)

    x_d = nc.dram_tensor("x", [C, HW], F32, kind="ExternalInput").ap()
    wq_d = nc.dram_tensor("wq", [C, 2 * HD], F32, kind="ExternalInput").ap()
    wk_d = nc.dram_tensor("wk", [C, 2 * HD], F32, kind="ExternalInput").ap()
    wv_d = nc.dram_tensor("wv", [C, 2 * HD], F32, kind="ExternalInput").ap()
    wo_d = nc.dram_tensor("wo", [2 * HD, C], F32, kind="ExternalInput").ap()
    gnp_d = nc.dram_tensor("gnp", [C, 2], F32, kind="ExternalInput").ap()
    gind_d = nc.dram_tensor("gind", [128, 32], F32, kind="ExternalInput").ap()
    gbc_d = nc.dram_tensor("gbc", [16, C], F32, kind="ExternalInput").ap()
    y_d = [
        nc.dram_tensor(f"y{h}", [C, HW], BF16, kind="ExternalOutput").ap()
        for h in range(2)
    ]
    dn_d = nc.dram_tensor("dns", [2, HW], BF16, kind="ExternalOutput").ap()

    with tile.TileContext(nc) as tc:
        with (
            tc.tile_pool(name="consts", bufs=1) as consts,
            tc.tile_pool(name="big", bufs=1) as big,
            tc.tile_pool(name="small", bufs=4) as small,
            tc.tile_pool(name="pt", bufs=3) as ptp,
        ):
            # ---- input x first (GN stats are the critical path) ----
            # halves go over both DMA queues (SP + ACT) in parallel
            x_sb, xn_sb = [], []
            for ct in range(2):
                t = big.tile([128, HW], F32, tag=f"x{ct}", name=f"x{ct}")
                nc.sync.dma_start(t[:, 0:HALF], x_d[ct * 128 : (ct + 1) * 128, 0:HALF])
                nc.scalar.dma_start(t[:, HALF:HW], x_d[ct * 128 : (ct + 1) * 128, HALF:HW])
                x_sb.append(t)
                xn_sb.append(big.tile([128, HW], mm_dt, tag=f"xn{ct}", name=f"xn{ct}"))

            # ---- constant / weight loads ----
            gind_sb = consts.tile([128, 32], F32, tag="gind", name="gind")
            nc.sync.dma_start(gind_sb[:], gind_d[:])
            gbc_sb = consts.tile([16, C], F32, tag="gbc", name="gbc")
            nc.sync.dma_start(gbc_sb[:], gbc_d[:])
            gnp_sb = []
            for ct in range(2):
                t = consts.tile([128, 2], F32, tag=f"gnp{ct}", name=f"gnp{ct}")
                nc.sync.dma_start(t[:], gnp_d[ct * 128 : (ct + 1) * 128, :])
                gnp_sb.append(t)
            w_sb = {}
            for name, d in (("wk", wk_d), ("wq", wq_d), ("wv", wv_d)):
                for kc in range(2):
                    tf = consts.tile([128, 2 * HD], F32, tag=f"{name}{kc}f", name=f"{name}{kc}f")
                    nc.sync.dma_start(tf[:], d[kc * 128 : (kc + 1) * 128, :])
                    t = consts.tile([128, 2 * HD], mm_dt, tag=f"{name}{kc}", name=f"{name}{kc}")
                    nc.vector.tensor_copy(t[:], tf[:])
                    w_sb[name, kc] = t
            # wo rows 0:64 (head 0) in place; rows 64:128 (head 1) also loaded at
            # base partition 0 so both heads' K=64 projections can stream from
            # partitions 0-63 (rhs = ho tile lives there).
            wof = consts.tile([128, C], F32, tag="wof", name="wof")
            nc.sync.dma_start(wof[:], wo_d[:])
            wo2f = consts.tile([64, C], F32, tag="wo2f", name="wo2f")
            nc.sync.dma_start(wo2f[:], wo_d[64:128, :])
            wo_sb = consts.tile([128, C], mm_dt, tag="wo", name="wo")
            nc.vector.tensor_copy(wo_sb[:], wof[:])
            wo2_sb = consts.tile([64, C], mm_dt, tag="wo2", name="wo2")
            nc.vector.tensor_copy(wo2_sb[:], wo2f[:])
            wo_h = {0: wo_sb, 1: wo2_sb}

            # ---- GroupNorm ----
            # per-channel sums on ACT (activation accumulate), sum-of-squares on
            # DVE (scalar_tensor_tensor accumulate) -> run concurrently.
            # activation output goes to xn_sb as scratch (overwritten below).
            stats = small.tile([128, 8], F32, tag="stats", name="stats")
            for ct in range(2):
                for hf in range(2):
                    sl = x_sb[ct][:, hf * HALF : (hf + 1) * HALF]
                    scratch = xn_sb[ct][:, hf * HALF : (hf + 1) * HALF]
                    i0 = 4 * ct + 2 * hf
                    nc.scalar.activation(
                        scratch, sl, AF.Copy, accum_out=stats[:, i0 : i0 + 1],
                    )
                    nc.vector.scalar_tensor_tensor(
                        scratch, sl, 1.0, sl,
                        op0=OP.mult, op1=OP.mult,
                        accum_out=stats[:, i0 + 1 : i0 + 2],
                    )
            with tc.tile_pool(name="ps_gn", bufs=2, space=bass.MemorySpace.PSUM) as ps_gn:
                # accumulate all four (ct, half) partial (sum, sumsq) into [16, 2]
                g_ps = ps_gn.tile([16, 2], F32, tag="g", name="g")
                for i, (ct, hf) in enumerate([(0, 0), (0, 1), (1, 0), (1, 1)]):
                    i0 = 4 * ct + 2 * hf
                    nc.tensor.matmul(
                        g_ps[:], gind_sb[:, 16 * ct : 16 * ct + 16],
                        stats[:, i0 : i0 + 2],
                        start=(i == 0), stop=(i == 3),
                    )
                mall = small.tile([16, 2], F32, tag="mall", name="mall")
                nc.vector.tensor_scalar_mul(mall[:], g_ps[:], 1.0 / (GC * HW))
                msq = small.tile([16, 1], F32, tag="msq", name="msq")
                nc.vector.tensor_tensor(msq[:], mall[:, 0:1], mall[:, 0:1], op=OP.mult)
                ve = small.tile([16, 1], F32, tag="ve", name="ve")
                nc.vector.tensor_tensor(ve[:], mall[:, 1:2], msq[:], op=OP.subtract)
                ve2 = small.tile([16, 1], F32, tag="ve2", name="ve2")
                nc.vector.tensor_scalar_add(ve2[:], ve[:], EPS)
                # rstd = exp(-0.5 * ln(v)) — keeps ACT in the natural_log_exp
                # table set (shared with attention's Exp: one table load)
                lg = small.tile([16, 1], F32, tag="lg", name="lg")
                nc.scalar.activation(lg[:], ve2[:], AF.Ln)
                # gvals [16, 2] = per-group (mean, rstd)
                gvals = small.tile([16, 2], F32, tag="gvals", name="gvals")
                nc.vector.tensor_copy(gvals[:, 0:1], mall[:, 0:1])
                nc.scalar.activation(gvals[:, 1:2], lg[:], AF.Exp, scale=-0.5)
                for ct in range(2):
                    cv = ps_gn.tile([128, 2], F32, tag="cv", name="cv")
                    nc.tensor.matmul(
                        cv[:], gbc_sb[:, ct * 128 : (ct + 1) * 128], gvals[:],
                        start=True, stop=True,
                    )
                    scale_t = small.tile([128, 1], F32, tag="scale", name="scale")
                    nc.vector.tensor_tensor(scale_t[:], gnp_sb[ct][:, 0:1], cv[:, 1:2], op=OP.mult)
                    tb = small.tile([128, 1], F32, tag="tb", name="tb")
                    nc.vector.tensor_tensor(tb[:], cv[:, 0:1], scale_t[:], op=OP.mult)
                    bias_t = small.tile([128, 1], F32, tag="bias", name="bias")
                    nc.vector.tensor_tensor(bias_t[:], gnp_sb[ct][:, 1:2], tb[:], op=OP.subtract)
                    # one normalize-apply per engine so they run concurrently
                    if ct == 0:
                        nc.vector.tensor_scalar(
                            xn_sb[ct][:], x_sb[ct][:], scale_t[:], bias_t[:],
                            op0=OP.mult, op1=OP.add,
                        )
                    else:
                        nc.scalar.activation(
                            xn_sb[ct][:], x_sb[ct][:], AF.Identity,
                            bias=bias_t[:], scale=scale_t[:],
                        )

            # ---- k and q[0:QSPLIT] projections (needed before attention) ----
            # q is split into two tiles at QSPLIT so the q-half1 projection can
            # be interleaved into the attention loop without false WAR hazards.
            k_sb = big.tile([128, HW], mm_dt, tag="k", name="k")
            q_a = big.tile([128, QSPLIT], mm_dt, tag="qa", name="qa")
            q_b = big.tile([128, HW - QSPLIT], mm_dt, tag="qb", name="qb")

            def q_ap(i0, i1):
                if i1 <= QSPLIT:
                    return q_a[:, i0:i1]
                assert i0 >= QSPLIT
                return q_b[:, i0 - QSPLIT : i1 - QSPLIT]

            # v^T in fp8, laid out [128 spatial, NJ subtiles of VST]: subtile jc
            # holds chunk jc's [64 v-channels + ones column(s)].  Pair 2p,2p+1
            # forms the DoubleRow K=256 stationary operand.
            vt_sb, vt_v = [], []
            for h in range(2):
                t = big.tile([128, NJ * VST], F8, tag=f"vt{h}", name=f"vt{h}")
                nc.gpsimd.memset(t[:], 1.0)  # ones columns (and padding)
                vt_sb.append(t)
                vt_v.append(t[:].rearrange("p (j c) -> p j c", c=VST))

            with tc.tile_pool(name="ps_qkv", bufs=2, space=bass.MemorySpace.PSUM) as ps_qkv:
                for half in range(2):
                    ps = ps_qkv.tile([128, HALF], F32, tag="qkv", name="qkv")
                    for kc in range(2):
                        for n0, n1 in _nchunks(HALF):
                            nc.tensor.matmul(
                                ps[:, n0:n1],
                                w_sb["wk", kc][:],
                                xn_sb[kc][:, half * HALF + n0 : half * HALF + n1],
                                start=(kc == 0), stop=(kc == 1),
                            )
                    nc.vector.tensor_copy(k_sb[:, half * HALF : (half + 1) * HALF], ps[:])
                # q half 0 -> q_a[0:1024] + q_b[0:128]
                ps = ps_qkv.tile([128, HALF], F32, tag="qkv", name="qkv")
                for kc in range(2):
                    for n0, n1 in _nchunks(HALF):
                        nc.tensor.matmul(
                            ps[:, n0:n1],
                            w_sb["wq", kc][:],
                            xn_sb[kc][:, n0:n1],
                            start=(kc == 0), stop=(kc == 1),
                        )
                nc.vector.tensor_copy(q_a[:], ps[:, 0:QSPLIT])
                nc.vector.tensor_copy(q_b[:, 0 : HALF - QSPLIT], ps[:, QSPLIT:HALF])

            # ---- attention, with v / q-half1 / output projection woven in ----
            with (
                tc.tile_pool(name="ps_att", bufs=1, space=bass.MemorySpace.PSUM) as ps_att,
                tc.tile_pool(name="ps_out", bufs=2, space=bass.MemorySpace.PSUM) as ps_out,
            ):
                def vjob(g):
                    # project v for chunks 4g..4g+3 (last group: 2 chunks) and
                    # cast into the fp8 v^T tiles
                    chunks = list(range(4 * g, min(4 * g + 4, NJ)))
                    w = len(chunks) * JC
                    vps = ps_out.tile([128, 512], F32, tag="yp", name="vps")
                    for ci, jc in enumerate(chunks):
                        for kc in range(2):
                            nc.tensor.matmul(
                                vps[:, ci * JC : (ci + 1) * JC],
                                xn_sb[kc][:, jc * JC : (jc + 1) * JC],
                                w_sb["wv", kc][:],
                                start=(kc == 0), stop=(kc == 1),
                            )
                    vps3 = vps[:, 0:w].rearrange("p (j c) -> p j c", c=128)
                    for h in range(2):
                        nc.vector.tensor_copy(
                            vt_v[h][:, chunks[0] : chunks[0] + len(chunks), 0:HD],
                            vps3[:, :, h * HD : (h + 1) * HD],
                        )

                def qjob(n0, n1):
                    # q half 1 chunk [HALF+n0, HALF+n1) -> q_b
                    ps = ps_out.tile([128, 512], F32, tag="yp", name="qps")
                    for kc in range(2):
                        nc.tensor.matmul(
                            ps[:, 0 : n1 - n0],
                            w_sb["wq", kc][:],
                            xn_sb[kc][:, HALF + n0 : HALF + n1],
                            start=(kc == 0), stop=(kc == 1),
                        )
                    nc.vector.tensor_copy(
                        q_b[:, HALF - QSPLIT + n0 : HALF - QSPLIT + n1],
                        ps[:, 0 : n1 - n0],
                    )

                pending = None  # closure emitting previous iblk's epilogue

                def emit_epilogue(i0, i1, u, last=False):
                    blk = i1 - i0
                    for h in range(2):
                        # u (64 channels + denominator row) psum -> sbuf bf16
                        ho = ptp.tile([PVM, blk], mm_dt, tag=f"ho{h}", name=f"ho{h}", bufs=2)
                        if h == 0:
                            nc.vector.tensor_copy(ho[:], u[h][:, 0:blk])
                        else:
                            nc.scalar.copy(ho[:], u[h][:, 0:blk])
                        nc.sync.dma_start(dn_d[h : h + 1, i0:i1], ho[HD : HD + 1, :])
                        for mt in range(2):
                            yp = ps_out.tile([128, blk], F32, tag="yp", name="yp")
                            nc.tensor.matmul(
                                yp[:],
                                wo_h[h][0:64, mt * 128 : (mt + 1) * 128],
                                ho[0:HD, :],
                                start=True, stop=True,
                            )
                            yo = small.tile([128, blk], mm_dt, tag="yo", name="yo")
                            if (h + mt) % 2 == 0:
                                nc.vector.tensor_copy(yo[:], yp[:])
                            else:
                                nc.scalar.copy(yo[:], yp[:])
                            # last iblk: spread stores over both DMA queues so
                            # the drain tail isn't serialized on one queue
                            dma_eng = nc.scalar if last and (h + mt) % 2 else nc.sync
                            dma_eng.dma_start(
                                y_d[h][mt * 128 : (mt + 1) * 128, i0:i1], yo[:],
                            )

                for ib, (i0, i1) in enumerate(IBLKS):
                    blk = i1 - i0
                    # u[h]: [65, blk] accumulator (64 channels + denominator row)
                    u = [
                        ps_att.tile([PVM, SALIGN], F32, tag=f"u{h}", name=f"u{h}", bufs=1)
                        for h in range(2)
                    ]

                    def emit_s(jc):
                        # S^T chunk for both heads, row-tiled (concurrent on PE).
                        # h0/h1 outputs land in different psum banks.
                        st = ps_att.tile([128, 2 * SALIGN], F32, tag="s", name="s", bufs=2)
                        for h in range(2):
                            nc.tensor.matmul(
                                st[:, h * SALIGN : h * SALIGN + blk],
                                k_sb[h * HD : (h + 1) * HD, jc * JC : (jc + 1) * JC],
                                q_ap(i0, i1)[h * HD : (h + 1) * HD, :],
                                start=True, stop=True,
                            )
                        return st

                    def emit_exp(jc, st, pair_v, s):
                        # pt8[:, h, s, :] = exp(st[:, h, :]/16) as fp8e4m3.
                        # slot 0 on ACT, slot 1 on DVE: the two exps of every
                        # pair run concurrently on different engines.
                        src = st[:].rearrange("p (h x) -> p h x", h=2)[:, :, 0:blk]
                        dst = pair_v[:, :, s, 0:blk]
                        if s == 0:
                            nc.scalar.activation(dst, src, AF.Exp, scale=1.0 / 16.0)
                        else:
                            nc.vector.tensor_scalar(
                                dst.bitcast(U8), src, EXPA, EXPB,
                                op0=OP.mult, op1=OP.add,
                            )

                    def emit_pv(pp, pair_v):
                        # DoubleRow fp8: contracts both chunks of the pair (K=256)
                        for h in range(2):
                            nc.tensor.matmul(
                                u[h][:, 0:blk],
                                vt_v[h][:, 2 * pp : 2 * pp + 2, 0:PVM],
                                pair_v[:, h, :, 0:blk],
                                start=(pp == 0), stop=(pp == NJP - 1),
                                perf_mode=PM.DoubleRow,
                            )

                    packed = 2 * blk <= SALIGN  # tail iblk: one exp per pair
                    prev_pair = None
                    for pp in range(NJP):
                        pt = ptp.tile([128, 4 * blk], F8, tag="pt8", name="pt8")
                        pair_v = pt[:].rearrange("p (h s x) -> p h s x", h=2, s=2)
                        if packed:
                            # both chunks' scores into one st tile -> single exp
                            st = ps_att.tile([128, 2 * SALIGN], F32, tag="s", name="s", bufs=2)
                            for s in range(2):
                                jc = 2 * pp + s
                                for h in range(2):
                                    nc.tensor.matmul(
                                        st[:, h * SALIGN + s * blk : h * SALIGN + (s + 1) * blk],
                                        k_sb[h * HD : (h + 1) * HD, jc * JC : (jc + 1) * JC],
                                        q_ap(i0, i1)[h * HD : (h + 1) * HD, :],
                                        start=True, stop=True,
                                    )
                            src4 = st[:].rearrange("p (h s x) -> p h s x", h=2, s=2)
                            dst4 = pair_v[:, :, :, 0:blk]
                            if pp % 2 == 0:
                                nc.scalar.activation(dst4, src4, AF.Exp, scale=1.0 / 16.0)
                            else:
                                nc.vector.tensor_scalar(
                                    dst4.bitcast(U8), src4, EXPA, EXPB,
                                    op0=OP.mult, op1=OP.add,
                                )
                        else:
                            for s in range(2):
                                jc = 2 * pp + s
                                st = emit_s(jc)
                                emit_exp(jc, st, pair_v, s)
                        if ib == 0 and pp < 5:
                            vjob(pp)
                        if ib == 1 and pp < 3:
                            qjob(*(_nchunks(HALF)[pp]))
                        if pp == 1 and pending is not None:
                            pending()
                            pending = None
                        if pp > 0:
                            emit_pv(pp - 1, prev_pair)
                        prev_pair = pair_v
                    emit_pv(NJP - 1, prev_pair)
                    pending = (lambda a=i0, b_=i1, t=u: emit_epilogue(a, b_, t))
                pending = (lambda a=i0, b_=i1, t=u: emit_epilogue(a, b_, t, last=True))
                pending()

    nc.compile()
    return nc


def _consts():
    # gind[:, 0:16]: tile-0 channel -> group one-hot; [:, 16:32]: tile-1 channel -> group
    gind = np.zeros((128, 32), np.float32)
    for c in range(128):
        gind[c, c // GC] = 1.0
        gind[c, 16 + 8 + c // GC] = 1.0
    gbc = np.zeros((16, C), np.float32)
    for c in range(C):
        gbc[c // GC, c] = 1.0
    return gind, gbc


def make_in_maps(x, gn_weight, gn_bias, qkv_w, out_w, out_b):
    x = np.asarray(x, np.float32)
    qkv_w = np.asarray(qkv_w, np.float32)
    out_w = np.asarray(out_w, np.float32)
    gn_weight = np.asarray(gn_weight, np.float32)
    gn_bias = np.asarray(gn_bias, np.float32)
    xr = np.ascontiguousarray(x.reshape(B, C, HW))
    gind, gbc = _consts()
    gnp = np.ascontiguousarray(np.stack([gn_weight, gn_bias], axis=1))
    in_maps = []
    for core in range(NCORES):
        b, hp = divmod(core, 2)
        heads = (2 * hp, 2 * hp + 1)
        qs = np.concatenate([qkv_w[n * 192 : n * 192 + 64] for n in heads], 0)
        ks = np.concatenate([qkv_w[n * 192 + 64 : n * 192 + 128] for n in heads], 0)
        vs = np.concatenate([qkv_w[n * 192 + 128 : n * 192 + 192] for n in heads], 0)
        in_maps.append({
            "x": xr[b],
            "wq": np.ascontiguousarray(qs.T),
            "wk": np.ascontiguousarray(ks.T),
            "wv": np.ascontiguousarray(vs.T),
            "wo": np.ascontiguousarray(out_w[:, hp * 128 : (hp + 1) * 128].T),
            "gnp": gnp,
            "gind": gind,
            "gbc": gbc,
        })
    return in_maps


def gather(results, x, out_b):
    """Host-side: divide per-head partials by softmax denominators, sum, add
    residual + bias."""
    x = np.asarray(x, np.float32)
    out_b = np.asarray(out_b, np.float32)
    xr = x.reshape(B, C, HW)
    y = np.empty((B, C, HW), np.float32)
    for b in range(B):
        acc = xr[b] + out_b[:, None]
        for hp in range(2):
            r = results[2 * b + hp]
            dns = np.asarray(r["dns"], np.float32)
            acc = acc + np.asarray(r["y0"], np.float32) / dns[0][None, :]
            acc = acc + np.asarray(r["y1"], np.float32) / dns[1][None, :]
        y[b] = acc
    return y.reshape(B, C, H, W)


_NC_CACHE = {}


def get_nc(mm_dt=BF16):
    key = str(mm_dt)
    if key not in _NC_CACHE:
        _NC_CACHE[key] = _build(mm_dt)
    return _NC_CACHE[key]


def kernel(x, gn_weight, gn_bias, qkv_w, out_w, out_b):
    nc = get_nc(BF16)
    in_maps = make_in_maps(x, gn_weight, gn_bias, qkv_w, out_w, out_b)
    res = bass_utils.run_bass_kernel_spmd(nc, in_maps, core_ids=list(range(NCORES)))
    return gather(res.results, x, out_b)


# revision 24
# speedup vs baseline: 1.4011x; 1.0057x over previous
"""Trainium2 Bass kernel for spatial attention (GroupNorm + QKV + softmax attention
+ output projection + residual), distributed over 8 NeuronCores.

Sharding: core = 2*b + hp handles image b (of 4) and head pair hp (heads 2hp, 2hp+1).
Each core computes GroupNorm(x[b]), its heads' q/k/v, full spatial attention for its
two heads, and per-head UNNORMALIZED partial output projections.  The softmax
denominators ship back with the partials; the host divides, sums the four partials
per image, and adds the residual + bias (cheap [C, HW] numpy ops, off the device
critical path).

Perf notes (v4):
- Scores bf16, row-tiled: both heads' S^T matmuls run concurrently on the PE.
- softmax exp split across engines: 5 of 9 key chunks on ACT (fp8 output),
  4 of 9 on the DVE via a Schraudolph bit-trick (uint8 = trunc(s*A+B)
  reinterpreted as fp8e4m3) — the 10.6M-element exp load is the bottleneck.
- PV runs as fp8 DoubleRow matmuls (two key chunks = K=256 per instruction),
  halving PV stream time; v^T carries a 65th all-ones column so the softmax
  denominator accumulates in the same matmul.
- No on-device normalize/residual: u (incl. denominator row) is copied psum->sbuf
  bf16, projected per head, and DMA'd out; host does the divides.
- v and q-half1 projections are interleaved INTO the first two attention iblks
  (sharing the ps_out psum tag) so the PE never idles while exp catches up.
"""

import math

import numpy as np

import concourse.bass as bass
import concourse.bacc as bacc
import concourse.tile as tile
from concourse import mybir
from concourse import bass_utils
from concourse.alu_op_type import AluOpType

B, C, H, W = 4, 256, 48, 48
HW = H * W  # 2304
NH, HD = 4, 64
G, GC = 16, 16  # 16 groups x 16 channels
EPS = 1e-5
NCORES = 8
JC = 128  # j (key spatial) chunk
NJ = HW // JC  # 18
NJP = NJ // 2  # 9 key-chunk pairs (DoubleRow K=256)
IBLKS = [(0, 512), (512, 1024), (1024, 1536), (1536, 2048), (2048, 2304)]
HALF = HW // 2  # 1152
QSPLIT = 1024  # q_sb split point (iblk-aligned)
PVM = HD + 1  # 65: 64 v channels + denominator ones row
VST = 80  # fp8 v^T subtile stride (16-byte aligned, >= PVM)
SALIGN = 512

F32 = mybir.dt.float32
BF16 = mybir.dt.bfloat16
F8 = mybir.dt.float8e4
U8 = mybir.dt.uint8
AX = mybir.AxisListType.X
AF = mybir.ActivationFunctionType
OP = AluOpType
PM = mybir.MatmulPerfMode

# Schraudolph exp into fp8e4m3 bit space: bits = trunc(s*EXPA + EXPB),
# value(bits) ~= exp(s/16).  EXPB tuned numerically for minimax rel err (~7%)
# assuming truncation on the DVE float->uint8 convert.
EXPA = 8.0 * math.log2(math.e) / 16.0
EXPB = 56.13


def _nchunks(size, step=512):
    # PSUM-bank-aligned chunks: a matmul output may not cross a 512-fp32 bank boundary
    return [(a, min(a + step, size)) for a in range(0, size, step)]


def _build(mm_dt=BF16):
    nc = bacc.Bacc("TRN2", target_bir_lowering=False, debug=False, enable_asserts=False)

    x_d = nc.dram_tensor("x", [C, HW], F32, kind="ExternalInput").ap()
    wq_d = nc.dram_tensor("wq", [C, 2 * HD], F32, kind="ExternalInput").ap()
    wk_d = nc.dram_tensor("wk", [C, 2 * HD], F32, kind="ExternalInput").ap()
    wv_d = nc.dram_tensor("wv", [C, 2 * HD], F32, kind="ExternalInput").ap()
    wo_d = nc.dram_tensor("wo", [2 * HD, C], F32, kind="ExternalInput").ap()
    gnp_d = nc.dram_tensor("gnp", [C, 2], F32, kind="ExternalInput").ap()
    gind_d = nc.dram_tensor("gind", [128, 32], F32, kind="ExternalInput").ap()
    gbc_d = nc.dram_tensor("gbc", [16, C], F32, kind="ExternalInput").ap()
    y_d = [
        nc.dram_tensor(f"y{h}", [C, HW], BF16, kind="ExternalOutput").ap()
        for h in range(2)
    ]
    dn_d = nc.dram_tensor("dns", [2, HW], BF16, kind="ExternalOutput").ap()

    with tile.TileContext(nc) as tc:
        with (
            tc.tile_pool(name="consts", bufs=1) as consts,
            tc.tile_pool(name="big", bufs=1) as big,
            tc.tile_pool(name="small", bufs=4) as small,
            tc.tile_pool(name="pt", bufs=3) as ptp,
        ):
            # ---- input x first (GN stats are the critical path) ----
            # halves go over both DMA queues (SP + ACT) in parallel
            x_sb, xn_sb = [], []
            for ct in range(2):
                t = big.tile([128, HW], F32, tag=f"x{ct}", name=f"x{ct}")
                nc.sync.dma_start(t[:, 0:HALF], x_d[ct * 128 : (ct + 1) * 128, 0:HALF])
                nc.scalar.dma_start(t[:, HALF:HW], x_d[ct * 128 : (ct + 1) * 128, HALF:HW])
                x_sb.append(t)
                xn_sb.append(big.tile([128, HW], mm_dt, tag=f"xn{ct}", name=f"xn{ct}"))

            # ---- constant / weight loads ----
            gind_sb = consts.tile([128, 32], F32, tag="gind", name="gind")
            nc.sync.dma_start(gind_sb[:], gind_d[:])
            gbc_sb = consts.tile([16, C], F32, tag="gbc", name="gbc")
            nc.sync.dma_start(gbc_sb[:], gbc_d[:])
            gnp_sb = []
            for ct in range(2):
                t = consts.tile([128, 2], F32, tag=f"gnp{ct}", name=f"gnp{ct}")
                nc.sync.dma_start(t[:], gnp_d[ct * 128 : (ct + 1) * 128, :])
                gnp_sb.append(t)
            w_sb = {}
            for name, d in (("wk", wk_d), ("wq", wq_d), ("wv", wv_d)):
                for kc in range(2):
                    tf = consts.tile([128, 2 * HD], F32, tag=f"{name}{kc}f", name=f"{name}{kc}f")
                    nc.sync.dma_start(tf[:], d[kc * 128 : (kc + 1) * 128, :])
                    t = consts.tile([128, 2 * HD], mm_dt, tag=f"{name}{kc}", name=f"{name}{kc}")
                    nc.vector.tensor_copy(t[:], tf[:])
                    w_sb[name, kc] = t
            # wo rows 0:64 (head 0) in place; rows 64:128 (head 1) also loaded at
            # base partition 0 so both heads' K=64 projections can stream from
            # partitions 0-63 (rhs = ho tile lives there).
            wof = consts.tile([128, C], F32, tag="wof", name="wof")
            nc.sync.dma_start(wof[:], wo_d[:])
            wo2f = consts.tile([64, C], F32, tag="wo2f", name="wo2f")
            nc.sync.dma_start(wo2f[:], wo_d[64:128, :])
            wo_sb = consts.tile([128, C], mm_dt, tag="wo", name="wo")
            nc.vector.tensor_copy(wo_sb[:], wof[:])
            wo2_sb = consts.tile([64, C], mm_dt, tag="wo2", name="wo2")
            nc.vector.tensor_copy(wo2_sb[:], wo2f[:])
            wo_h = {0: wo_sb, 1: wo2_sb}

            # ---- GroupNorm ----
            # per-channel sums on ACT (activation accumulate), sum-of-squares on
            # DVE (scalar_tensor_tensor accumulate) -> run concurrently.
            # activation output goes to xn_sb as scratch (overwritten below).
            stats = small.tile([128, 8], F32, tag="stats", name="stats")
            for ct in range(2):
                for hf in range(2):
                    sl = x_sb[ct][:, hf * HALF : (hf + 1) * HALF]
                    scratch = xn_sb[ct][:, hf * HALF : (hf + 1) * HALF]
                    i0 = 4 * ct + 2 * hf
                    nc.scalar.activation(
                        scratch, sl, AF.Copy, accum_out=stats[:, i0 : i0 + 1],
                    )
                    nc.vector.scalar_tensor_tensor(
                        scratch, sl, 1.0, sl,
                        op0=OP.mult, op1=OP.mult,
                        accum_out=stats[:, i0 + 1 : i0 + 2],
                    )
            with tc.tile_pool(name="ps_gn", bufs=2, space=bass.MemorySpace.PSUM) as ps_gn:
                # accumulate all four (ct, half) partial (sum, sumsq) into [16, 2]
                g_ps = ps_gn.tile([16, 2], F32, tag="g", name="g")
                for i, (ct, hf) in enumerate([(0, 0), (0, 1), (1, 0), (1, 1)]):
                    i0 = 4 * ct + 2 * hf
                    nc.tensor.matmul(
                        g_ps[:], gind_sb[:, 16 * ct : 16 * ct + 16],
                        stats[:, i0 : i0 + 2],
                        start=(i == 0), stop=(i == 3),
                    )
                mall = small.tile([16, 2], F32, tag="mall", name="mall")
                nc.vector.tensor_scalar_mul(mall[:], g_ps[:], 1.0 / (GC * HW))
                msq = small.tile([16, 1], F32, tag="msq", name="msq")
                nc.vector.tensor_tensor(msq[:], mall[:, 0:1], mall[:, 0:1], op=OP.mult)
                ve = small.tile([16, 1], F32, tag="ve", name="ve")
                nc.vector.tensor_tensor(ve[:], mall[:, 1:2], msq[:], op=OP.subtract)
                ve2 = small.tile([16, 1], F32, tag="ve2", name="ve2")
                nc.vector.tensor_scalar_add(ve2[:], ve[:], EPS)
                # rstd = exp(-0.5 * ln(v)) — keeps ACT in the natural_log_exp
                # table set (shared with attention's Exp: one table load)
                lg = small.tile([16, 1], F32, tag="lg", name="lg")
                nc.scalar.activation(lg[:], ve2[:], AF.Ln)
                # gvals [16, 2] = per-group (mean, rstd)
                gvals = small.tile([16, 2], F32, tag="gvals", name="gvals")
                nc.vector.tensor_copy(gvals[:, 0:1], mall[:, 0:1])
                nc.scalar.activation(gvals[:, 1:2], lg[:], AF.Exp, scale=-0.5)
                for ct in range(2):
                    cv = ps_gn.tile([128, 2], F32, tag="cv", name="cv")
                    nc.tensor.matmul(
                        cv[:], gbc_sb[:, ct * 128 : (ct + 1) * 128], gvals[:],
                        start=True, stop=True,
                    )
                    scale_t = small.tile([128, 1], F32, tag="scale", name="scale")
                    nc.vector.tensor_tensor(scale_t[:], gnp_sb[ct][:, 0:1], cv[:, 1:2], op=OP.mult)
                    tb = small.tile([128, 1], F32, tag="tb", name="tb")
                    nc.vector.tensor_tensor(tb[:], cv[:, 0:1], scale_t[:], op=OP.mult)
                    bias_t = small.tile([128, 1], F32, tag="bias", name="bias")
                    nc.vector.tensor_tensor(bias_t[:], gnp_sb[ct][:, 1:2], tb[:], op=OP.subtract)
                    # one normalize-apply per engine so they run concurrently
                    if ct == 0:
                        nc.vector.tensor_scalar(
                            xn_sb[ct][:], x_sb[ct][:], scale_t[:], bias_t[:],
                            op0=OP.mult, op1=OP.add,
                        )
                    else:
                        nc.scalar.activation(
                            xn_sb[ct][:], x_sb[ct][:], AF.Identity,
                            bias=bias_t[:], scale=scale_t[:],
                        )

            # ---- QKV projections (pre-attention phase) ----
            k_sb = big.tile([128, HW], mm_dt, tag="k", name="k")
            q_sb = big.tile([128, HW], mm_dt, tag="q", name="q")

            def q_ap(i0, i1):
                return q_sb[:, i0:i1]

            # v^T in fp8, laid out [128 spatial, NJ subtiles of VST]: subtile jc
            # holds chunk jc's [64 v-channels + ones column(s)].  Pair 2p,2p+1
            # forms the DoubleRow K=256 stationary operand.
            vt_sb, vt_v = [], []
            for h in range(2):
                t = big.tile([128, NJ * VST], F8, tag=f"vt{h}", name=f"vt{h}")
                nc.gpsimd.memset(t[:], 1.0)  # ones columns (and padding)
                vt_sb.append(t)
                vt_v.append(t[:].rearrange("p (j c) -> p j c", c=VST))

            with tc.tile_pool(name="ps_qkv", bufs=2, space=bass.MemorySpace.PSUM) as ps_qkv:
                for dst, wname in ((k_sb, "wk"), (q_sb, "wq")):
                    for half in range(2):
                        ps = ps_qkv.tile([128, HALF], F32, tag="qkv", name="qkv")
                        for kc in range(2):
                            for n0, n1 in _nchunks(HALF):
                                nc.tensor.matmul(
                                    ps[:, n0:n1],
                                    w_sb[wname, kc][:],
                                    xn_sb[kc][:, half * HALF + n0 : half * HALF + n1],
                                    start=(kc == 0), stop=(kc == 1),
                                )
                        eng = nc.vector if (wname == "wk") == (half == 0) else nc.scalar
                        if eng is nc.vector:
                            nc.vector.tensor_copy(dst[:, half * HALF : (half + 1) * HALF], ps[:])
                        else:
                            nc.scalar.copy(dst[:, half * HALF : (half + 1) * HALF], ps[:])
                for half in range(2):
                    vps = ps_qkv.tile([128, HALF], F32, tag="qkv", name="qkv")
                    for j9 in range(9):
                        jc = half * 9 + j9
                        for kc in range(2):
                            nc.tensor.matmul(
                                vps[:, j9 * 128 : (j9 + 1) * 128],
                                xn_sb[kc][:, jc * JC : (jc + 1) * JC],
                                w_sb["wv", kc][:],
                                start=(kc == 0), stop=(kc == 1),
                            )
                    vps3 = vps[:].rearrange("p (j c) -> p j c", c=128)
                    for h in range(2):
                        nc.vector.tensor_copy(
                            vt_v[h][:, half * 9 : (half + 1) * 9, 0:HD],
                            vps3[:, :, h * HD : (h + 1) * HD],
                        )

            # ---- attention ----
            # st gets 3 psum buffers (6 banks) so the scores->exp->WAR chain
            # never gates the pipeline; with u (2 banks) that is all of PSUM,
            # so the output projections run in a separate phase afterwards.
            ho_saved = []
            with (
                tc.tile_pool(name="ps_att", bufs=1, space=bass.MemorySpace.PSUM) as ps_att,
            ):
                def emit_epilogue(i0, i1, u):
                    blk = i1 - i0
                    hos = []
                    for h in range(2):
                        # u (64 channels + denominator row) psum -> sbuf bf16
                        ho = ptp.tile([PVM, blk], mm_dt, tag=f"ho{h}", name=f"ho{h}", bufs=5)
                        if h == 0:
                            nc.vector.tensor_copy(ho[:], u[h][:, 0:blk])
                        else:
                            nc.scalar.copy(ho[:], u[h][:, 0:blk])
                        nc.sync.dma_start(dn_d[h : h + 1, i0:i1], ho[HD : HD + 1, :])
                        hos.append(ho)
                    ho_saved.append((i0, i1, hos))

                for ib, (i0, i1) in enumerate(IBLKS):
                    blk = i1 - i0
                    # u[h]: [65, blk] accumulator (64 channels + denominator row)
                    u = [
                        ps_att.tile([PVM, SALIGN], F32, tag=f"u{h}", name=f"u{h}", bufs=1)
                        for h in range(2)
                    ]

                    def emit_s(jc):
                        # S^T chunk for both heads, row-tiled (concurrent on PE).
                        # h0/h1 outputs land in different psum banks.
                        st = ps_att.tile([128, 2 * SALIGN], F32, tag="s", name="s", bufs=3)
                        for h in range(2):
                            nc.tensor.matmul(
                                st[:, h * SALIGN : h * SALIGN + blk],
                                k_sb[h * HD : (h + 1) * HD, jc * JC : (jc + 1) * JC],
                                q_ap(i0, i1)[h * HD : (h + 1) * HD, :],
                                start=True, stop=True,
                            )
                        return st

                    def emit_exp(jc, st, pair_v, s):
                        # pt8[:, h, s, :] = exp(st[:, h, :]/16) as fp8e4m3.
                        # slot 0 on ACT, slot 1 on DVE: the two exps of every
                        # pair run concurrently on different engines.
                        src = st[:].rearrange("p (h x) -> p h x", h=2)[:, :, 0:blk]
                        dst = pair_v[:, :, s, 0:blk]
                        if s == 0:
                            nc.scalar.activation(dst, src, AF.Exp, scale=1.0 / 16.0)
                        else:
                            nc.vector.tensor_scalar(
                                dst.bitcast(U8), src, EXPA, EXPB,
                                op0=OP.mult, op1=OP.add,
                            )

                    def emit_pv(pp, pair_v):
                        # DoubleRow fp8: contracts both chunks of the pair (K=256)
                        for h in range(2):
                            nc.tensor.matmul(
                                u[h][:, 0:blk],
                                vt_v[h][:, 2 * pp : 2 * pp + 2, 0:PVM],
                                pair_v[:, h, :, 0:blk],
                                start=(pp == 0), stop=(pp == NJP - 1),
                                perf_mode=PM.DoubleRow,
                            )

                    packed = 2 * blk <= SALIGN  # tail iblk: one exp per pair
                    prev_pair = None
                    for pp in range(NJP):
                        pt = ptp.tile([128, 4 * blk], F8, tag="pt8", name="pt8")
                        pair_v = pt[:].rearrange("p (h s x) -> p h s x", h=2, s=2)
                        if packed:
                            # both chunks' scores into one st tile -> single exp
                            st = ps_att.tile([128, 2 * SALIGN], F32, tag="s", name="s", bufs=3)
                            for s in range(2):
                                jc = 2 * pp + s
                                for h in range(2):
                                    nc.tensor.matmul(
                                        st[:, h * SALIGN + s * blk : h * SALIGN + (s + 1) * blk],
                                        k_sb[h * HD : (h + 1) * HD, jc * JC : (jc + 1) * JC],
                                        q_ap(i0, i1)[h * HD : (h + 1) * HD, :],
                                        start=True, stop=True,
                                    )
                            src4 = st[:].rearrange("p (h s x) -> p h s x", h=2, s=2)
                            dst4 = pair_v[:, :, :, 0:blk]
                            if pp % 2 == 0:
                                nc.scalar.activation(dst4, src4, AF.Exp, scale=1.0 / 16.0)
                            else:
                                nc.vector.tensor_scalar(
                                    dst4.bitcast(U8), src4, EXPA, EXPB,
                                    op0=OP.mult, op1=OP.add,
                                )
                        else:
                            for s in range(2):
                                jc = 2 * pp + s
                                st = emit_s(jc)
                                emit_exp(jc, st, pair_v, s)
                        if pp > 0:
                            emit_pv(pp - 1, prev_pair)
                        prev_pair = pair_v
                    emit_pv(NJP - 1, prev_pair)
                    emit_epilogue(i0, i1, u)

            # ---- output projections (per head, unnormalized) + stores ----
            with tc.tile_pool(name="ps_fin", bufs=4, space=bass.MemorySpace.PSUM) as ps_fin:
                n = 0
                for i0, i1, hos in ho_saved:
                    blk = i1 - i0
                    for h in range(2):
                        for mt in range(2):
                            yp = ps_fin.tile([128, SALIGN], F32, tag="yp", name="yp")
                            nc.tensor.matmul(
                                yp[:, 0:blk],
                                wo_h[h][0:64, mt * 128 : (mt + 1) * 128],
                                hos[h][0:HD, :],
                                start=True, stop=True,
                            )
                            yo = small.tile([128, blk], mm_dt, tag="yo", name="yo")
                            if n % 2 == 0:
                                nc.vector.tensor_copy(yo[:], yp[:, 0:blk])
                            else:
                                nc.scalar.copy(yo[:], yp[:, 0:blk])
                            dma_eng = nc.scalar if n % 2 else nc.sync
                            dma_eng.dma_start(
                                y_d[h][mt * 128 : (mt + 1) * 128, i0:i1], yo[:],
                            )
                            n += 1

    nc.compile()
    return nc


def _consts():
    # gind[:, 0:16]: tile-0 channel -> group one-hot; [:, 16:32]: tile-1 channel -> group
    gind = np.zeros((128, 32), np.float32)
    for c in range(128):
        gind[c, c // GC] = 1.0
        gind[c, 16 + 8 + c // GC] = 1.0
    gbc = np.zeros((16, C), np.float32)
    for c in range(C):
        gbc[c // GC, c] = 1.0
    return gind, gbc


def make_in_maps(x, gn_weight, gn_bias, qkv_w, out_w, out_b):
    x = np.asarray(x, np.float32)
    qkv_w = np.asarray(qkv_w, np.float32)
    out_w = np.asarray(out_w, np.float32)
    gn_weight = np.asarray(gn_weight, np.float32)
    gn_bias = np.asarray(gn_bias, np.float32)
    xr = np.ascontiguousarray(x.reshape(B, C, HW))
    gind, gbc = _consts()
    gnp = np.ascontiguousarray(np.stack([gn_weight, gn_bias], axis=1))
    in_maps = []
    for core in range(NCORES):
        b, hp = divmod(core, 2)
        heads = (2 * hp, 2 * hp + 1)
        qs = np.concatenate([qkv_w[n * 192 : n * 192 + 64] for n in heads], 0)
        ks = np.concatenate([qkv_w[n * 192 + 64 : n * 192 + 128] for n in heads], 0)
        vs = np.concatenate([qkv_w[n * 192 + 128 : n * 192 + 192] for n in heads], 0)
        in_maps.append({
            "x": xr[b],
            "wq": np.ascontiguousarray(qs.T),
            "wk": np.ascontiguousarray(ks.T),
            "wv": np.ascontiguousarray(vs.T),
            "wo": np.ascontiguousarray(out_w[:, hp * 128 : (hp + 1) * 128].T),
            "gnp": gnp,
            "gind": gind,
            "gbc": gbc,
        })
    return in_maps


def gather(results, x, out_b):
    """Host-side: divide per-head partials by softmax denominators, sum, add
    residual + bias."""
    x = np.asarray(x, np.float32)
    out_b = np.asarray(out_b, np.float32)
    xr = x.reshape(B, C, HW)
    y = np.empty((B, C, HW), np.float32)
    for b in range(B):
        acc = xr[b] + out_b[:, None]
        for hp in range(2):
            r = results[2 * b + hp]
            dns = np.asarray(r["dns"], np.float32)
            acc = acc + np.asarray(r["y0"], np.float32) / dns[0][None, :]
            acc = acc + np.asarray(r["y1"], np.float32) / dns[1][None, :]
        y[b] = acc
    return y.reshape(B, C, H, W)


_NC_CACHE = {}


def get_nc(mm_dt=BF16):
    key = str(mm_dt)
    if key not in _NC_CACHE:
        _NC_CACHE[key] = _build(mm_dt)
    return _NC_CACHE[key]


def kernel(x, gn_weight, gn_bias, qkv_w, out_w, out_b):
    nc = get_nc(BF16)
    in_maps = make_in_maps(x, gn_weight, gn_bias, qkv_w, out_w, out_b)
    res = bass_utils.run_bass_kernel_spmd(nc, in_maps, core_ids=list(range(NCORES)))
    return gather(res.results, x, out_b)


# revision 28
# speedup vs baseline: 1.4230x; 1.0156x over previous
"""Trainium2 Bass kernel for spatial attention (GroupNorm + QKV + softmax attention
+ output projection + residual), distributed over 8 NeuronCores.

Sharding: core = 2*b + hp handles image b (of 4) and head pair hp (heads 2hp, 2hp+1).
Each core computes GroupNorm(x[b]), its heads' q/k/v, full spatial attention for its
two heads, and per-head UNNORMALIZED partial output projections.  The softmax
denominators ship back with the partials; the host divides, sums the four partials
per image, and adds the residual + bias (cheap [C, HW] numpy ops, off the device
critical path).

Perf notes (v4):
- Scores bf16, row-tiled: both heads' S^T matmuls run concurrently on the PE.
- softmax exp split across engines: 5 of 9 key chunks on ACT (fp8 output),
  4 of 9 on the DVE via a Schraudolph bit-trick (uint8 = trunc(s*A+B)
  reinterpreted as fp8e4m3) — the 10.6M-element exp load is the bottleneck.
- PV runs as fp8 DoubleRow matmuls (two key chunks = K=256 per instruction),
  halving PV stream time; v^T carries a 65th all-ones column so the softmax
  denominator accumulates in the same matmul.
- No on-device normalize/residual: u (incl. denominator row) is copied psum->sbuf
  bf16, projected per head, and DMA'd out; host does the divides.
- v and q-half1 projections are interleaved INTO the first two attention iblks
  (sharing the ps_out psum tag) so the PE never idles while exp catches up.
"""

import math

import numpy as np

import concourse.bass as bass
import concourse.bacc as bacc
import concourse.tile as tile
from concourse import mybir
from concourse import bass_utils
from concourse.alu_op_type import AluOpType

B, C, H, W = 4, 256, 48, 48
HW = H * W  # 2304
NH, HD = 4, 64
G, GC = 16, 16  # 16 groups x 16 channels
EPS = 1e-5
NCORES = 8
JC = 128  # j (key spatial) chunk
NJ = HW // JC  # 18
NJP = NJ // 2  # 9 key-chunk pairs (DoubleRow K=256)
IBLKS = [(0, 512), (512, 1024), (1024, 1536), (1536, 2048), (2048, 2304)]
HALF = HW // 2  # 1152
QSPLIT = 1024  # q_sb split point (iblk-aligned)
PVM = HD + 1  # 65: 64 v channels + denominator ones row
VST = 80  # fp8 v^T subtile stride (16-byte aligned, >= PVM)
SALIGN = 512

F32 = mybir.dt.float32
BF16 = mybir.dt.bfloat16
F8 = mybir.dt.float8e4
U8 = mybir.dt.uint8
AX = mybir.AxisListType.X
AF = mybir.ActivationFunctionType
OP = AluOpType
PM = mybir.MatmulPerfMode

# Schraudolph exp into fp8e4m3 bit space: bits = trunc(s*EXPA + EXPB),
# value(bits) ~= exp(s/16).  EXPB tuned numerically for minimax rel err (~7%)
# assuming truncation on the DVE float->uint8 convert.
EXPA = 8.0 * math.log2(math.e) / 16.0
EXPB = 56.13


def _nchunks(size, step=512):
    # PSUM-bank-aligned chunks: a matmul output may not cross a 512-fp32 bank boundary
    return [(a, min(a + step, size)) for a in range(0, size, step)]


def _build(mm_dt=BF16):
    nc = bacc.Bacc("TRN2", target_bir_lowering=False, debug=False, enable_asserts=False)

    x_d = nc.dram_tensor("x", [C, HW], F32, kind="ExternalInput").ap()
    wq_d = nc.dram_tensor("wq", [C, 2 * HD], F32, kind="ExternalInput").ap()
    wk_d = nc.dram_tensor("wk", [C, 2 * HD], F32, kind="ExternalInput").ap()
    wv_d = nc.dram_tensor("wv", [C, 2 * HD], F32, kind="ExternalInput").ap()
    wo_d = nc.dram_tensor("wo", [2 * HD, C], F32, kind="ExternalInput").ap()
    gnp_d = nc.dram_tensor("gnp", [C, 2], F32, kind="ExternalInput").ap()
    gind_d = nc.dram_tensor("gind", [128, 32], F32, kind="ExternalInput").ap()
    gbc_d = nc.dram_tensor("gbc", [16, C], F32, kind="ExternalInput").ap()
    y_d = [
        nc.dram_tensor(f"y{h}", [C, HW], BF16, kind="ExternalOutput").ap()
        for h in range(2)
    ]
    dn_d = nc.dram_tensor("dns", [2, HW], BF16, kind="ExternalOutput").ap()

    with tile.TileContext(nc) as tc:
        with (
            tc.tile_pool(name="consts", bufs=1) as consts,
            tc.tile_pool(name="big", bufs=1) as big,
            tc.tile_pool(name="small", bufs=4) as small,
            tc.tile_pool(name="pt", bufs=3) as ptp,
        ):
            # ---- input x first (GN stats are the critical path) ----
            # halves go over both DMA queues (SP + ACT) in parallel
            x_sb, xn_sb = [], []
            for ct in range(2):
                t = big.tile([128, HW], F32, tag=f"x{ct}", name=f"x{ct}")
                nc.sync.dma_start(t[:, 0:HALF], x_d[ct * 128 : (ct + 1) * 128, 0:HALF])
                nc.scalar.dma_start(t[:, HALF:HW], x_d[ct * 128 : (ct + 1) * 128, HALF:HW])
                x_sb.append(t)
                xn_sb.append(big.tile([128, HW], mm_dt, tag=f"xn{ct}", name=f"xn{ct}"))

            # ---- constant / weight loads ----
            gind_sb = consts.tile([128, 32], F32, tag="gind", name="gind")
            nc.sync.dma_start(gind_sb[:], gind_d[:])
            gbc_sb = consts.tile([16, C], F32, tag="gbc", name="gbc")
            nc.sync.dma_start(gbc_sb[:], gbc_d[:])
            gnp_sb = []
            for ct in range(2):
                t = consts.tile([128, 2], F32, tag=f"gnp{ct}", name=f"gnp{ct}")
                nc.sync.dma_start(t[:], gnp_d[ct * 128 : (ct + 1) * 128, :])
                gnp_sb.append(t)
            # dummy exp: forces the ACT exp table load NOW (overlapped with the
            # x DMA) instead of inside the GN/attention critical path
            warm = small.tile([128, 2], F32, tag="warm", name="warm")
            nc.scalar.activation(warm[:], gnp_sb[0][:], AF.Exp)
            w_sb = {}
            for name, d in (("wk", wk_d), ("wq", wq_d), ("wv", wv_d)):
                for kc in range(2):
                    tf = consts.tile([128, 2 * HD], F32, tag=f"{name}{kc}f", name=f"{name}{kc}f")
                    nc.sync.dma_start(tf[:], d[kc * 128 : (kc + 1) * 128, :])
                    t = consts.tile([128, 2 * HD], mm_dt, tag=f"{name}{kc}", name=f"{name}{kc}")
                    nc.vector.tensor_copy(t[:], tf[:])
                    w_sb[name, kc] = t
            # wo rows 0:64 (head 0) in place; rows 64:128 (head 1) also loaded at
            # base partition 0 so both heads' K=64 projections can stream from
            # partitions 0-63 (rhs = ho tile lives there).
            wof = consts.tile([128, C], F32, tag="wof", name="wof")
            nc.sync.dma_start(wof[:], wo_d[:])
            wo2f = consts.tile([64, C], F32, tag="wo2f", name="wo2f")
            nc.sync.dma_start(wo2f[:], wo_d[64:128, :])
            wo_sb = consts.tile([128, C], mm_dt, tag="wo", name="wo")
            nc.vector.tensor_copy(wo_sb[:], wof[:])
            wo2_sb = consts.tile([64, C], mm_dt, tag="wo2", name="wo2")
            nc.vector.tensor_copy(wo2_sb[:], wo2f[:])
            wo_h = {0: wo_sb, 1: wo2_sb}

            # ---- GroupNorm ----
            # per-channel sums on ACT (activation accumulate), sum-of-squares on
            # DVE (scalar_tensor_tensor accumulate) -> run concurrently.
            # activation output goes to xn_sb as scratch (overwritten below).
            stats = small.tile([128, 8], F32, tag="stats", name="stats")
            for ct in range(2):
                for hf in range(2):
                    sl = x_sb[ct][:, hf * HALF : (hf + 1) * HALF]
                    scratch = xn_sb[ct][:, hf * HALF : (hf + 1) * HALF]
                    i0 = 4 * ct + 2 * hf
                    nc.scalar.activation(
                        scratch, sl, AF.Copy, accum_out=stats[:, i0 : i0 + 1],
                    )
                    nc.vector.scalar_tensor_tensor(
                        scratch, sl, 1.0, sl,
                        op0=OP.mult, op1=OP.mult,
                        accum_out=stats[:, i0 + 1 : i0 + 2],
                    )
            with tc.tile_pool(name="ps_gn", bufs=2, space=bass.MemorySpace.PSUM) as ps_gn:
                # accumulate all four (ct, half) partial (sum, sumsq) into [16, 2]
                g_ps = ps_gn.tile([16, 2], F32, tag="g", name="g")
                for i, (ct, hf) in enumerate([(0, 0), (0, 1), (1, 0), (1, 1)]):
                    i0 = 4 * ct + 2 * hf
                    nc.tensor.matmul(
                        g_ps[:], gind_sb[:, 16 * ct : 16 * ct + 16],
                        stats[:, i0 : i0 + 2],
                        start=(i == 0), stop=(i == 3),
                    )
                mall = small.tile([16, 2], F32, tag="mall", name="mall")
                nc.vector.tensor_scalar_mul(mall[:], g_ps[:], 1.0 / (GC * HW))
                msq = small.tile([16, 1], F32, tag="msq", name="msq")
                nc.vector.tensor_tensor(msq[:], mall[:, 0:1], mall[:, 0:1], op=OP.mult)
                ve = small.tile([16, 1], F32, tag="ve", name="ve")
                nc.vector.tensor_tensor(ve[:], mall[:, 1:2], msq[:], op=OP.subtract)
                ve2 = small.tile([16, 1], F32, tag="ve2", name="ve2")
                nc.vector.tensor_scalar_add(ve2[:], ve[:], EPS)
                # rstd via DVE-only bit-trick rsqrt + 2 Newton steps (no ACT
                # table loads on the GN critical path)
                I32 = mybir.dt.int32
                vi = small.tile([16, 1], I32, tag="vi", name="vi")
                nc.vector.tensor_scalar(
                    vi[:], ve2[:].bitcast(I32), 1, None, op0=OP.arith_shift_right,
                )
                gvals = small.tile([16, 2], F32, tag="gvals", name="gvals")
                nc.vector.tensor_copy(gvals[:, 0:1], mall[:, 0:1])
                r = small.tile([16, 1], F32, tag="rs0", name="rs0")
                nc.vector.tensor_scalar(
                    r[:].bitcast(I32), vi[:], -1, 0x5F3759DF, op0=OP.mult, op1=OP.add,
                )
                for it in range(2):
                    t1 = small.tile([16, 1], F32, tag=f"rs{it}a", name=f"rs{it}a")
                    nc.vector.tensor_tensor(t1[:], r[:], r[:], op=OP.mult)
                    t2 = small.tile([16, 1], F32, tag=f"rs{it}b", name=f"rs{it}b")
                    nc.vector.tensor_tensor(t2[:], ve2[:], t1[:], op=OP.mult)
                    t3 = small.tile([16, 1], F32, tag=f"rs{it}c", name=f"rs{it}c")
                    nc.vector.tensor_scalar(t3[:], t2[:], -0.5, 1.5, op0=OP.mult, op1=OP.add)
                    dst = gvals[:, 1:2] if it == 1 else small.tile([16, 1], F32, tag=f"rs{it}d", name=f"rs{it}d")[:]
                    nc.vector.tensor_tensor(dst, r[:], t3[:], op=OP.mult)
                    if it == 0:
                        r = small.tile([16, 1], F32, tag="rs0d_", name="rs0d_")
                        nc.vector.tensor_copy(r[:], dst)
                for ct in range(2):
                    cv = ps_gn.tile([128, 2], F32, tag="cv", name="cv")
                    nc.tensor.matmul(
                        cv[:], gbc_sb[:, ct * 128 : (ct + 1) * 128], gvals[:],
                        start=True, stop=True,
                    )
                    scale_t = small.tile([128, 1], F32, tag="scale", name="scale")
                    nc.vector.tensor_tensor(scale_t[:], gnp_sb[ct][:, 0:1], cv[:, 1:2], op=OP.mult)
                    tb = small.tile([128, 1], F32, tag="tb", name="tb")
                    nc.vector.tensor_tensor(tb[:], cv[:, 0:1], scale_t[:], op=OP.mult)
                    bias_t = small.tile([128, 1], F32, tag="bias", name="bias")
                    nc.vector.tensor_tensor(bias_t[:], gnp_sb[ct][:, 1:2], tb[:], op=OP.subtract)
                    # one normalize-apply per engine so they run concurrently
                    if ct == 0:
                        nc.vector.tensor_scalar(
                            xn_sb[ct][:], x_sb[ct][:], scale_t[:], bias_t[:],
                            op0=OP.mult, op1=OP.add,
                        )
                    else:
                        nc.scalar.activation(
                            xn_sb[ct][:], x_sb[ct][:], AF.Identity,
                            bias=bias_t[:], scale=scale_t[:],
                        )

            # ---- QKV is woven into the attention loop (using the "s" psum
            # tag as scratch), so there is no separate projection phase ----
            k_sb = big.tile([128, HW], mm_dt, tag="k", name="k")
            q_t = [
                big.tile([128, i1 - i0], mm_dt, tag=f"q{ib}", name=f"q{ib}")
                for ib, (i0, i1) in enumerate(IBLKS)
            ]
            # per-head-pair merged output accumulators (4 big DMAs at the end)
            y_sb = [
                [big.tile([128, HW], mm_dt, tag=f"y{h}{mt}", name=f"y{h}{mt}") for mt in range(2)]
                for h in range(2)
            ]

            # v^T in fp8, laid out [128 spatial, NJ subtiles of VST]: subtile jc
            # holds chunk jc's [64 v-channels + ones column(s)].  Pair 2p,2p+1
            # forms the DoubleRow K=256 stationary operand.
            vt_sb, vt_v = [], []
            for h in range(2):
                t = big.tile([128, NJ * VST], F8, tag=f"vt{h}", name=f"vt{h}")
                nc.gpsimd.memset(t[:], 1.0)  # ones columns (and padding)
                vt_sb.append(t)
                vt_v.append(t[:].rearrange("p (j c) -> p j c", c=VST))

            # ---- attention ----
            # st gets 3 psum buffers (6 banks) so the scores->exp->WAR chain
            # never gates the pipeline; with u (2 banks) that is all of PSUM,
            # so the output projections run in a separate phase afterwards.
            ho_saved = []
            with (
                tc.tile_pool(name="ps_att", bufs=1, space=bass.MemorySpace.PSUM) as ps_att,
            ):
                def emit_epilogue(i0, i1, u):
                    blk = i1 - i0
                    hos = []
                    for h in range(2):
                        # u (64 channels + denominator row) psum -> sbuf bf16
                        ho = ptp.tile([PVM, blk], mm_dt, tag=f"ho{h}", name=f"ho{h}", bufs=5)
                        if h == 0:
                            nc.vector.tensor_copy(ho[:], u[h][:, 0:blk])
                        else:
                            nc.scalar.copy(ho[:], u[h][:, 0:blk])
                        nc.sync.dma_start(dn_d[h : h + 1, i0:i1], ho[HD : HD + 1, :])
                        hos.append(ho)
                    ho_saved.append((i0, i1, hos))

                def scratch_ps():
                    return ps_att.tile([128, 2 * SALIGN], F32, tag="s", name="s", bufs=3)

                def kjob(g):
                    a, b_ = 512 * g, min(512 * g + 512, HW)
                    ps = scratch_ps()
                    for kc in range(2):
                        nc.tensor.matmul(
                            ps[:, 0 : b_ - a], w_sb["wk", kc][:],
                            xn_sb[kc][:, a:b_], start=(kc == 0), stop=(kc == 1),
                        )
                    if g % 2 == 0:
                        nc.vector.tensor_copy(k_sb[:, a:b_], ps[:, 0 : b_ - a])
                    else:
                        nc.scalar.copy(k_sb[:, a:b_], ps[:, 0 : b_ - a])

                def qjob(ib_):
                    a, b_ = IBLKS[ib_]
                    ps = scratch_ps()
                    for kc in range(2):
                        nc.tensor.matmul(
                            ps[:, 0 : b_ - a], w_sb["wq", kc][:],
                            xn_sb[kc][:, a:b_], start=(kc == 0), stop=(kc == 1),
                        )
                    if ib_ % 2 == 0:
                        nc.vector.tensor_copy(q_t[ib_][:], ps[:, 0 : b_ - a])
                    else:
                        nc.scalar.copy(q_t[ib_][:], ps[:, 0 : b_ - a])

                def vjob(g):
                    chunks = list(range(4 * g, min(4 * g + 4, NJ)))
                    w = len(chunks) * JC
                    ps = scratch_ps()
                    for ci, jc in enumerate(chunks):
                        for kc in range(2):
                            nc.tensor.matmul(
                                ps[:, ci * JC : (ci + 1) * JC],
                                xn_sb[kc][:, jc * JC : (jc + 1) * JC],
                                w_sb["wv", kc][:],
                                start=(kc == 0), stop=(kc == 1),
                            )
                    vps3 = ps[:, 0:w].rearrange("p (j c) -> p j c", c=128)
                    for h in range(2):
                        nc.vector.tensor_copy(
                            vt_v[h][:, chunks[0] : chunks[0] + len(chunks), 0:HD],
                            vps3[:, :, h * HD : (h + 1) * HD],
                        )

                # jobs woven between pairs: (ib, pp) -> thunk.  Each job is
                # needed 1-2 pairs after its slot (scores/PV deps noted inline).
                jobs = {
                    (0, 0): lambda: kjob(1), (0, 1): lambda: vjob(1),
                    (0, 2): lambda: kjob(2), (0, 3): lambda: vjob(2),
                    (0, 4): lambda: kjob(3), (0, 5): lambda: vjob(3),
                    (0, 6): lambda: kjob(4), (0, 7): lambda: vjob(4),
                    (0, 8): lambda: qjob(1),
                    (1, 0): lambda: qjob(2), (1, 1): lambda: qjob(3),
                    (1, 2): lambda: qjob(4),
                }
                # minimal pre-attention set: first k/v chunk groups + iblk0's q
                kjob(0)
                vjob(0)
                qjob(0)

                for ib, (i0, i1) in enumerate(IBLKS):
                    blk = i1 - i0
                    # u[h]: [65, blk] accumulator (64 channels + denominator row)
                    u = [
                        ps_att.tile([PVM, SALIGN], F32, tag=f"u{h}", name=f"u{h}", bufs=1)
                        for h in range(2)
                    ]

                    def emit_s(jc):
                        # S^T chunk for both heads, row-tiled (concurrent on PE).
                        # h0/h1 outputs land in different psum banks.
                        st = ps_att.tile([128, 2 * SALIGN], F32, tag="s", name="s", bufs=3)
                        for h in range(2):
                            nc.tensor.matmul(
                                st[:, h * SALIGN : h * SALIGN + blk],
                                k_sb[h * HD : (h + 1) * HD, jc * JC : (jc + 1) * JC],
                                q_t[ib][h * HD : (h + 1) * HD, :],
                                start=True, stop=True,
                            )
                        return st

                    def emit_exp(jc, st, pair_v, s):
                        # pt8[:, h, s, :] = exp(st[:, h, :]/16) as fp8e4m3.
                        # slot 0 on ACT, slot 1 on DVE: the two exps of every
                        # pair run concurrently on different engines.
                        src = st[:].rearrange("p (h x) -> p h x", h=2)[:, :, 0:blk]
                        dst = pair_v[:, :, s, 0:blk]
                        if s == 0:
                            nc.scalar.activation(dst, src, AF.Exp, scale=1.0 / 16.0)
                        else:
                            nc.vector.tensor_scalar(
                                dst.bitcast(U8), src, EXPA, EXPB,
                                op0=OP.mult, op1=OP.add,
                            )

                    def emit_pv(pp, pair_v):
                        # DoubleRow fp8: contracts both chunks of the pair (K=256)
                        for h in range(2):
                            nc.tensor.matmul(
                                u[h][:, 0:blk],
                                vt_v[h][:, 2 * pp : 2 * pp + 2, 0:PVM],
                                pair_v[:, h, :, 0:blk],
                                start=(pp == 0), stop=(pp == NJP - 1),
                                perf_mode=PM.DoubleRow,
                            )

                    packed = 2 * blk <= SALIGN  # tail iblk: one exp per pair
                    prev_pair = None
                    for pp in range(NJP):
                        pt = ptp.tile([128, 4 * blk], F8, tag="pt8", name="pt8")
                        pair_v = pt[:].rearrange("p (h s x) -> p h s x", h=2, s=2)
                        if packed:
                            # both chunks' scores into one st tile -> single exp
                            st = ps_att.tile([128, 2 * SALIGN], F32, tag="s", name="s", bufs=3)
                            for s in range(2):
                                jc = 2 * pp + s
                                for h in range(2):
                                    nc.tensor.matmul(
                                        st[:, h * SALIGN + s * blk : h * SALIGN + (s + 1) * blk],
                                        k_sb[h * HD : (h + 1) * HD, jc * JC : (jc + 1) * JC],
                                        q_t[ib][h * HD : (h + 1) * HD, :],
                                        start=True, stop=True,
                                    )
                            src4 = st[:].rearrange("p (h s x) -> p h s x", h=2, s=2)
                            dst4 = pair_v[:, :, :, 0:blk]
                            if pp % 2 == 0:
                                nc.scalar.activation(dst4, src4, AF.Exp, scale=1.0 / 16.0)
                            else:
                                nc.vector.tensor_scalar(
                                    dst4.bitcast(U8), src4, EXPA, EXPB,
                                    op0=OP.mult, op1=OP.add,
                                )
                        else:
                            for s in range(2):
                                jc = 2 * pp + s
                                st = emit_s(jc)
                                emit_exp(jc, st, pair_v, s)
                        job = jobs.pop((ib, pp), None)
                        if job is not None:
                            job()
                        if pp > 0:
                            emit_pv(pp - 1, prev_pair)
                        prev_pair = pair_v
                    emit_pv(NJP - 1, prev_pair)
                    emit_epilogue(i0, i1, u)

            # ---- output projections (per head, unnormalized) + stores ----
            # copies land in the merged y_sb accumulators; four big DMAs at the
            # end (two per queue) replace 20 small serialized stores
            with tc.tile_pool(name="ps_fin", bufs=4, space=bass.MemorySpace.PSUM) as ps_fin:
                n = 0
                for i0, i1, hos in ho_saved:
                    blk = i1 - i0
                    for h in range(2):
                        for mt in range(2):
                            yp = ps_fin.tile([128, SALIGN], F32, tag="yp", name="yp")
                            nc.tensor.matmul(
                                yp[:, 0:blk],
                                wo_h[h][0:64, mt * 128 : (mt + 1) * 128],
                                hos[h][0:HD, :],
                                start=True, stop=True,
                            )
                            # 3 of 5 copies on DVE (0.5us) vs 2 on ACT (0.72us)
                            if n % 5 < 3:
                                nc.vector.tensor_copy(y_sb[h][mt][:, i0:i1], yp[:, 0:blk])
                            else:
                                nc.scalar.copy(y_sb[h][mt][:, i0:i1], yp[:, 0:blk])
                            n += 1
                for h in range(2):
                    for mt in range(2):
                        eng = nc.sync if mt == 0 else nc.scalar
                        eng.dma_start(
                            y_d[h][mt * 128 : (mt + 1) * 128, :], y_sb[h][mt][:],
                        )

    nc.compile()
    return nc


def _consts():
    # gind[:, 0:16]: tile-0 channel -> group one-hot; [:, 16:32]: tile-1 channel -> group
    gind = np.zeros((128, 32), np.float32)
    for c in range(128):
        gind[c, c // GC] = 1.0
        gind[c, 16 + 8 + c // GC] = 1.0
    gbc = np.zeros((16, C), np.float32)
    for c in range(C):
        gbc[c // GC, c] = 1.0
    return gind, gbc


def make_in_maps(x, gn_weight, gn_bias, qkv_w, out_w, out_b):
    x = np.asarray(x, np.float32)
    qkv_w = np.asarray(qkv_w, np.float32)
    out_w = np.asarray(out_w, np.float32)
    gn_weight = np.asarray(gn_weight, np.float32)
    gn_bias = np.asarray(gn_bias, np.float32)
    xr = np.ascontiguousarray(x.reshape(B, C, HW))
    gind, gbc = _consts()
    gnp = np.ascontiguousarray(np.stack([gn_weight, gn_bias], axis=1))
    in_maps = []
    for core in range(NCORES):
        b, hp = divmod(core, 2)
        heads = (2 * hp, 2 * hp + 1)
        qs = np.concatenate([qkv_w[n * 192 : n * 192 + 64] for n in heads], 0)
        ks = np.concatenate([qkv_w[n * 192 + 64 : n * 192 + 128] for n in heads], 0)
        vs = np.concatenate([qkv_w[n * 192 + 128 : n * 192 + 192] for n in heads], 0)
        in_maps.append({
            "x": xr[b],
            "wq": np.ascontiguousarray(qs.T),
            "wk": np.ascontiguousarray(ks.T),
            "wv": np.ascontiguousarray(vs.T),
            "wo": np.ascontiguousarray(out_w[:, hp * 128 : (hp + 1) * 128].T),
            "gnp": gnp,
            "gind": gind,
            "gbc": gbc,
        })
    return in_maps


def gather(results, x, out_b):
    """Host-side: divide per-head partials by softmax denominators, sum, add
    residual + bias."""
    x = np.asarray(x, np.float32)
    out_b = np.asarray(out_b, np.float32)
    xr = x.reshape(B, C, HW)
    y = np.empty((B, C, HW), np.float32)
    for b in range(B):
        acc = xr[b] + out_b[:, None]
        for hp in range(2):
            r = results[2 * b + hp]
            dns = np.asarray(r["dns"], np.float32)
            acc = acc + np.asarray(r["y0"], np.float32) / dns[0][None, :]
            acc = acc + np.asarray(r["y1"], np.float32) / dns[1][None, :]
        y[b] = acc
    return y.reshape(B, C, H, W)


_NC_CACHE = {}


def get_nc(mm_dt=BF16):
    key = str(mm_dt)
    if key not in _NC_CACHE:
        _NC_CACHE[key] = _build(mm_dt)
    return _NC_CACHE[key]


def kernel(x, gn_weight, gn_bias, qkv_w, out_w, out_b):
    nc = get_nc(BF16)
    in_maps = make_in_maps(x, gn_weight, gn_bias, qkv_w, out_w, out_b)
    res = bass_utils.run_bass_kernel_spmd(nc, in_maps, core_ids=list(range(NCORES)))
    return gather(res.results, x, out_b)


# revision 29
# speedup vs baseline: 1.4816x; 1.0412x over previous
"""Trainium2 Bass kernel for spatial attention (GroupNorm + QKV + softmax attention
+ output projection + residual), distributed over 8 NeuronCores.

Sharding: core = 2*b + hp handles image b (of 4) and head pair hp (heads 2hp, 2hp+1).
Each core computes GroupNorm(x[b]), its heads' q/k/v, full spatial attention for its
two heads, and per-head UNNORMALIZED partial output projections.  The softmax
denominators ship back with the partials; the host divides, sums the four partials
per image, and adds the residual + bias (cheap [C, HW] numpy ops, off the device
critical path).

Perf notes (v4):
- Scores bf16, row-tiled: both heads' S^T matmuls run concurrently on the PE.
- softmax exp split across engines: 5 of 9 key chunks on ACT (fp8 output),
  4 of 9 on the DVE via a Schraudolph bit-trick (uint8 = trunc(s*A+B)
  reinterpreted as fp8e4m3) — the 10.6M-element exp load is the bottleneck.
- PV runs as fp8 DoubleRow matmuls (two key chunks = K=256 per instruction),
  halving PV stream time; v^T carries a 65th all-ones column so the softmax
  denominator accumulates in the same matmul.
- No on-device normalize/residual: u (incl. denominator row) is copied psum->sbuf
  bf16, projected per head, and DMA'd out; host does the divides.
- v and q-half1 projections are interleaved INTO the first two attention iblks
  (sharing the ps_out psum tag) so the PE never idles while exp catches up.
"""

import math

import numpy as np

import concourse.bass as bass
import concourse.bacc as bacc
import concourse.tile as tile
from concourse import mybir
from concourse import bass_utils
from concourse.alu_op_type import AluOpType

B, C, H, W = 4, 256, 48, 48
HW = H * W  # 2304
NH, HD = 4, 64
G, GC = 16, 16  # 16 groups x 16 channels
EPS = 1e-5
NCORES = 8
JC = 128  # j (key spatial) chunk
NJ = HW // JC  # 18
NJP = NJ // 2  # 9 key-chunk pairs (DoubleRow K=256)
IBLKS = [(0, 512), (512, 1024), (1024, 1536), (1536, 2048), (2048, 2304)]
HALF = HW // 2  # 1152
QSPLIT = 1024  # q_sb split point (iblk-aligned)
PVM = HD + 1  # 65: 64 v channels + denominator ones row
VST = 80  # fp8 v^T subtile stride (16-byte aligned, >= PVM)
SALIGN = 512

F32 = mybir.dt.float32
BF16 = mybir.dt.bfloat16
F8 = mybir.dt.float8e4
U8 = mybir.dt.uint8
AX = mybir.AxisListType.X
AF = mybir.ActivationFunctionType
OP = AluOpType
PM = mybir.MatmulPerfMode

# Schraudolph exp into fp8e4m3 bit space: bits = trunc(s*EXPA + EXPB),
# value(bits) ~= exp(s/16).  EXPB tuned numerically for minimax rel err (~7%)
# assuming truncation on the DVE float->uint8 convert.
EXPA = 8.0 * math.log2(math.e) / 16.0
EXPB = 56.13


def _nchunks(size, step=512):
    # PSUM-bank-aligned chunks: a matmul output may not cross a 512-fp32 bank boundary
    return [(a, min(a + step, size)) for a in range(0, size, step)]


def _build(mm_dt=BF16):
    nc = bacc.Bacc("TRN2", target_bir_lowering=False, debug=False, enable_asserts=False)

    x_d = nc.dram_tensor("x", [C, HW], F32, kind="ExternalInput").ap()
    wq_d = nc.dram_tensor("wq", [C, 2 * HD], F32, kind="ExternalInput").ap()
    wk_d = nc.dram_tensor("wk", [C, 2 * HD], F32, kind="ExternalInput").ap()
    wv_d = nc.dram_tensor("wv", [C, 2 * HD], F32, kind="ExternalInput").ap()
    wo_d = nc.dram_tensor("wo", [2 * HD, C], F32, kind="ExternalInput").ap()
    gnp_d = nc.dram_tensor("gnp", [C, 2], F32, kind="ExternalInput").ap()
    gind_d = nc.dram_tensor("gind", [128, 32], F32, kind="ExternalInput").ap()
    gbc_d = nc.dram_tensor("gbc", [16, C], F32, kind="ExternalInput").ap()
    y_d = [
        nc.dram_tensor(f"y{h}", [C, HW], BF16, kind="ExternalOutput").ap()
        for h in range(2)
    ]
    dn_d = nc.dram_tensor("dns", [2, HW], BF16, kind="ExternalOutput").ap()

    with tile.TileContext(nc) as tc:
        with (
            tc.tile_pool(name="consts", bufs=1) as consts,
            tc.tile_pool(name="big", bufs=1) as big,
            tc.tile_pool(name="small", bufs=4) as small,
            tc.tile_pool(name="pt", bufs=3) as ptp,
        ):
            # ---- input x first (GN stats are the critical path) ----
            # halves go over both DMA queues (SP + ACT) in parallel
            x_sb, xn_sb = [], []
            for ct in range(2):
                t = big.tile([128, HW], F32, tag=f"x{ct}", name=f"x{ct}")
                for ci in range(4):
                    a, b_ = ci * (HW // 4), (ci + 1) * (HW // 4)
                    eng = nc.sync if ci % 2 == 0 else nc.scalar
                    eng.dma_start(t[:, a:b_], x_d[ct * 128 : (ct + 1) * 128, a:b_])
                x_sb.append(t)
                xn_sb.append(big.tile([128, HW], mm_dt, tag=f"xn{ct}", name=f"xn{ct}"))

            # ---- constant / weight loads ----
            gind_sb = consts.tile([128, 32], F32, tag="gind", name="gind")
            nc.sync.dma_start(gind_sb[:], gind_d[:])
            gbc_sb = consts.tile([16, C], F32, tag="gbc", name="gbc")
            nc.sync.dma_start(gbc_sb[:], gbc_d[:])
            gnp_sb = []
            for ct in range(2):
                t = consts.tile([128, 2], F32, tag=f"gnp{ct}", name=f"gnp{ct}")
                nc.sync.dma_start(t[:], gnp_d[ct * 128 : (ct + 1) * 128, :])
                gnp_sb.append(t)
            # dummy exp: forces the ACT exp table load NOW (overlapped with the
            # x DMA) instead of inside the GN/attention critical path
            warm = small.tile([128, 2], F32, tag="warm", name="warm")
            nc.scalar.activation(warm[:], gnp_sb[0][:], AF.Exp)
            w_sb = {}
            for name, d in (("wk", wk_d), ("wq", wq_d), ("wv", wv_d)):
                for kc in range(2):
                    tf = consts.tile([128, 2 * HD], F32, tag=f"{name}{kc}f", name=f"{name}{kc}f")
                    nc.sync.dma_start(tf[:], d[kc * 128 : (kc + 1) * 128, :])
                    t = consts.tile([128, 2 * HD], mm_dt, tag=f"{name}{kc}", name=f"{name}{kc}")
                    nc.vector.tensor_copy(t[:], tf[:])
                    w_sb[name, kc] = t
            # wo rows 0:64 (head 0) in place; rows 64:128 (head 1) also loaded at
            # base partition 0 so both heads' K=64 projections can stream from
            # partitions 0-63 (rhs = ho tile lives there).
            wof = consts.tile([128, C], F32, tag="wof", name="wof")
            nc.sync.dma_start(wof[:], wo_d[:])
            wo2f = consts.tile([64, C], F32, tag="wo2f", name="wo2f")
            nc.sync.dma_start(wo2f[:], wo_d[64:128, :])
            wo_sb = consts.tile([128, C], mm_dt, tag="wo", name="wo")
            nc.vector.tensor_copy(wo_sb[:], wof[:])
            wo2_sb = consts.tile([64, C], mm_dt, tag="wo2", name="wo2")
            nc.vector.tensor_copy(wo2_sb[:], wo2f[:])
            wo_h = {0: wo_sb, 1: wo2_sb}

            # ---- GroupNorm ----
            # per-channel sums on ACT (activation accumulate), sum-of-squares on
            # DVE (scalar_tensor_tensor accumulate) -> run concurrently.
            # activation output goes to xn_sb as scratch (overwritten below).
            stats = small.tile([128, 8], F32, tag="stats", name="stats")
            for ct in range(2):
                for hf in range(2):
                    sl = x_sb[ct][:, hf * HALF : (hf + 1) * HALF]
                    scratch = xn_sb[ct][:, hf * HALF : (hf + 1) * HALF]
                    i0 = 4 * ct + 2 * hf
                    nc.scalar.activation(
                        scratch, sl, AF.Copy, accum_out=stats[:, i0 : i0 + 1],
                    )
                    nc.vector.scalar_tensor_tensor(
                        scratch, sl, 1.0, sl,
                        op0=OP.mult, op1=OP.mult,
                        accum_out=stats[:, i0 + 1 : i0 + 2],
                    )
            with tc.tile_pool(name="ps_gn", bufs=2, space=bass.MemorySpace.PSUM) as ps_gn:
                # accumulate all four (ct, half) partial (sum, sumsq) into [16, 2]
                g_ps = ps_gn.tile([16, 2], F32, tag="g", name="g")
                for i, (ct, hf) in enumerate([(0, 0), (0, 1), (1, 0), (1, 1)]):
                    i0 = 4 * ct + 2 * hf
                    nc.tensor.matmul(
                        g_ps[:], gind_sb[:, 16 * ct : 16 * ct + 16],
                        stats[:, i0 : i0 + 2],
                        start=(i == 0), stop=(i == 3),
                    )
                mall = small.tile([16, 2], F32, tag="mall", name="mall")
                nc.vector.tensor_scalar_mul(mall[:], g_ps[:], 1.0 / (GC * HW))
                msq = small.tile([16, 1], F32, tag="msq", name="msq")
                nc.vector.tensor_tensor(msq[:], mall[:, 0:1], mall[:, 0:1], op=OP.mult)
                ve = small.tile([16, 1], F32, tag="ve", name="ve")
                nc.vector.tensor_tensor(ve[:], mall[:, 1:2], msq[:], op=OP.subtract)
                ve2 = small.tile([16, 1], F32, tag="ve2", name="ve2")
                nc.vector.tensor_scalar_add(ve2[:], ve[:], EPS)
                # rstd via DVE-only bit-trick rsqrt + 2 Newton steps (no ACT
                # table loads on the GN critical path)
                I32 = mybir.dt.int32
                vi = small.tile([16, 1], I32, tag="vi", name="vi")
                nc.vector.tensor_scalar(
                    vi[:], ve2[:].bitcast(I32), 1, None, op0=OP.arith_shift_right,
                )
                gvals = small.tile([16, 2], F32, tag="gvals", name="gvals")
                nc.vector.tensor_copy(gvals[:, 0:1], mall[:, 0:1])
                r = small.tile([16, 1], F32, tag="rs0", name="rs0")
                nc.vector.tensor_scalar(
                    r[:].bitcast(I32), vi[:], -1, 0x5F3759DF, op0=OP.mult, op1=OP.add,
                )
                for it in range(2):
                    t1 = small.tile([16, 1], F32, tag=f"rs{it}a", name=f"rs{it}a")
                    nc.vector.tensor_tensor(t1[:], r[:], r[:], op=OP.mult)
                    t2 = small.tile([16, 1], F32, tag=f"rs{it}b", name=f"rs{it}b")
                    nc.vector.tensor_tensor(t2[:], ve2[:], t1[:], op=OP.mult)
                    t3 = small.tile([16, 1], F32, tag=f"rs{it}c", name=f"rs{it}c")
                    nc.vector.tensor_scalar(t3[:], t2[:], -0.5, 1.5, op0=OP.mult, op1=OP.add)
                    rn = small.tile([16, 1], F32, tag=f"rs{it}d", name=f"rs{it}d")
                    nc.vector.tensor_tensor(
                        gvals[:, 1:2] if it == 1 else rn[:], r[:], t3[:], op=OP.mult,
                    )
                    r = rn
                for ct in range(2):
                    cv = ps_gn.tile([128, 2], F32, tag="cv", name="cv")
                    nc.tensor.matmul(
                        cv[:], gbc_sb[:, ct * 128 : (ct + 1) * 128], gvals[:],
                        start=True, stop=True,
                    )
                    scale_t = small.tile([128, 1], F32, tag="scale", name="scale")
                    nc.vector.tensor_tensor(scale_t[:], gnp_sb[ct][:, 0:1], cv[:, 1:2], op=OP.mult)
                    tb = small.tile([128, 1], F32, tag="tb", name="tb")
                    nc.vector.tensor_tensor(tb[:], cv[:, 0:1], scale_t[:], op=OP.mult)
                    bias_t = small.tile([128, 1], F32, tag="bias", name="bias")
                    nc.vector.tensor_tensor(bias_t[:], gnp_sb[ct][:, 1:2], tb[:], op=OP.subtract)
                    # one normalize-apply per engine so they run concurrently
                    if ct == 0:
                        nc.vector.tensor_scalar(
                            xn_sb[ct][:], x_sb[ct][:], scale_t[:], bias_t[:],
                            op0=OP.mult, op1=OP.add,
                        )
                    else:
                        nc.scalar.activation(
                            xn_sb[ct][:], x_sb[ct][:], AF.Identity,
                            bias=bias_t[:], scale=scale_t[:],
                        )

            # ---- QKV is woven into the attention loop (using the "s" psum
            # tag as scratch), so there is no separate projection phase ----
            k_sb = big.tile([128, HW], mm_dt, tag="k", name="k")
            q_t = [
                big.tile([128, i1 - i0], mm_dt, tag=f"q{ib}", name=f"q{ib}")
                for ib, (i0, i1) in enumerate(IBLKS)
            ]
            # per-head-pair merged output accumulators (4 big DMAs at the end)
            y_sb = [
                [big.tile([128, HW], mm_dt, tag=f"y{h}{mt}", name=f"y{h}{mt}") for mt in range(2)]
                for h in range(2)
            ]

            # v^T in fp8, laid out [128 spatial, NJ subtiles of VST]: subtile jc
            # holds chunk jc's [64 v-channels + ones column(s)].  Pair 2p,2p+1
            # forms the DoubleRow K=256 stationary operand.
            vt_sb, vt_v = [], []
            for h in range(2):
                t = big.tile([128, NJ * VST], F8, tag=f"vt{h}", name=f"vt{h}")
                nc.gpsimd.memset(t[:], 1.0)  # ones columns (and padding)
                vt_sb.append(t)
                vt_v.append(t[:].rearrange("p (j c) -> p j c", c=VST))

            # ---- attention ----
            # st gets 3 psum buffers (6 banks) so the scores->exp->WAR chain
            # never gates the pipeline; with u (2 banks) that is all of PSUM,
            # so the output projections run in a separate phase afterwards.
            ho_saved = []
            with (
                tc.tile_pool(name="ps_att", bufs=1, space=bass.MemorySpace.PSUM) as ps_att,
            ):
                def emit_epilogue(i0, i1, u):
                    blk = i1 - i0
                    hos = []
                    for h in range(2):
                        # u (64 channels + denominator row) psum -> sbuf bf16
                        ho = ptp.tile([PVM, blk], mm_dt, tag=f"ho{h}", name=f"ho{h}", bufs=5)
                        if h == 0:
                            nc.vector.tensor_copy(ho[:], u[h][:, 0:blk])
                        else:
                            nc.scalar.copy(ho[:], u[h][:, 0:blk])
                        nc.sync.dma_start(dn_d[h : h + 1, i0:i1], ho[HD : HD + 1, :])
                        hos.append(ho)
                    ho_saved.append((i0, i1, hos))

                def scratch_ps():
                    return ps_att.tile([128, 2 * SALIGN], F32, tag="s", name="s", bufs=3)

                def kjob(g):
                    a, b_ = 512 * g, min(512 * g + 512, HW)
                    ps = scratch_ps()
                    for kc in range(2):
                        nc.tensor.matmul(
                            ps[:, 0 : b_ - a], w_sb["wk", kc][:],
                            xn_sb[kc][:, a:b_], start=(kc == 0), stop=(kc == 1),
                        )
                    if g % 2 == 0:
                        nc.vector.tensor_copy(k_sb[:, a:b_], ps[:, 0 : b_ - a])
                    else:
                        nc.scalar.copy(k_sb[:, a:b_], ps[:, 0 : b_ - a])

                def qjob(ib_):
                    a, b_ = IBLKS[ib_]
                    ps = scratch_ps()
                    for kc in range(2):
                        nc.tensor.matmul(
                            ps[:, 0 : b_ - a], w_sb["wq", kc][:],
                            xn_sb[kc][:, a:b_], start=(kc == 0), stop=(kc == 1),
                        )
                    if ib_ % 2 == 0:
                        nc.vector.tensor_copy(q_t[ib_][:], ps[:, 0 : b_ - a])
                    else:
                        nc.scalar.copy(q_t[ib_][:], ps[:, 0 : b_ - a])

                def vjob(g):
                    chunks = list(range(4 * g, min(4 * g + 4, NJ)))
                    w = len(chunks) * JC
                    ps = scratch_ps()
                    for ci, jc in enumerate(chunks):
                        for kc in range(2):
                            nc.tensor.matmul(
                                ps[:, ci * JC : (ci + 1) * JC],
                                xn_sb[kc][:, jc * JC : (jc + 1) * JC],
                                w_sb["wv", kc][:],
                                start=(kc == 0), stop=(kc == 1),
                            )
                    vps3 = ps[:, 0:w].rearrange("p (j c) -> p j c", c=128)
                    for h in range(2):
                        nc.vector.tensor_copy(
                            vt_v[h][:, chunks[0] : chunks[0] + len(chunks), 0:HD],
                            vps3[:, :, h * HD : (h + 1) * HD],
                        )

                # jobs woven between pairs: (ib, pp) -> thunk.  Each job is
                # needed 1-2 pairs after its slot (scores/PV deps noted inline).
                jobs = {
                    (0, 0): lambda: kjob(1), (0, 1): lambda: vjob(1),
                    (0, 2): lambda: kjob(2), (0, 3): lambda: vjob(2),
                    (0, 4): lambda: kjob(3), (0, 5): lambda: vjob(3),
                    (0, 6): lambda: kjob(4), (0, 7): lambda: vjob(4),
                    (0, 8): lambda: qjob(1),
                    (1, 0): lambda: qjob(2), (2, 0): lambda: qjob(3),
                    (3, 0): lambda: qjob(4),
                }
                # minimal pre-attention set: first k/v chunk groups + iblk0's q
                kjob(0)
                vjob(0)
                qjob(0)

                for ib, (i0, i1) in enumerate(IBLKS):
                    blk = i1 - i0
                    # u[h]: [65, blk] accumulator (64 channels + denominator row)
                    u = [
                        ps_att.tile([PVM, SALIGN], F32, tag=f"u{h}", name=f"u{h}", bufs=1)
                        for h in range(2)
                    ]

                    def emit_s(jc):
                        # S^T chunk for both heads, row-tiled (concurrent on PE).
                        # h0/h1 outputs land in different psum banks.
                        st = ps_att.tile([128, 2 * SALIGN], F32, tag="s", name="s", bufs=3)
                        for h in range(2):
                            nc.tensor.matmul(
                                st[:, h * SALIGN : h * SALIGN + blk],
                                k_sb[h * HD : (h + 1) * HD, jc * JC : (jc + 1) * JC],
                                q_t[ib][h * HD : (h + 1) * HD, :],
                                start=True, stop=True,
                            )
                        return st

                    def emit_exp(jc, st, pair_v, s):
                        # pt8[:, h, s, :] = exp(st[:, h, :]/16) as fp8e4m3.
                        # slot 0 on ACT, slot 1 on DVE: the two exps of every
                        # pair run concurrently on different engines.
                        src = st[:].rearrange("p (h x) -> p h x", h=2)[:, :, 0:blk]
                        dst = pair_v[:, :, s, 0:blk]
                        if s == 0:
                            nc.scalar.activation(dst, src, AF.Exp, scale=1.0 / 16.0)
                        else:
                            nc.vector.tensor_scalar(
                                dst.bitcast(U8), src, EXPA, EXPB,
                                op0=OP.mult, op1=OP.add,
                            )

                    def emit_pv(pp, pair_v):
                        # DoubleRow fp8: contracts both chunks of the pair (K=256)
                        for h in range(2):
                            nc.tensor.matmul(
                                u[h][:, 0:blk],
                                vt_v[h][:, 2 * pp : 2 * pp + 2, 0:PVM],
                                pair_v[:, h, :, 0:blk],
                                start=(pp == 0), stop=(pp == NJP - 1),
                                perf_mode=PM.DoubleRow,
                            )

                    packed = 2 * blk <= SALIGN  # tail iblk: one exp per pair
                    prev_pair = None
                    for pp in range(NJP):
                        pt = ptp.tile([128, 4 * blk], F8, tag="pt8", name="pt8")
                        pair_v = pt[:].rearrange("p (h s x) -> p h s x", h=2, s=2)
                        if packed:
                            # both chunks' scores into one st tile -> single exp
                            st = ps_att.tile([128, 2 * SALIGN], F32, tag="s", name="s", bufs=3)
                            for s in range(2):
                                jc = 2 * pp + s
                                for h in range(2):
                                    nc.tensor.matmul(
                                        st[:, h * SALIGN + s * blk : h * SALIGN + (s + 1) * blk],
                                        k_sb[h * HD : (h + 1) * HD, jc * JC : (jc + 1) * JC],
                                        q_t[ib][h * HD : (h + 1) * HD, :],
                                        start=True, stop=True,
                                    )
                            src4 = st[:].rearrange("p (h s x) -> p h s x", h=2, s=2)
                            dst4 = pair_v[:, :, :, 0:blk]
                            if pp % 2 == 0:
                                nc.scalar.activation(dst4, src4, AF.Exp, scale=1.0 / 16.0)
                            else:
                                nc.vector.tensor_scalar(
                                    dst4.bitcast(U8), src4, EXPA, EXPB,
                                    op0=OP.mult, op1=OP.add,
                                )
                        else:
                            for s in range(2):
                                jc = 2 * pp + s
                                st = emit_s(jc)
                                emit_exp(jc, st, pair_v, s)
                        job = jobs.pop((ib, pp), None)
                        if job is not None:
                            job()
                        if pp > 0:
                            emit_pv(pp - 1, prev_pair)
                        prev_pair = pair_v
                    emit_pv(NJP - 1, prev_pair)
                    emit_epilogue(i0, i1, u)

            # ---- output projections (per head, unnormalized) + stores ----
            # copies land in the merged y_sb accumulators; four big DMAs at the
            # end (two per queue) replace 20 small serialized stores
            with tc.tile_pool(name="ps_fin", bufs=4, space=bass.MemorySpace.PSUM) as ps_fin:
                n = 0
                for i0, i1, hos in ho_saved:
                    blk = i1 - i0
                    for h in range(2):
                        for mt in range(2):
                            yp = ps_fin.tile([128, SALIGN], F32, tag="yp", name="yp")
                            nc.tensor.matmul(
                                yp[:, 0:blk],
                                wo_h[h][0:64, mt * 128 : (mt + 1) * 128],
                                hos[h][0:HD, :],
                                start=True, stop=True,
                            )
                            # 3 of 5 copies on DVE (0.5us) vs 2 on ACT (0.72us)
                            if n % 5 < 3:
                                nc.vector.tensor_copy(y_sb[h][mt][:, i0:i1], yp[:, 0:blk])
                            else:
                                nc.scalar.copy(y_sb[h][mt][:, i0:i1], yp[:, 0:blk])
                            n += 1
                    if i1 == 1024:
                        # first two iblks projected: ship y[:, 0:1024] now so the
                        # final drain only waits on the second wave
                        for h in range(2):
                            for mt in range(2):
                                eng = nc.sync if (h + mt) % 2 == 0 else nc.scalar
                                eng.dma_start(
                                    y_d[h][mt * 128 : (mt + 1) * 128, 0:1024],
                                    y_sb[h][mt][:, 0:1024],
                                )
                for h in range(2):
                    for mt in range(2):
                        eng = nc.sync if (h + mt) % 2 == 0 else nc.scalar
                        eng.dma_start(
                            y_d[h][mt * 128 : (mt + 1) * 128, 1024:HW],
                            y_sb[h][mt][:, 1024:HW],
                        )

    nc.compile()
    return nc


def _consts():
    # gind[:, 0:16]: tile-0 channel -> group one-hot; [:, 16:32]: tile-1 channel -> group
    gind = np.zeros((128, 32), np.float32)
    for c in range(128):
        gind[c, c // GC] = 1.0
        gind[c, 16 + 8 + c // GC] = 1.0
    gbc = np.zeros((16, C), np.float32)
    for c in range(C):
        gbc[c // GC, c] = 1.0
    return gind, gbc


def make_in_maps(x, gn_weight, gn_bias, qkv_w, out_w, out_b):
    x = np.asarray(x, np.float32)
    qkv_w = np.asarray(qkv_w, np.float32)
    out_w = np.asarray(out_w, np.float32)
    gn_weight = np.asarray(gn_weight, np.float32)
    gn_bias = np.asarray(gn_bias, np.float32)
    xr = np.ascontiguousarray(x.reshape(B, C, HW))
    gind, gbc = _consts()
    gnp = np.ascontiguousarray(np.stack([gn_weight, gn_bias], axis=1))
    in_maps = []
    for core in range(NCORES):
        b, hp = divmod(core, 2)
        heads = (2 * hp, 2 * hp + 1)
        qs = np.concatenate([qkv_w[n * 192 : n * 192 + 64] for n in heads], 0)
        ks = np.concatenate([qkv_w[n * 192 + 64 : n * 192 + 128] for n in heads], 0)
        vs = np.concatenate([qkv_w[n * 192 + 128 : n * 192 + 192] for n in heads], 0)
        in_maps.append({
            "x": xr[b],
            "wq": np.ascontiguousarray(qs.T),
            "wk": np.ascontiguousarray(ks.T),
            "wv": np.ascontiguousarray(vs.T),
            "wo": np.ascontiguousarray(out_w[:, hp * 128 : (hp + 1) * 128].T),
            "gnp": gnp,
            "gind": gind,
            "gbc": gbc,
        })
    return in_maps


def gather(results, x, out_b):
    """Host-side: divide per-head partials by softmax denominators, sum, add
    residual + bias."""
    x = np.asarray(x, np.float32)
    out_b = np.asarray(out_b, np.float32)
    xr = x.reshape(B, C, HW)
    y = np.empty((B, C, HW), np.float32)
    for b in range(B):
        acc = xr[b] + out_b[:, None]
        for hp in range(2):
            r = results[2 * b + hp]
            dns = np.asarray(r["dns"], np.float32)
            acc = acc + np.asarray(r["y0"], np.float32) / dns[0][None, :]
            acc = acc + np.asarray(r["y1"], np.float32) / dns[1][None, :]
        y[b] = acc
    return y.reshape(B, C, H, W)


_NC_CACHE = {}


def get_nc(mm_dt=BF16):
    key = str(mm_dt)
    if key not in _NC_CACHE:
        _NC_CACHE[key] = _build(mm_dt)
    return _NC_CACHE[key]


def kernel(x, gn_weight, gn_bias, qkv_w, out_w, out_b):
    nc = get_nc(BF16)
    in_maps = make_in_maps(x, gn_weight, gn_bias, qkv_w, out_w, out_b)
    res = bass_utils.run_bass_kernel_spmd(nc, in_maps, core_ids=list(range(NCORES)))
    return gather(res.results, x, out_b)


# revision 30
# speedup vs baseline: 1.4998x; 1.0123x over previous
"""Trainium2 Bass kernel for spatial attention (GroupNorm + QKV + softmax attention
+ output projection + residual), distributed over 8 NeuronCores.

Sharding: core = 2*b + hp handles image b (of 4) and head pair hp (heads 2hp, 2hp+1).
Each core computes GroupNorm(x[b]), its heads' q/k/v, full spatial attention for its
two heads, and per-head UNNORMALIZED partial output projections.  The softmax
denominators ship back with the partials; the host divides, sums the four partials
per image, and adds the residual + bias (cheap [C, HW] numpy ops, off the device
critical path).

Perf notes (v4):
- Scores bf16, row-tiled: both heads' S^T matmuls run concurrently on the PE.
- softmax exp split across engines: 5 of 9 key chunks on ACT (fp8 output),
  4 of 9 on the DVE via a Schraudolph bit-trick (uint8 = trunc(s*A+B)
  reinterpreted as fp8e4m3) — the 10.6M-element exp load is the bottleneck.
- PV runs as fp8 DoubleRow matmuls (two key chunks = K=256 per instruction),
  halving PV stream time; v^T carries a 65th all-ones column so the softmax
  denominator accumulates in the same matmul.
- No on-device normalize/residual: u (incl. denominator row) is copied psum->sbuf
  bf16, projected per head, and DMA'd out; host does the divides.
- v and q-half1 projections are interleaved INTO the first two attention iblks
  (sharing the ps_out psum tag) so the PE never idles while exp catches up.
"""

import math

import numpy as np

import concourse.bass as bass
import concourse.bacc as bacc
import concourse.tile as tile
from concourse import mybir
from concourse import bass_utils
from concourse.alu_op_type import AluOpType

B, C, H, W = 4, 256, 48, 48
HW = H * W  # 2304
NH, HD = 4, 64
G, GC = 16, 16  # 16 groups x 16 channels
EPS = 1e-5
NCORES = 8
JC = 128  # j (key spatial) chunk
NJ = HW // JC  # 18
NJP = NJ // 2  # 9 key-chunk pairs (DoubleRow K=256)
IBLKS = [(0, 512), (512, 1024), (1024, 1536), (1536, 2048), (2048, 2304)]
HALF = HW // 2  # 1152
QSPLIT = 1024  # q_sb split point (iblk-aligned)
PVM = HD + 1  # 65: 64 v channels + denominator ones row
VST = 80  # fp8 v^T subtile stride (16-byte aligned, >= PVM)
SALIGN = 512

F32 = mybir.dt.float32
BF16 = mybir.dt.bfloat16
F8 = mybir.dt.float8e4
U8 = mybir.dt.uint8
AX = mybir.AxisListType.X
AF = mybir.ActivationFunctionType
OP = AluOpType
PM = mybir.MatmulPerfMode

# Schraudolph exp into fp8e4m3 bit space: bits = trunc(s*EXPA + EXPB),
# value(bits) ~= exp(s/16).  EXPB tuned numerically for minimax rel err (~7%)
# assuming truncation on the DVE float->uint8 convert.
EXPA = 8.0 * math.log2(math.e) / 16.0
EXPB = 56.13


def _nchunks(size, step=512):
    # PSUM-bank-aligned chunks: a matmul output may not cross a 512-fp32 bank boundary
    return [(a, min(a + step, size)) for a in range(0, size, step)]


def _build(mm_dt=BF16):
    nc = bacc.Bacc("TRN2", target_bir_lowering=False, debug=False, enable_asserts=False)

    x_d = nc.dram_tensor("x", [C, HW], BF16, kind="ExternalInput").ap()
    wq_d = nc.dram_tensor("wq", [C, 2 * HD], F32, kind="ExternalInput").ap()
    wk_d = nc.dram_tensor("wk", [C, 2 * HD], F32, kind="ExternalInput").ap()
    wv_d = nc.dram_tensor("wv", [C, 2 * HD], F32, kind="ExternalInput").ap()
    wo_d = nc.dram_tensor("wo", [2 * HD, C], F32, kind="ExternalInput").ap()
    gnp_d = nc.dram_tensor("gnp", [C, 2], F32, kind="ExternalInput").ap()
    gind_d = nc.dram_tensor("gind", [128, 32], F32, kind="ExternalInput").ap()
    gbc_d = nc.dram_tensor("gbc", [16, C], F32, kind="ExternalInput").ap()
    y_d = [
        nc.dram_tensor(f"y{h}", [C, HW], BF16, kind="ExternalOutput").ap()
        for h in range(2)
    ]
    dn_d = nc.dram_tensor("dns", [2, HW], BF16, kind="ExternalOutput").ap()

    with tile.TileContext(nc) as tc:
        with (
            tc.tile_pool(name="consts", bufs=1) as consts,
            tc.tile_pool(name="big", bufs=1) as big,
            tc.tile_pool(name="small", bufs=4) as small,
            tc.tile_pool(name="pt", bufs=3) as ptp,
        ):
            # ---- input x first (GN stats are the critical path) ----
            # halves go over both DMA queues (SP + ACT) in parallel
            x_sb, xn_sb = [], []
            for ct in range(2):
                t = big.tile([128, HW], BF16, tag=f"x{ct}", name=f"x{ct}")
                for ci in range(4):
                    a, b_ = ci * (HW // 4), (ci + 1) * (HW // 4)
                    eng = nc.sync if ci % 2 == 0 else nc.scalar
                    eng.dma_start(t[:, a:b_], x_d[ct * 128 : (ct + 1) * 128, a:b_])
                x_sb.append(t)
                xn_sb.append(big.tile([128, HW], mm_dt, tag=f"xn{ct}", name=f"xn{ct}"))

            # ---- constant / weight loads ----
            gind_sb = consts.tile([128, 32], F32, tag="gind", name="gind")
            nc.sync.dma_start(gind_sb[:], gind_d[:])
            gbc_sb = consts.tile([16, C], F32, tag="gbc", name="gbc")
            nc.sync.dma_start(gbc_sb[:], gbc_d[:])
            gnp_sb = []
            for ct in range(2):
                t = consts.tile([128, 2], F32, tag=f"gnp{ct}", name=f"gnp{ct}")
                nc.sync.dma_start(t[:], gnp_d[ct * 128 : (ct + 1) * 128, :])
                gnp_sb.append(t)
            # dummy exp: forces the ACT exp table load NOW (overlapped with the
            # x DMA) instead of inside the GN/attention critical path
            warm = small.tile([128, 2], F32, tag="warm", name="warm")
            nc.scalar.activation(warm[:], gnp_sb[0][:], AF.Exp)
            w_sb = {}
            for name, d in (("wk", wk_d), ("wq", wq_d), ("wv", wv_d)):
                for kc in range(2):
                    tf = consts.tile([128, 2 * HD], F32, tag=f"{name}{kc}f", name=f"{name}{kc}f")
                    nc.sync.dma_start(tf[:], d[kc * 128 : (kc + 1) * 128, :])
                    t = consts.tile([128, 2 * HD], mm_dt, tag=f"{name}{kc}", name=f"{name}{kc}")
                    nc.vector.tensor_copy(t[:], tf[:])
                    w_sb[name, kc] = t
            # wo rows 0:64 (head 0) in place; rows 64:128 (head 1) also loaded at
            # base partition 0 so both heads' K=64 projections can stream from
            # partitions 0-63 (rhs = ho tile lives there).
            wof = consts.tile([128, C], F32, tag="wof", name="wof")
            nc.sync.dma_start(wof[:], wo_d[:])
            wo2f = consts.tile([64, C], F32, tag="wo2f", name="wo2f")
            nc.sync.dma_start(wo2f[:], wo_d[64:128, :])
            wo_sb = consts.tile([128, C], mm_dt, tag="wo", name="wo")
            nc.vector.tensor_copy(wo_sb[:], wof[:])
            wo2_sb = consts.tile([64, C], mm_dt, tag="wo2", name="wo2")
            nc.vector.tensor_copy(wo2_sb[:], wo2f[:])
            wo_h = {0: wo_sb, 1: wo2_sb}

            # ---- GroupNorm ----
            # per-channel sums on ACT (activation accumulate), sum-of-squares on
            # DVE (scalar_tensor_tensor accumulate) -> run concurrently.
            # activation output goes to xn_sb as scratch (overwritten below).
            stats = small.tile([128, 8], F32, tag="stats", name="stats")
            for ct in range(2):
                for hf in range(2):
                    sl = x_sb[ct][:, hf * HALF : (hf + 1) * HALF]
                    scratch = xn_sb[ct][:, hf * HALF : (hf + 1) * HALF]
                    i0 = 4 * ct + 2 * hf
                    nc.scalar.activation(
                        scratch, sl, AF.Copy, accum_out=stats[:, i0 : i0 + 1],
                    )
                    nc.vector.scalar_tensor_tensor(
                        scratch, sl, 1.0, sl,
                        op0=OP.mult, op1=OP.mult,
                        accum_out=stats[:, i0 + 1 : i0 + 2],
                    )
            with tc.tile_pool(name="ps_gn", bufs=2, space=bass.MemorySpace.PSUM) as ps_gn:
                # accumulate all four (ct, half) partial (sum, sumsq) into [16, 2]
                g_ps = ps_gn.tile([16, 2], F32, tag="g", name="g")
                for i, (ct, hf) in enumerate([(0, 0), (0, 1), (1, 0), (1, 1)]):
                    i0 = 4 * ct + 2 * hf
                    nc.tensor.matmul(
                        g_ps[:], gind_sb[:, 16 * ct : 16 * ct + 16],
                        stats[:, i0 : i0 + 2],
                        start=(i == 0), stop=(i == 3),
                    )
                mall = small.tile([16, 2], F32, tag="mall", name="mall")
                nc.vector.tensor_scalar_mul(mall[:], g_ps[:], 1.0 / (GC * HW))
                msq = small.tile([16, 1], F32, tag="msq", name="msq")
                nc.vector.tensor_tensor(msq[:], mall[:, 0:1], mall[:, 0:1], op=OP.mult)
                ve = small.tile([16, 1], F32, tag="ve", name="ve")
                nc.vector.tensor_tensor(ve[:], mall[:, 1:2], msq[:], op=OP.subtract)
                ve2 = small.tile([16, 1], F32, tag="ve2", name="ve2")
                nc.vector.tensor_scalar_add(ve2[:], ve[:], EPS)
                # rstd via DVE-only bit-trick rsqrt + 2 Newton steps (no ACT
                # table loads on the GN critical path)
                I32 = mybir.dt.int32
                vi = small.tile([16, 1], I32, tag="vi", name="vi")
                nc.vector.tensor_scalar(
                    vi[:], ve2[:].bitcast(I32), 1, None, op0=OP.arith_shift_right,
                )
                gvals = small.tile([16, 2], F32, tag="gvals", name="gvals")
                nc.vector.tensor_copy(gvals[:, 0:1], mall[:, 0:1])
                r = small.tile([16, 1], F32, tag="rs0", name="rs0")
                nc.vector.tensor_scalar(
                    r[:].bitcast(I32), vi[:], -1, 0x5F3759DF, op0=OP.mult, op1=OP.add,
                )
                for it in range(2):
                    t1 = small.tile([16, 1], F32, tag=f"rs{it}a", name=f"rs{it}a")
                    nc.vector.tensor_tensor(t1[:], r[:], r[:], op=OP.mult)
                    t2 = small.tile([16, 1], F32, tag=f"rs{it}b", name=f"rs{it}b")
                    nc.vector.tensor_tensor(t2[:], ve2[:], t1[:], op=OP.mult)
                    t3 = small.tile([16, 1], F32, tag=f"rs{it}c", name=f"rs{it}c")
                    nc.vector.tensor_scalar(t3[:], t2[:], -0.5, 1.5, op0=OP.mult, op1=OP.add)
                    rn = small.tile([16, 1], F32, tag=f"rs{it}d", name=f"rs{it}d")
                    nc.vector.tensor_tensor(
                        gvals[:, 1:2] if it == 1 else rn[:], r[:], t3[:], op=OP.mult,
                    )
                    r = rn
                for ct in range(2):
                    cv = ps_gn.tile([128, 2], F32, tag="cv", name="cv")
                    nc.tensor.matmul(
                        cv[:], gbc_sb[:, ct * 128 : (ct + 1) * 128], gvals[:],
                        start=True, stop=True,
                    )
                    scale_t = small.tile([128, 1], F32, tag="scale", name="scale")
                    nc.vector.tensor_tensor(scale_t[:], gnp_sb[ct][:, 0:1], cv[:, 1:2], op=OP.mult)
                    tb = small.tile([128, 1], F32, tag="tb", name="tb")
                    nc.vector.tensor_tensor(tb[:], cv[:, 0:1], scale_t[:], op=OP.mult)
                    bias_t = small.tile([128, 1], F32, tag="bias", name="bias")
                    nc.vector.tensor_tensor(bias_t[:], gnp_sb[ct][:, 1:2], tb[:], op=OP.subtract)
                    # one normalize-apply per engine so they run concurrently
                    if ct == 0:
                        nc.vector.tensor_scalar(
                            xn_sb[ct][:], x_sb[ct][:], scale_t[:], bias_t[:],
                            op0=OP.mult, op1=OP.add,
                        )
                    else:
                        nc.scalar.activation(
                            xn_sb[ct][:], x_sb[ct][:], AF.Identity,
                            bias=bias_t[:], scale=scale_t[:],
                        )

            # ---- QKV is woven into the attention loop (using the "s" psum
            # tag as scratch), so there is no separate projection phase ----
            k_sb = big.tile([128, HW], mm_dt, tag="k", name="k")
            q_t = [
                big.tile([128, i1 - i0], mm_dt, tag=f"q{ib}", name=f"q{ib}")
                for ib, (i0, i1) in enumerate(IBLKS)
            ]
            # per-head-pair merged output accumulators (4 big DMAs at the end)
            y_sb = [
                [big.tile([128, HW], mm_dt, tag=f"y{h}{mt}", name=f"y{h}{mt}") for mt in range(2)]
                for h in range(2)
            ]

            # v^T in fp8, laid out [128 spatial, NJ subtiles of VST]: subtile jc
            # holds chunk jc's [64 v-channels + ones column(s)].  Pair 2p,2p+1
            # forms the DoubleRow K=256 stationary operand.
            vt_sb, vt_v = [], []
            for h in range(2):
                t = big.tile([128, NJ * VST], F8, tag=f"vt{h}", name=f"vt{h}")
                nc.gpsimd.memset(t[:], 1.0)  # ones columns (and padding)
                vt_sb.append(t)
                vt_v.append(t[:].rearrange("p (j c) -> p j c", c=VST))

            # ---- attention ----
            # st gets 3 psum buffers (6 banks) so the scores->exp->WAR chain
            # never gates the pipeline; with u (2 banks) that is all of PSUM,
            # so the output projections run in a separate phase afterwards.
            ho_saved = []
            with (
                tc.tile_pool(name="ps_att", bufs=1, space=bass.MemorySpace.PSUM) as ps_att,
            ):
                def emit_epilogue(i0, i1, u):
                    blk = i1 - i0
                    hos = []
                    for h in range(2):
                        # u (64 channels + denominator row) psum -> sbuf bf16
                        ho = ptp.tile([PVM, blk], mm_dt, tag=f"ho{h}", name=f"ho{h}", bufs=5)
                        if h == 0:
                            nc.vector.tensor_copy(ho[:], u[h][:, 0:blk])
                        else:
                            nc.scalar.copy(ho[:], u[h][:, 0:blk])
                        nc.sync.dma_start(dn_d[h : h + 1, i0:i1], ho[HD : HD + 1, :])
                        hos.append(ho)
                    ho_saved.append((i0, i1, hos))

                def scratch_ps():
                    return ps_att.tile([128, 2 * SALIGN], F32, tag="s", name="s", bufs=3)

                def kjob(g):
                    a, b_ = 512 * g, min(512 * g + 512, HW)
                    ps = scratch_ps()
                    for kc in range(2):
                        nc.tensor.matmul(
                            ps[:, 0 : b_ - a], w_sb["wk", kc][:],
                            xn_sb[kc][:, a:b_], start=(kc == 0), stop=(kc == 1),
                        )
                    if g % 2 == 0:
                        nc.vector.tensor_copy(k_sb[:, a:b_], ps[:, 0 : b_ - a])
                    else:
                        nc.scalar.copy(k_sb[:, a:b_], ps[:, 0 : b_ - a])

                def qjob(ib_):
                    a, b_ = IBLKS[ib_]
                    ps = scratch_ps()
                    for kc in range(2):
                        nc.tensor.matmul(
                            ps[:, 0 : b_ - a], w_sb["wq", kc][:],
                            xn_sb[kc][:, a:b_], start=(kc == 0), stop=(kc == 1),
                        )
                    if ib_ % 2 == 0:
                        nc.vector.tensor_copy(q_t[ib_][:], ps[:, 0 : b_ - a])
                    else:
                        nc.scalar.copy(q_t[ib_][:], ps[:, 0 : b_ - a])

                def vjob(g):
                    chunks = list(range(4 * g, min(4 * g + 4, NJ)))
                    w = len(chunks) * JC
                    ps = scratch_ps()
                    for ci, jc in enumerate(chunks):
                        for kc in range(2):
                            nc.tensor.matmul(
                                ps[:, ci * JC : (ci + 1) * JC],
                                xn_sb[kc][:, jc * JC : (jc + 1) * JC],
                                w_sb["wv", kc][:],
                                start=(kc == 0), stop=(kc == 1),
                            )
                    vps3 = ps[:, 0:w].rearrange("p (j c) -> p j c", c=128)
                    for h in range(2):
                        nc.vector.tensor_copy(
                            vt_v[h][:, chunks[0] : chunks[0] + len(chunks), 0:HD],
                            vps3[:, :, h * HD : (h + 1) * HD],
                        )

                # jobs woven between pairs: (ib, pp) -> thunk.  Each job is
                # needed 1-2 pairs after its slot (scores/PV deps noted inline).
                jobs = {
                    (0, 0): lambda: kjob(1), (0, 1): lambda: vjob(1),
                    (0, 2): lambda: kjob(2), (0, 3): lambda: vjob(2),
                    (0, 4): lambda: kjob(3), (0, 5): lambda: vjob(3),
                    (0, 6): lambda: kjob(4), (0, 7): lambda: vjob(4),
                    (0, 8): lambda: qjob(1),
                    (1, 0): lambda: qjob(2), (2, 0): lambda: qjob(3),
                    (3, 0): lambda: qjob(4),
                }
                # minimal pre-attention set: first k/v chunk groups + iblk0's q
                kjob(0)
                vjob(0)
                qjob(0)

                for ib, (i0, i1) in enumerate(IBLKS):
                    blk = i1 - i0
                    # u[h]: [65, blk] accumulator (64 channels + denominator row)
                    u = [
                        ps_att.tile([PVM, SALIGN], F32, tag=f"u{h}", name=f"u{h}", bufs=1)
                        for h in range(2)
                    ]

                    def emit_s(jc):
                        # S^T chunk for both heads, row-tiled (concurrent on PE).
                        # h0/h1 outputs land in different psum banks.
                        st = ps_att.tile([128, 2 * SALIGN], F32, tag="s", name="s", bufs=3)
                        for h in range(2):
                            nc.tensor.matmul(
                                st[:, h * SALIGN : h * SALIGN + blk],
                                k_sb[h * HD : (h + 1) * HD, jc * JC : (jc + 1) * JC],
                                q_t[ib][h * HD : (h + 1) * HD, :],
                                start=True, stop=True,
                            )
                        return st

                    def emit_exp(jc, st, pair_v, s):
                        # pt8[:, h, s, :] = exp(st[:, h, :]/16) as fp8e4m3.
                        # slot 0 on ACT, slot 1 on DVE: the two exps of every
                        # pair run concurrently on different engines.
                        src = st[:].rearrange("p (h x) -> p h x", h=2)[:, :, 0:blk]
                        dst = pair_v[:, :, s, 0:blk]
                        if s == 0:
                            nc.scalar.activation(dst, src, AF.Exp, scale=1.0 / 16.0)
                        else:
                            nc.vector.tensor_scalar(
                                dst.bitcast(U8), src, EXPA, EXPB,
                                op0=OP.mult, op1=OP.add,
                            )

                    def emit_pv(pp, pair_v):
                        # DoubleRow fp8: contracts both chunks of the pair (K=256)
                        for h in range(2):
                            nc.tensor.matmul(
                                u[h][:, 0:blk],
                                vt_v[h][:, 2 * pp : 2 * pp + 2, 0:PVM],
                                pair_v[:, h, :, 0:blk],
                                start=(pp == 0), stop=(pp == NJP - 1),
                                perf_mode=PM.DoubleRow,
                            )

                    packed = 2 * blk <= SALIGN  # tail iblk: one exp per pair
                    prev_pair = None
                    for pp in range(NJP):
                        pt = ptp.tile([128, 4 * blk], F8, tag="pt8", name="pt8")
                        pair_v = pt[:].rearrange("p (h s x) -> p h s x", h=2, s=2)
                        if packed:
                            # both chunks' scores into one st tile -> single exp
                            st = ps_att.tile([128, 2 * SALIGN], F32, tag="s", name="s", bufs=3)
                            for s in range(2):
                                jc = 2 * pp + s
                                for h in range(2):
                                    nc.tensor.matmul(
                                        st[:, h * SALIGN + s * blk : h * SALIGN + (s + 1) * blk],
                                        k_sb[h * HD : (h + 1) * HD, jc * JC : (jc + 1) * JC],
                                        q_t[ib][h * HD : (h + 1) * HD, :],
                                        start=True, stop=True,
                                    )
                            src4 = st[:].rearrange("p (h s x) -> p h s x", h=2, s=2)
                            dst4 = pair_v[:, :, :, 0:blk]
                            if pp % 2 == 0:
                                nc.scalar.activation(dst4, src4, AF.Exp, scale=1.0 / 16.0)
                            else:
                                nc.vector.tensor_scalar(
                                    dst4.bitcast(U8), src4, EXPA, EXPB,
                                    op0=OP.mult, op1=OP.add,
                                )
                        else:
                            for s in range(2):
                                jc = 2 * pp + s
                                st = emit_s(jc)
                                emit_exp(jc, st, pair_v, s)
                        job = jobs.pop((ib, pp), None)
                        if job is not None:
                            job()
                        if pp > 0:
                            emit_pv(pp - 1, prev_pair)
                        prev_pair = pair_v
                    emit_pv(NJP - 1, prev_pair)
                    emit_epilogue(i0, i1, u)

            # ---- output projections (per head, unnormalized) + stores ----
            # copies land in the merged y_sb accumulators; four big DMAs at the
            # end (two per queue) replace 20 small serialized stores
            with tc.tile_pool(name="ps_fin", bufs=4, space=bass.MemorySpace.PSUM) as ps_fin:
                n = 0
                for i0, i1, hos in ho_saved:
                    blk = i1 - i0
                    for h in range(2):
                        for mt in range(2):
                            yp = ps_fin.tile([128, SALIGN], F32, tag="yp", name="yp")
                            nc.tensor.matmul(
                                yp[:, 0:blk],
                                wo_h[h][0:64, mt * 128 : (mt + 1) * 128],
                                hos[h][0:HD, :],
                                start=True, stop=True,
                            )
                            # 3 of 5 copies on DVE (0.5us) vs 2 on ACT (0.72us)
                            if n % 5 < 3:
                                nc.vector.tensor_copy(y_sb[h][mt][:, i0:i1], yp[:, 0:blk])
                            else:
                                nc.scalar.copy(y_sb[h][mt][:, i0:i1], yp[:, 0:blk])
                            n += 1
                    if i1 == 1024:
                        # first two iblks projected: ship y[:, 0:1024] now so the
                        # final drain only waits on the second wave
                        for h in range(2):
                            for mt in range(2):
                                eng = nc.sync if (h + mt) % 2 == 0 else nc.scalar
                                eng.dma_start(
                                    y_d[h][mt * 128 : (mt + 1) * 128, 0:1024],
                                    y_sb[h][mt][:, 0:1024],
                                )
                for h in range(2):
                    for mt in range(2):
                        eng = nc.sync if (h + mt) % 2 == 0 else nc.scalar
                        eng.dma_start(
                            y_d[h][mt * 128 : (mt + 1) * 128, 1024:HW],
                            y_sb[h][mt][:, 1024:HW],
                        )

    nc.compile()
    return nc


def _consts():
    # gind[:, 0:16]: tile-0 channel -> group one-hot; [:, 16:32]: tile-1 channel -> group
    gind = np.zeros((128, 32), np.float32)
    for c in range(128):
        gind[c, c // GC] = 1.0
        gind[c, 16 + 8 + c // GC] = 1.0
    gbc = np.zeros((16, C), np.float32)
    for c in range(C):
        gbc[c // GC, c] = 1.0
    return gind, gbc


def make_in_maps(x, gn_weight, gn_bias, qkv_w, out_w, out_b):
    import ml_dtypes
    x = np.asarray(x, np.float32)
    qkv_w = np.asarray(qkv_w, np.float32)
    out_w = np.asarray(out_w, np.float32)
    gn_weight = np.asarray(gn_weight, np.float32)
    gn_bias = np.asarray(gn_bias, np.float32)
    xr = np.ascontiguousarray(x.reshape(B, C, HW).astype(ml_dtypes.bfloat16))
    gind, gbc = _consts()
    gnp = np.ascontiguousarray(np.stack([gn_weight, gn_bias], axis=1))
    in_maps = []
    for core in range(NCORES):
        b, hp = divmod(core, 2)
        heads = (2 * hp, 2 * hp + 1)
        qs = np.concatenate([qkv_w[n * 192 : n * 192 + 64] for n in heads], 0)
        ks = np.concatenate([qkv_w[n * 192 + 64 : n * 192 + 128] for n in heads], 0)
        vs = np.concatenate([qkv_w[n * 192 + 128 : n * 192 + 192] for n in heads], 0)
        in_maps.append({
            "x": xr[b],
            "wq": np.ascontiguousarray(qs.T),
            "wk": np.ascontiguousarray(ks.T),
            "wv": np.ascontiguousarray(vs.T),
            "wo": np.ascontiguousarray(out_w[:, hp * 128 : (hp + 1) * 128].T),
            "gnp": gnp,
            "gind": gind,
            "gbc": gbc,
        })
    return in_maps


def gather(results, x, out_b):
    """Host-side: divide per-head partials by softmax denominators, sum, add
    residual + bias."""
    x = np.asarray(x, np.float32)
    out_b = np.asarray(out_b, np.float32)
    xr = x.reshape(B, C, HW)
    y = np.empty((B, C, HW), np.float32)
    for b in range(B):
        acc = xr[b] + out_b[:, None]
        for hp in range(2):
            r = results[2 * b + hp]
            dns = np.asarray(r["dns"], np.float32)
            acc = acc + np.asarray(r["y0"], np.float32) / dns[0][None, :]
            acc = acc + np.asarray(r["y1"], np.float32) / dns[1][None, :]
        y[b] = acc
    return y.reshape(B, C, H, W)


_NC_CACHE = {}


def get_nc(mm_dt=BF16):
    key = str(mm_dt)
    if key not in _NC_CACHE:
        _NC_CACHE[key] = _build(mm_dt)
    return _NC_CACHE[key]


def kernel(x, gn_weight, gn_bias, qkv_w, out_w, out_b):
    nc = get_nc(BF16)
    in_maps = make_in_maps(x, gn_weight, gn_bias, qkv_w, out_w, out_b)
    res = bass_utils.run_bass_kernel_spmd(nc, in_maps, core_ids=list(range(NCORES)))
    return gather(res.results, x, out_b)


# revision 31
# speedup vs baseline: 1.5144x; 1.0098x over previous
"""Trainium2 Bass kernel for spatial attention (GroupNorm + QKV + softmax attention
+ output projection + residual), distributed over 8 NeuronCores.

Sharding: core = 2*b + hp handles image b (of 4) and head pair hp (heads 2hp, 2hp+1).
Each core computes GroupNorm(x[b]), its heads' q/k/v, full spatial attention for its
two heads, and per-head UNNORMALIZED partial output projections.  The softmax
denominators ship back with the partials; the host divides, sums the four partials
per image, and adds the residual + bias (cheap [C, HW] numpy ops, off the device
critical path).

Perf notes (v4):
- Scores bf16, row-tiled: both heads' S^T matmuls run concurrently on the PE.
- softmax exp split across engines: 5 of 9 key chunks on ACT (fp8 output),
  4 of 9 on the DVE via a Schraudolph bit-trick (uint8 = trunc(s*A+B)
  reinterpreted as fp8e4m3) — the 10.6M-element exp load is the bottleneck.
- PV runs as fp8 DoubleRow matmuls (two key chunks = K=256 per instruction),
  halving PV stream time; v^T carries a 65th all-ones column so the softmax
  denominator accumulates in the same matmul.
- No on-device normalize/residual: u (incl. denominator row) is copied psum->sbuf
  bf16, projected per head, and DMA'd out; host does the divides.
- v and q-half1 projections are interleaved INTO the first two attention iblks
  (sharing the ps_out psum tag) so the PE never idles while exp catches up.
"""

import math

import numpy as np

import concourse.bass as bass
import concourse.bacc as bacc
import concourse.tile as tile
from concourse import mybir
from concourse import bass_utils
from concourse.alu_op_type import AluOpType

B, C, H, W = 4, 256, 48, 48
HW = H * W  # 2304
NH, HD = 4, 64
G, GC = 16, 16  # 16 groups x 16 channels
EPS = 1e-5
NCORES = 8
JC = 128  # j (key spatial) chunk
NJ = HW // JC  # 18
NJP = NJ // 2  # 9 key-chunk pairs (DoubleRow K=256)
IBLKS = [(2048, 2304), (0, 512), (512, 1024), (1024, 1536), (1536, 2048)]
HALF = HW // 2  # 1152
QSPLIT = 1024  # q_sb split point (iblk-aligned)
PVM = HD + 1  # 65: 64 v channels + denominator ones row
VST = 80  # fp8 v^T subtile stride (16-byte aligned, >= PVM)
SALIGN = 512

F32 = mybir.dt.float32
BF16 = mybir.dt.bfloat16
F8 = mybir.dt.float8e4
U8 = mybir.dt.uint8
AX = mybir.AxisListType.X
AF = mybir.ActivationFunctionType
OP = AluOpType
PM = mybir.MatmulPerfMode

# Schraudolph exp into fp8e4m3 bit space: bits = trunc(s*EXPA + EXPB),
# value(bits) ~= exp(s/16).  EXPB tuned numerically for minimax rel err (~7%)
# assuming truncation on the DVE float->uint8 convert.
EXPA = 8.0 * math.log2(math.e) / 16.0
EXPB = 56.13


def _nchunks(size, step=512):
    # PSUM-bank-aligned chunks: a matmul output may not cross a 512-fp32 bank boundary
    return [(a, min(a + step, size)) for a in range(0, size, step)]


def _build(mm_dt=BF16):
    nc = bacc.Bacc("TRN2", target_bir_lowering=False, debug=False, enable_asserts=False)

    x_d = nc.dram_tensor("x", [C, HW], BF16, kind="ExternalInput").ap()
    wq_d = nc.dram_tensor("wq", [C, 2 * HD], F32, kind="ExternalInput").ap()
    wk_d = nc.dram_tensor("wk", [C, 2 * HD], F32, kind="ExternalInput").ap()
    wv_d = nc.dram_tensor("wv", [C, 2 * HD], F32, kind="ExternalInput").ap()
    wo_d = nc.dram_tensor("wo", [2 * HD, C], F32, kind="ExternalInput").ap()
    gnp_d = nc.dram_tensor("gnp", [C, 2], F32, kind="ExternalInput").ap()
    gind_d = nc.dram_tensor("gind", [128, 32], F32, kind="ExternalInput").ap()
    gbc_d = nc.dram_tensor("gbc", [16, C], F32, kind="ExternalInput").ap()
    y_d = [
        nc.dram_tensor(f"y{h}", [C, HW], BF16, kind="ExternalOutput").ap()
        for h in range(2)
    ]
    dn_d = nc.dram_tensor("dns", [2, HW], BF16, kind="ExternalOutput").ap()

    with tile.TileContext(nc) as tc:
        with (
            tc.tile_pool(name="consts", bufs=1) as consts,
            tc.tile_pool(name="big", bufs=1) as big,
            tc.tile_pool(name="small", bufs=4) as small,
            tc.tile_pool(name="pt", bufs=3) as ptp,
        ):
            # ---- input x first (GN stats are the critical path) ----
            # halves go over both DMA queues (SP + ACT) in parallel
            x_sb, xn_sb = [], []
            for ct in range(2):
                t = big.tile([128, HW], BF16, tag=f"x{ct}", name=f"x{ct}")
                for ci in range(4):
                    a, b_ = ci * (HW // 4), (ci + 1) * (HW // 4)
                    eng = nc.sync if ci % 2 == 0 else nc.scalar
                    eng.dma_start(t[:, a:b_], x_d[ct * 128 : (ct + 1) * 128, a:b_])
                x_sb.append(t)
                xn_sb.append(big.tile([128, HW], mm_dt, tag=f"xn{ct}", name=f"xn{ct}"))

            # ---- constant / weight loads ----
            gind_sb = consts.tile([128, 32], F32, tag="gind", name="gind")
            nc.sync.dma_start(gind_sb[:], gind_d[:])
            gbc_sb = consts.tile([16, C], F32, tag="gbc", name="gbc")
            nc.sync.dma_start(gbc_sb[:], gbc_d[:])
            gnp_sb = []
            for ct in range(2):
                t = consts.tile([128, 2], F32, tag=f"gnp{ct}", name=f"gnp{ct}")
                nc.sync.dma_start(t[:], gnp_d[ct * 128 : (ct + 1) * 128, :])
                gnp_sb.append(t)
            # dummy exp: forces the ACT exp table load NOW (overlapped with the
            # x DMA) instead of inside the GN/attention critical path
            warm = small.tile([128, 2], F32, tag="warm", name="warm")
            nc.scalar.activation(warm[:], gnp_sb[0][:], AF.Exp)
            w_sb = {}
            for name, d in (("wk", wk_d), ("wq", wq_d), ("wv", wv_d)):
                for kc in range(2):
                    tf = consts.tile([128, 2 * HD], F32, tag=f"{name}{kc}f", name=f"{name}{kc}f")
                    nc.sync.dma_start(tf[:], d[kc * 128 : (kc + 1) * 128, :])
                    t = consts.tile([128, 2 * HD], mm_dt, tag=f"{name}{kc}", name=f"{name}{kc}")
                    nc.vector.tensor_copy(t[:], tf[:])
                    w_sb[name, kc] = t
            # wo rows 0:64 (head 0) in place; rows 64:128 (head 1) also loaded at
            # base partition 0 so both heads' K=64 projections can stream from
            # partitions 0-63 (rhs = ho tile lives there).
            wof = consts.tile([128, C], F32, tag="wof", name="wof")
            nc.sync.dma_start(wof[:], wo_d[:])
            wo2f = consts.tile([64, C], F32, tag="wo2f", name="wo2f")
            nc.sync.dma_start(wo2f[:], wo_d[64:128, :])
            wo_sb = consts.tile([128, C], mm_dt, tag="wo", name="wo")
            nc.vector.tensor_copy(wo_sb[:], wof[:])
            wo2_sb = consts.tile([64, C], mm_dt, tag="wo2", name="wo2")
            nc.vector.tensor_copy(wo2_sb[:], wo2f[:])
            wo_h = {0: wo_sb, 1: wo2_sb}

            # ---- GroupNorm ----
            # per-channel sums on ACT (activation accumulate), sum-of-squares on
            # DVE (scalar_tensor_tensor accumulate) -> run concurrently.
            # activation output goes to xn_sb as scratch (overwritten below).
            stats = small.tile([128, 8], F32, tag="stats", name="stats")
            for ct in range(2):
                for hf in range(2):
                    sl = x_sb[ct][:, hf * HALF : (hf + 1) * HALF]
                    scratch = xn_sb[ct][:, hf * HALF : (hf + 1) * HALF]
                    i0 = 4 * ct + 2 * hf
                    nc.scalar.activation(
                        scratch, sl, AF.Copy, accum_out=stats[:, i0 : i0 + 1],
                    )
                    nc.vector.scalar_tensor_tensor(
                        scratch, sl, 1.0, sl,
                        op0=OP.mult, op1=OP.mult,
                        accum_out=stats[:, i0 + 1 : i0 + 2],
                    )
            with tc.tile_pool(name="ps_gn", bufs=2, space=bass.MemorySpace.PSUM) as ps_gn:
                # accumulate all four (ct, half) partial (sum, sumsq) into [16, 2]
                g_ps = ps_gn.tile([16, 2], F32, tag="g", name="g")
                for i, (ct, hf) in enumerate([(0, 0), (0, 1), (1, 0), (1, 1)]):
                    i0 = 4 * ct + 2 * hf
                    nc.tensor.matmul(
                        g_ps[:], gind_sb[:, 16 * ct : 16 * ct + 16],
                        stats[:, i0 : i0 + 2],
                        start=(i == 0), stop=(i == 3),
                    )
                mall = small.tile([16, 2], F32, tag="mall", name="mall")
                nc.vector.tensor_scalar_mul(mall[:], g_ps[:], 1.0 / (GC * HW))
                msq = small.tile([16, 1], F32, tag="msq", name="msq")
                nc.vector.tensor_tensor(msq[:], mall[:, 0:1], mall[:, 0:1], op=OP.mult)
                ve = small.tile([16, 1], F32, tag="ve", name="ve")
                nc.vector.tensor_tensor(ve[:], mall[:, 1:2], msq[:], op=OP.subtract)
                ve2 = small.tile([16, 1], F32, tag="ve2", name="ve2")
                nc.vector.tensor_scalar_add(ve2[:], ve[:], EPS)
                # rstd via DVE-only bit-trick rsqrt + 2 Newton steps (no ACT
                # table loads on the GN critical path)
                I32 = mybir.dt.int32
                vi = small.tile([16, 1], I32, tag="vi", name="vi")
                nc.vector.tensor_scalar(
                    vi[:], ve2[:].bitcast(I32), 1, None, op0=OP.arith_shift_right,
                )
                gvals = small.tile([16, 2], F32, tag="gvals", name="gvals")
                nc.vector.tensor_copy(gvals[:, 0:1], mall[:, 0:1])
                r = small.tile([16, 1], F32, tag="rs0", name="rs0")
                nc.vector.tensor_scalar(
                    r[:].bitcast(I32), vi[:], -1, 0x5F3759DF, op0=OP.mult, op1=OP.add,
                )
                for it in range(2):
                    t1 = small.tile([16, 1], F32, tag=f"rs{it}a", name=f"rs{it}a")
                    nc.vector.tensor_tensor(t1[:], r[:], r[:], op=OP.mult)
                    t2 = small.tile([16, 1], F32, tag=f"rs{it}b", name=f"rs{it}b")
                    nc.vector.tensor_tensor(t2[:], ve2[:], t1[:], op=OP.mult)
                    t3 = small.tile([16, 1], F32, tag=f"rs{it}c", name=f"rs{it}c")
                    nc.vector.tensor_scalar(t3[:], t2[:], -0.5, 1.5, op0=OP.mult, op1=OP.add)
                    rn = small.tile([16, 1], F32, tag=f"rs{it}d", name=f"rs{it}d")
                    nc.vector.tensor_tensor(
                        gvals[:, 1:2] if it == 1 else rn[:], r[:], t3[:], op=OP.mult,
                    )
                    r = rn
                for ct in range(2):
                    cv = ps_gn.tile([128, 2], F32, tag="cv", name="cv")
                    nc.tensor.matmul(
                        cv[:], gbc_sb[:, ct * 128 : (ct + 1) * 128], gvals[:],
                        start=True, stop=True,
                    )
                    scale_t = small.tile([128, 1], F32, tag="scale", name="scale")
                    nc.vector.tensor_tensor(scale_t[:], gnp_sb[ct][:, 0:1], cv[:, 1:2], op=OP.mult)
                    tb = small.tile([128, 1], F32, tag="tb", name="tb")
                    nc.vector.tensor_tensor(tb[:], cv[:, 0:1], scale_t[:], op=OP.mult)
                    bias_t = small.tile([128, 1], F32, tag="bias", name="bias")
                    nc.vector.tensor_tensor(bias_t[:], gnp_sb[ct][:, 1:2], tb[:], op=OP.subtract)
                    # one normalize-apply per engine so they run concurrently
                    if ct == 0:
                        nc.vector.tensor_scalar(
                            xn_sb[ct][:], x_sb[ct][:], scale_t[:], bias_t[:],
                            op0=OP.mult, op1=OP.add,
                        )
                    else:
                        nc.scalar.activation(
                            xn_sb[ct][:], x_sb[ct][:], AF.Identity,
                            bias=bias_t[:], scale=scale_t[:],
                        )

            # ---- QKV is woven into the attention loop (using the "s" psum
            # tag as scratch), so there is no separate projection phase ----
            k_sb = big.tile([128, HW], mm_dt, tag="k", name="k")
            q_t = [
                big.tile([128, i1 - i0], mm_dt, tag=f"q{ib}", name=f"q{ib}")
                for ib, (i0, i1) in enumerate(IBLKS)
            ]
            # per-head-pair merged output accumulators (4 big DMAs at the end)
            y_sb = [
                [big.tile([128, HW], mm_dt, tag=f"y{h}{mt}", name=f"y{h}{mt}") for mt in range(2)]
                for h in range(2)
            ]

            # v^T in fp8, laid out [128 spatial, NJ subtiles of VST]: subtile jc
            # holds chunk jc's [64 v-channels + ones column(s)].  Pair 2p,2p+1
            # forms the DoubleRow K=256 stationary operand.
            vt_sb, vt_v = [], []
            for h in range(2):
                t = big.tile([128, NJ * VST], F8, tag=f"vt{h}", name=f"vt{h}")
                nc.gpsimd.memset(t[:], 1.0)  # ones columns (and padding)
                vt_sb.append(t)
                vt_v.append(t[:].rearrange("p (j c) -> p j c", c=VST))

            # ---- attention ----
            # st gets 3 psum buffers (6 banks) so the scores->exp->WAR chain
            # never gates the pipeline; with u (2 banks) that is all of PSUM,
            # so the output projections run in a separate phase afterwards.
            ho_saved = []
            with (
                tc.tile_pool(name="ps_att", bufs=1, space=bass.MemorySpace.PSUM) as ps_att,
            ):
                def emit_epilogue(i0, i1, u):
                    blk = i1 - i0
                    hos = []
                    for h in range(2):
                        # u (64 channels + denominator row) psum -> sbuf bf16
                        ho = ptp.tile([PVM, blk], mm_dt, tag=f"ho{h}", name=f"ho{h}", bufs=5)
                        if h == 0:
                            nc.vector.tensor_copy(ho[:], u[h][:, 0:blk])
                        else:
                            nc.scalar.copy(ho[:], u[h][:, 0:blk])
                        nc.sync.dma_start(dn_d[h : h + 1, i0:i1], ho[HD : HD + 1, :])
                        hos.append(ho)
                    ho_saved.append((i0, i1, hos))

                def scratch_ps():
                    return ps_att.tile([128, 2 * SALIGN], F32, tag="s", name="s", bufs=3)

                def kjob(g):
                    a, b_ = 512 * g, min(512 * g + 512, HW)
                    ps = scratch_ps()
                    for kc in range(2):
                        nc.tensor.matmul(
                            ps[:, 0 : b_ - a], w_sb["wk", kc][:],
                            xn_sb[kc][:, a:b_], start=(kc == 0), stop=(kc == 1),
                        )
                    if g % 2 == 0:
                        nc.vector.tensor_copy(k_sb[:, a:b_], ps[:, 0 : b_ - a])
                    else:
                        nc.scalar.copy(k_sb[:, a:b_], ps[:, 0 : b_ - a])

                def qjob(ib_):
                    a, b_ = IBLKS[ib_]
                    ps = scratch_ps()
                    for kc in range(2):
                        nc.tensor.matmul(
                            ps[:, 0 : b_ - a], w_sb["wq", kc][:],
                            xn_sb[kc][:, a:b_], start=(kc == 0), stop=(kc == 1),
                        )
                    if ib_ % 2 == 0:
                        nc.vector.tensor_copy(q_t[ib_][:], ps[:, 0 : b_ - a])
                    else:
                        nc.scalar.copy(q_t[ib_][:], ps[:, 0 : b_ - a])

                def vjob(g):
                    chunks = list(range(4 * g, min(4 * g + 4, NJ)))
                    w = len(chunks) * JC
                    ps = scratch_ps()
                    for ci, jc in enumerate(chunks):
                        for kc in range(2):
                            nc.tensor.matmul(
                                ps[:, ci * JC : (ci + 1) * JC],
                                xn_sb[kc][:, jc * JC : (jc + 1) * JC],
                                w_sb["wv", kc][:],
                                start=(kc == 0), stop=(kc == 1),
                            )
                    vps3 = ps[:, 0:w].rearrange("p (j c) -> p j c", c=128)
                    for h in range(2):
                        nc.vector.tensor_copy(
                            vt_v[h][:, chunks[0] : chunks[0] + len(chunks), 0:HD],
                            vps3[:, :, h * HD : (h + 1) * HD],
                        )

                # jobs woven between pairs: (ib, pp) -> thunk.  Each job is
                # needed 1-2 pairs after its slot (scores/PV deps noted inline).
                jobs = {
                    (0, 0): lambda: kjob(1), (0, 1): lambda: vjob(1),
                    (0, 2): lambda: kjob(2), (0, 3): lambda: vjob(2),
                    (0, 4): lambda: kjob(3), (0, 5): lambda: vjob(3),
                    (0, 6): lambda: kjob(4), (0, 7): lambda: vjob(4),
                    (0, 8): lambda: qjob(1),
                    (1, 0): lambda: qjob(2), (2, 0): lambda: qjob(3),
                    (3, 0): lambda: qjob(4),
                }
                # minimal pre-attention set; k/q first (iblk0's scores need
                # their copies), v's casts can trail into pair 0
                kjob(0)
                qjob(0)
                vjob(0)

                for ib, (i0, i1) in enumerate(IBLKS):
                    blk = i1 - i0
                    # u[h]: [65, blk] accumulator (64 channels + denominator row)
                    u = [
                        ps_att.tile([PVM, SALIGN], F32, tag=f"u{h}", name=f"u{h}", bufs=1)
                        for h in range(2)
                    ]

                    def emit_s(jc):
                        # S^T chunk for both heads, row-tiled (concurrent on PE).
                        # h0/h1 outputs land in different psum banks.
                        st = ps_att.tile([128, 2 * SALIGN], F32, tag="s", name="s", bufs=3)
                        for h in range(2):
                            nc.tensor.matmul(
                                st[:, h * SALIGN : h * SALIGN + blk],
                                k_sb[h * HD : (h + 1) * HD, jc * JC : (jc + 1) * JC],
                                q_t[ib][h * HD : (h + 1) * HD, :],
                                start=True, stop=True,
                            )
                        return st

                    def emit_exp(jc, st, pair_v, s):
                        # pt8[:, h, s, :] = exp(st[:, h, :]/16) as fp8e4m3.
                        # slot 0 on ACT, slot 1 on DVE: the two exps of every
                        # pair run concurrently on different engines.
                        src = st[:].rearrange("p (h x) -> p h x", h=2)[:, :, 0:blk]
                        dst = pair_v[:, :, s, 0:blk]
                        if s == 0:
                            nc.scalar.activation(dst, src, AF.Exp, scale=1.0 / 16.0)
                        else:
                            nc.vector.tensor_scalar(
                                dst.bitcast(U8), src, EXPA, EXPB,
                                op0=OP.mult, op1=OP.add,
                            )

                    def emit_pv(pp, pair_v):
                        # DoubleRow fp8: contracts both chunks of the pair (K=256)
                        for h in range(2):
                            nc.tensor.matmul(
                                u[h][:, 0:blk],
                                vt_v[h][:, 2 * pp : 2 * pp + 2, 0:PVM],
                                pair_v[:, h, :, 0:blk],
                                start=(pp == 0), stop=(pp == NJP - 1),
                                perf_mode=PM.DoubleRow,
                            )

                    packed = 2 * blk <= SALIGN  # tail iblk: one exp per pair
                    prev_pair = None
                    for pp in range(NJP):
                        pt = ptp.tile([128, 4 * blk], F8, tag="pt8", name="pt8")
                        pair_v = pt[:].rearrange("p (h s x) -> p h s x", h=2, s=2)
                        if packed:
                            # both chunks' scores into one st tile -> single exp
                            st = ps_att.tile([128, 2 * SALIGN], F32, tag="s", name="s", bufs=3)
                            for s in range(2):
                                jc = 2 * pp + s
                                for h in range(2):
                                    nc.tensor.matmul(
                                        st[:, h * SALIGN + s * blk : h * SALIGN + (s + 1) * blk],
                                        k_sb[h * HD : (h + 1) * HD, jc * JC : (jc + 1) * JC],
                                        q_t[ib][h * HD : (h + 1) * HD, :],
                                        start=True, stop=True,
                                    )
                            src4 = st[:].rearrange("p (h s x) -> p h s x", h=2, s=2)
                            dst4 = pair_v[:, :, :, 0:blk]
                            if pp % 2 == 0:
                                nc.scalar.activation(dst4, src4, AF.Exp, scale=1.0 / 16.0)
                            else:
                                nc.vector.tensor_scalar(
                                    dst4.bitcast(U8), src4, EXPA, EXPB,
                                    op0=OP.mult, op1=OP.add,
                                )
                        else:
                            for s in range(2):
                                jc = 2 * pp + s
                                st = emit_s(jc)
                                emit_exp(jc, st, pair_v, s)
                        job = jobs.pop((ib, pp), None)
                        if job is not None:
                            job()
                        if pp > 0:
                            emit_pv(pp - 1, prev_pair)
                        prev_pair = pair_v
                    emit_pv(NJP - 1, prev_pair)
                    emit_epilogue(i0, i1, u)

            # ---- output projections (per head, unnormalized) + stores ----
            # copies land in the merged y_sb accumulators; four big DMAs at the
            # end (two per queue) replace 20 small serialized stores
            with tc.tile_pool(name="ps_fin", bufs=4, space=bass.MemorySpace.PSUM) as ps_fin:
                n = 0
                for i0, i1, hos in ho_saved:
                    blk = i1 - i0
                    for h in range(2):
                        for mt in range(2):
                            yp = ps_fin.tile([128, SALIGN], F32, tag="yp", name="yp")
                            nc.tensor.matmul(
                                yp[:, 0:blk],
                                wo_h[h][0:64, mt * 128 : (mt + 1) * 128],
                                hos[h][0:HD, :],
                                start=True, stop=True,
                            )
                            # 3 of 5 copies on DVE (0.5us) vs 2 on ACT (0.72us)
                            if n % 5 < 3:
                                nc.vector.tensor_copy(y_sb[h][mt][:, i0:i1], yp[:, 0:blk])
                            else:
                                nc.scalar.copy(y_sb[h][mt][:, i0:i1], yp[:, 0:blk])
                            n += 1
                    if i1 == 1024:
                        # first two iblks projected: ship y[:, 0:1024] now so the
                        # final drain only waits on the second wave
                        for h in range(2):
                            for mt in range(2):
                                eng = nc.sync if (h + mt) % 2 == 0 else nc.scalar
                                eng.dma_start(
                                    y_d[h][mt * 128 : (mt + 1) * 128, 0:1024],
                                    y_sb[h][mt][:, 0:1024],
                                )
                for h in range(2):
                    for mt in range(2):
                        eng = nc.sync if (h + mt) % 2 == 0 else nc.scalar
                        eng.dma_start(
                            y_d[h][mt * 128 : (mt + 1) * 128, 1024:HW],
                            y_sb[h][mt][:, 1024:HW],
                        )

    nc.compile()
    return nc


def _consts():
    # gind[:, 0:16]: tile-0 channel -> group one-hot; [:, 16:32]: tile-1 channel -> group
    gind = np.zeros((128, 32), np.float32)
    for c in range(128):
        gind[c, c // GC] = 1.0
        gind[c, 16 + 8 + c // GC] = 1.0
    gbc = np.zeros((16, C), np.float32)
    for c in range(C):
        gbc[c // GC, c] = 1.0
    return gind, gbc


def make_in_maps(x, gn_weight, gn_bias, qkv_w, out_w, out_b):
    import ml_dtypes
    x = np.asarray(x, np.float32)
    qkv_w = np.asarray(qkv_w, np.float32)
    out_w = np.asarray(out_w, np.float32)
    gn_weight = np.asarray(gn_weight, np.float32)
    gn_bias = np.asarray(gn_bias, np.float32)
    xr = np.ascontiguousarray(x.reshape(B, C, HW).astype(ml_dtypes.bfloat16))
    gind, gbc = _consts()
    gnp = np.ascontiguousarray(np.stack([gn_weight, gn_bias], axis=1))
    in_maps = []
    for core in range(NCORES):
        b, hp = divmod(core, 2)
        heads = (2 * hp, 2 * hp + 1)
        qs = np.concatenate([qkv_w[n * 192 : n * 192 + 64] for n in heads], 0)
        ks = np.concatenate([qkv_w[n * 192 + 64 : n * 192 + 128] for n in heads], 0)
        vs = np.concatenate([qkv_w[n * 192 + 128 : n * 192 + 192] for n in heads], 0)
        in_maps.append({
            "x": xr[b],
            "wq": np.ascontiguousarray(qs.T),
            "wk": np.ascontiguousarray(ks.T),
            "wv": np.ascontiguousarray(vs.T),
            "wo": np.ascontiguousarray(out_w[:, hp * 128 : (hp + 1) * 128].T),
            "gnp": gnp,
            "gind": gind,
            "gbc": gbc,
        })
    return in_maps


def gather(results, x, out_b):
    """Host-side: divide per-head partials by softmax denominators, sum, add
    residual + bias."""
    x = np.asarray(x, np.float32)
    out_b = np.asarray(out_b, np.float32)
    xr = x.reshape(B, C, HW)
    y = np.empty((B, C, HW), np.float32)
    for b in range(B):
        acc = xr[b] + out_b[:, None]
        for hp in range(2):
            r = results[2 * b + hp]
            dns = np.asarray(r["dns"], np.float32)
            acc = acc + np.asarray(r["y0"], np.float32) / dns[0][None, :]
            acc = acc + np.asarray(r["y1"], np.float32) / dns[1][None, :]
        y[b] = acc
    return y.reshape(B, C, H, W)


_NC_CACHE = {}


def get_nc(mm_dt=BF16):
    key = str(mm_dt)
    if key not in _NC_CACHE:
        _NC_CACHE[key] = _build(mm_dt)
    return _NC_CACHE[key]


def kernel(x, gn_weight, gn_bias, qkv_w, out_w, out_b):
    nc = get_nc(BF16)
    in_maps = make_in_maps(x, gn_weight, gn_bias, qkv_w, out_w, out_b)
    res = bass_utils.run_bass_kernel_spmd(nc, in_maps, core_ids=list(range(NCORES)))
    return gather(res.results, x, out_b)


# revision 33
# speedup vs baseline: 1.5331x; 1.0123x over previous
"""Trainium2 Bass kernel for spatial attention (GroupNorm + QKV + softmax attention
+ output projection + residual), distributed over 8 NeuronCores.

Sharding: core = 2*b + hp handles image b (of 4) and head pair hp (heads 2hp, 2hp+1).
Each core computes GroupNorm(x[b]), its heads' q/k/v, full spatial attention for its
two heads, and per-head UNNORMALIZED partial output projections.  The softmax
denominators ship back with the partials; the host divides, sums the four partials
per image, and adds the residual + bias (cheap [C, HW] numpy ops, off the device
critical path).

Perf notes (v10, ~111us vs 162us baseline):
- Scores bf16, row-tiled: both heads' S^T matmuls run concurrently on the PE.
- softmax exp (10.6M elements/core — the bottleneck) split per key-chunk pair:
  slot 0 on ACT (fp8 output), slot 1 on the DVE via a Schraudolph bit-trick
  (uint8 = trunc(s*A+B) reinterpreted as fp8e4m3), so both run concurrently.
- Scores psum gets 3 buffers (6 banks) so the scores->exp buffer-reuse chain
  never gates the pipeline (it was the critical path at 2 buffers).
- PV runs as fp8 DoubleRow matmuls (two key chunks = K=256 per instruction),
  halving PV stream time; v^T carries a 65th all-ones column so the softmax
  denominator accumulates in the same matmul.
- No on-device normalize/residual: u (incl. denominator row) is copied psum->sbuf
  bf16, projected per head (unnormalized), and DMA'd out; host divides by the
  denominators and adds the residual.
- ALL of q/k/v projection is woven into the attention loop as jobs using the
  scores psum tag as scratch (no separate QKV phase); x ships as bf16 to halve
  the input DMA; output projections run in a final phase with two DMA waves.
- The small 256-wide query block runs FIRST (it absorbs the woven QKV jobs on
  a light base load and keeps the PE warm into the projection phase).
"""

import math

import numpy as np

import concourse.bass as bass
import concourse.bacc as bacc
import concourse.tile as tile
from concourse import mybir
from concourse import bass_utils
from concourse.alu_op_type import AluOpType

B, C, H, W = 4, 256, 48, 48
HW = H * W  # 2304
NH, HD = 4, 64
G, GC = 16, 16  # 16 groups x 16 channels
EPS = 1e-5
NCORES = 8
JC = 128  # j (key spatial) chunk
NJ = HW // JC  # 18
NJP = NJ // 2  # 9 key-chunk pairs (DoubleRow K=256)
IBLKS = [(2048, 2304), (0, 512), (512, 1024), (1024, 1536), (1536, 2048)]
HALF = HW // 2  # 1152
QSPLIT = 1024  # q_sb split point (iblk-aligned)
PVM = HD + 1  # 65: 64 v channels + denominator ones row
VST = 80  # fp8 v^T subtile stride (16-byte aligned, >= PVM)
SALIGN = 512

F32 = mybir.dt.float32
BF16 = mybir.dt.bfloat16
F8 = mybir.dt.float8e4
U8 = mybir.dt.uint8
AX = mybir.AxisListType.X
AF = mybir.ActivationFunctionType
OP = AluOpType
PM = mybir.MatmulPerfMode

# Schraudolph exp into fp8e4m3 bit space: bits = trunc(s*EXPA + EXPB),
# value(bits) ~= exp(s/16).  EXPB tuned numerically for minimax rel err (~7%)
# assuming truncation on the DVE float->uint8 convert.
EXPA = 8.0 * math.log2(math.e) / 16.0
EXPB = 56.13


def _nchunks(size, step=512):
    # PSUM-bank-aligned chunks: a matmul output may not cross a 512-fp32 bank boundary
    return [(a, min(a + step, size)) for a in range(0, size, step)]


def _build(mm_dt=BF16):
    nc = bacc.Bacc("TRN2", target_bir_lowering=False, debug=False, enable_asserts=False)

    x_d = nc.dram_tensor("x", [C, HW], BF16, kind="ExternalInput").ap()
    wq_d = nc.dram_tensor("wq", [C, 2 * HD], F32, kind="ExternalInput").ap()
    wk_d = nc.dram_tensor("wk", [C, 2 * HD], F32, kind="ExternalInput").ap()
    wv_d = nc.dram_tensor("wv", [C, 2 * HD], F32, kind="ExternalInput").ap()
    wo_d = nc.dram_tensor("wo", [2 * HD, C], F32, kind="ExternalInput").ap()
    gnp_d = nc.dram_tensor("gnp", [C, 2], F32, kind="ExternalInput").ap()
    gind_d = nc.dram_tensor("gind", [128, 32], F32, kind="ExternalInput").ap()
    gbc_d = nc.dram_tensor("gbc", [16, C], F32, kind="ExternalInput").ap()
    y_d = [
        nc.dram_tensor(f"y{h}", [C, HW], BF16, kind="ExternalOutput").ap()
        for h in range(2)
    ]
    dn_d = nc.dram_tensor("dns", [2, HW], BF16, kind="ExternalOutput").ap()

    with tile.TileContext(nc) as tc:
        with (
            tc.tile_pool(name="consts", bufs=1) as consts,
            tc.tile_pool(name="big", bufs=1) as big,
            tc.tile_pool(name="small", bufs=4) as small,
            tc.tile_pool(name="pt", bufs=3) as ptp,
        ):
            # ---- input x first (GN stats are the critical path) ----
            # halves go over both DMA queues (SP + ACT) in parallel
            x_sb, xn_sb = [], []
            for ct in range(2):
                t = big.tile([128, HW], BF16, tag=f"x{ct}", name=f"x{ct}")
                for ci in range(4):
                    a, b_ = ci * (HW // 4), (ci + 1) * (HW // 4)
                    eng = nc.sync if ci % 2 == 0 else nc.scalar
                    eng.dma_start(t[:, a:b_], x_d[ct * 128 : (ct + 1) * 128, a:b_])
                x_sb.append(t)
                xn_sb.append(big.tile([128, HW], mm_dt, tag=f"xn{ct}", name=f"xn{ct}"))

            # ---- constant / weight loads ----
            gind_sb = consts.tile([128, 32], F32, tag="gind", name="gind")
            nc.sync.dma_start(gind_sb[:], gind_d[:])
            gbc_sb = consts.tile([16, C], F32, tag="gbc", name="gbc")
            nc.sync.dma_start(gbc_sb[:], gbc_d[:])
            gnp_sb = []
            for ct in range(2):
                t = consts.tile([128, 2], F32, tag=f"gnp{ct}", name=f"gnp{ct}")
                nc.sync.dma_start(t[:], gnp_d[ct * 128 : (ct + 1) * 128, :])
                gnp_sb.append(t)
            # dummy exp: forces the ACT exp table load NOW (overlapped with the
            # x DMA) instead of inside the GN/attention critical path
            warm = small.tile([128, 2], F32, tag="warm", name="warm")
            nc.scalar.activation(warm[:], gnp_sb[0][:], AF.Exp)
            w_sb = {}
            for name, d in (("wk", wk_d), ("wq", wq_d), ("wv", wv_d)):
                for kc in range(2):
                    tf = consts.tile([128, 2 * HD], F32, tag=f"{name}{kc}f", name=f"{name}{kc}f")
                    nc.sync.dma_start(tf[:], d[kc * 128 : (kc + 1) * 128, :])
                    t = consts.tile([128, 2 * HD], mm_dt, tag=f"{name}{kc}", name=f"{name}{kc}")
                    nc.vector.tensor_copy(t[:], tf[:])
                    w_sb[name, kc] = t
            # wo rows 0:64 (head 0) in place; rows 64:128 (head 1) also loaded at
            # base partition 0 so both heads' K=64 projections can stream from
            # partitions 0-63 (rhs = ho tile lives there).
            wof = consts.tile([128, C], F32, tag="wof", name="wof")
            nc.sync.dma_start(wof[:], wo_d[:])
            wo2f = consts.tile([64, C], F32, tag="wo2f", name="wo2f")
            nc.sync.dma_start(wo2f[:], wo_d[64:128, :])
            wo_sb = consts.tile([128, C], mm_dt, tag="wo", name="wo")
            nc.vector.tensor_copy(wo_sb[:], wof[:])
            wo2_sb = consts.tile([64, C], mm_dt, tag="wo2", name="wo2")
            nc.vector.tensor_copy(wo2_sb[:], wo2f[:])
            wo_h = {0: wo_sb, 1: wo2_sb}

            # ---- GroupNorm ----
            # per-channel sums on ACT (activation accumulate), sum-of-squares on
            # DVE (scalar_tensor_tensor accumulate) -> run concurrently.
            # activation output goes to xn_sb as scratch (overwritten below).
            stats = small.tile([128, 8], F32, tag="stats", name="stats")
            for ct in range(2):
                for hf in range(2):
                    sl = x_sb[ct][:, hf * HALF : (hf + 1) * HALF]
                    scratch = xn_sb[ct][:, hf * HALF : (hf + 1) * HALF]
                    i0 = 4 * ct + 2 * hf
                    nc.scalar.activation(
                        scratch, sl, AF.Copy, accum_out=stats[:, i0 : i0 + 1],
                    )
                    nc.vector.scalar_tensor_tensor(
                        scratch, sl, 1.0, sl,
                        op0=OP.mult, op1=OP.mult,
                        accum_out=stats[:, i0 + 1 : i0 + 2],
                    )
            with tc.tile_pool(name="ps_gn", bufs=2, space=bass.MemorySpace.PSUM) as ps_gn:
                # accumulate all four (ct, half) partial (sum, sumsq) into [16, 2]
                g_ps = ps_gn.tile([16, 2], F32, tag="g", name="g")
                for i, (ct, hf) in enumerate([(0, 0), (0, 1), (1, 0), (1, 1)]):
                    i0 = 4 * ct + 2 * hf
                    nc.tensor.matmul(
                        g_ps[:], gind_sb[:, 16 * ct : 16 * ct + 16],
                        stats[:, i0 : i0 + 2],
                        start=(i == 0), stop=(i == 3),
                    )
                mall = small.tile([16, 2], F32, tag="mall", name="mall")
                nc.vector.tensor_scalar_mul(mall[:], g_ps[:], 1.0 / (GC * HW))
                # nmsq = -mean^2;  ve2 = (nmsq + EPS) + meansq
                nmsq = small.tile([16, 1], F32, tag="msq", name="msq")
                nc.vector.scalar_tensor_tensor(
                    nmsq[:], mall[:, 0:1], -1.0, mall[:, 0:1], op0=OP.mult, op1=OP.mult,
                )
                ve2 = small.tile([16, 1], F32, tag="ve2", name="ve2")
                nc.vector.scalar_tensor_tensor(
                    ve2[:], nmsq[:], EPS, mall[:, 1:2], op0=OP.add, op1=OP.add,
                )
                # rstd via DVE-only bit-trick rsqrt + 2 Newton steps (no ACT
                # table loads on the GN critical path)
                I32 = mybir.dt.int32
                vi = small.tile([16, 1], I32, tag="vi", name="vi")
                nc.vector.tensor_scalar(
                    vi[:], ve2[:].bitcast(I32), 1, None, op0=OP.arith_shift_right,
                )
                gvals = small.tile([16, 2], F32, tag="gvals", name="gvals")
                nc.vector.tensor_copy(gvals[:, 0:1], mall[:, 0:1])
                r = small.tile([16, 1], F32, tag="rs0", name="rs0")
                nc.vector.tensor_scalar(
                    r[:].bitcast(I32), vi[:], -1, 0x5F3759DF, op0=OP.mult, op1=OP.add,
                )
                for it in range(2):
                    t2 = small.tile([16, 1], F32, tag=f"rs{it}b", name=f"rs{it}b")
                    nc.vector.scalar_tensor_tensor(
                        t2[:], r[:], 1.0, r[:], op0=OP.mult, op1=OP.mult,
                    )
                    t3 = small.tile([16, 1], F32, tag=f"rs{it}c", name=f"rs{it}c")
                    nc.vector.scalar_tensor_tensor(
                        t3[:], t2[:], -0.5, ve2[:], op0=OP.mult, op1=OP.mult,
                    )
                    nc.vector.tensor_scalar(t3[:], t3[:], 1.0, 1.5, op0=OP.mult, op1=OP.add)
                    rn = small.tile([16, 1], F32, tag=f"rs{it}d", name=f"rs{it}d")
                    nc.vector.tensor_tensor(
                        gvals[:, 1:2] if it == 1 else rn[:], r[:], t3[:], op=OP.mult,
                    )
                    r = rn
                for ct in range(2):
                    cv = ps_gn.tile([128, 2], F32, tag="cv", name="cv")
                    nc.tensor.matmul(
                        cv[:], gbc_sb[:, ct * 128 : (ct + 1) * 128], gvals[:],
                        start=True, stop=True,
                    )
                    scale_t = small.tile([128, 1], F32, tag="scale", name="scale")
                    nc.vector.tensor_tensor(scale_t[:], gnp_sb[ct][:, 0:1], cv[:, 1:2], op=OP.mult)
                    tb = small.tile([128, 1], F32, tag="tb", name="tb")
                    nc.vector.tensor_tensor(tb[:], cv[:, 0:1], scale_t[:], op=OP.mult)
                    bias_t = small.tile([128, 1], F32, tag="bias", name="bias")
                    nc.vector.tensor_tensor(bias_t[:], gnp_sb[ct][:, 1:2], tb[:], op=OP.subtract)
                    # bf16 in/out hits the DVE 2x path (~0.9us each) — both
                    # applies on DVE beat one 2.3us ACT Identity
                    nc.vector.tensor_scalar(
                        xn_sb[ct][:], x_sb[ct][:], scale_t[:], bias_t[:],
                        op0=OP.mult, op1=OP.add,
                    )

            # ---- QKV is woven into the attention loop (using the "s" psum
            # tag as scratch), so there is no separate projection phase ----
            k_sb = big.tile([128, HW], mm_dt, tag="k", name="k")
            q_t = [
                big.tile([128, i1 - i0], mm_dt, tag=f"q{ib}", name=f"q{ib}")
                for ib, (i0, i1) in enumerate(IBLKS)
            ]
            # per-head-pair merged output accumulators (4 big DMAs at the end)
            y_sb = [
                [big.tile([128, HW], mm_dt, tag=f"y{h}{mt}", name=f"y{h}{mt}") for mt in range(2)]
                for h in range(2)
            ]

            # v^T in fp8, laid out [128 spatial, NJ subtiles of VST]: subtile jc
            # holds chunk jc's [64 v-channels + ones column(s)].  Pair 2p,2p+1
            # forms the DoubleRow K=256 stationary operand.
            vt_sb, vt_v = [], []
            for h in range(2):
                t = big.tile([128, NJ * VST], F8, tag=f"vt{h}", name=f"vt{h}")
                nc.gpsimd.memset(t[:], 1.0)  # ones columns (and padding)
                vt_sb.append(t)
                vt_v.append(t[:].rearrange("p (j c) -> p j c", c=VST))

            # ---- attention ----
            # st gets 3 psum buffers (6 banks) so the scores->exp->WAR chain
            # never gates the pipeline; with u (2 banks) that is all of PSUM,
            # so the output projections run in a separate phase afterwards.
            ho_saved = []
            with (
                tc.tile_pool(name="ps_att", bufs=1, space=bass.MemorySpace.PSUM) as ps_att,
            ):
                def emit_epilogue(i0, i1, u):
                    blk = i1 - i0
                    hos = []
                    for h in range(2):
                        # u (64 channels + denominator row) psum -> sbuf bf16
                        ho = ptp.tile([PVM, blk], mm_dt, tag=f"ho{h}", name=f"ho{h}", bufs=5)
                        if h == 0:
                            nc.vector.tensor_copy(ho[:], u[h][:, 0:blk])
                        else:
                            nc.scalar.copy(ho[:], u[h][:, 0:blk])
                        nc.sync.dma_start(dn_d[h : h + 1, i0:i1], ho[HD : HD + 1, :])
                        hos.append(ho)
                    ho_saved.append((i0, i1, hos))

                def scratch_ps():
                    return ps_att.tile([128, 2 * SALIGN], F32, tag="s", name="s", bufs=3)

                def kjob(g):
                    a, b_ = 512 * g, min(512 * g + 512, HW)
                    ps = scratch_ps()
                    for kc in range(2):
                        nc.tensor.matmul(
                            ps[:, 0 : b_ - a], w_sb["wk", kc][:],
                            xn_sb[kc][:, a:b_], start=(kc == 0), stop=(kc == 1),
                        )
                    if g % 2 == 0:
                        nc.vector.tensor_copy(k_sb[:, a:b_], ps[:, 0 : b_ - a])
                    else:
                        nc.scalar.copy(k_sb[:, a:b_], ps[:, 0 : b_ - a])

                def qjob(ib_):
                    a, b_ = IBLKS[ib_]
                    ps = scratch_ps()
                    for kc in range(2):
                        nc.tensor.matmul(
                            ps[:, 0 : b_ - a], w_sb["wq", kc][:],
                            xn_sb[kc][:, a:b_], start=(kc == 0), stop=(kc == 1),
                        )
                    if ib_ % 2 == 0:
                        nc.vector.tensor_copy(q_t[ib_][:], ps[:, 0 : b_ - a])
                    else:
                        nc.scalar.copy(q_t[ib_][:], ps[:, 0 : b_ - a])

                def vjob(g):
                    chunks = list(range(4 * g, min(4 * g + 4, NJ)))
                    w = len(chunks) * JC
                    ps = scratch_ps()
                    for ci, jc in enumerate(chunks):
                        for kc in range(2):
                            nc.tensor.matmul(
                                ps[:, ci * JC : (ci + 1) * JC],
                                xn_sb[kc][:, jc * JC : (jc + 1) * JC],
                                w_sb["wv", kc][:],
                                start=(kc == 0), stop=(kc == 1),
                            )
                    vps3 = ps[:, 0:w].rearrange("p (j c) -> p j c", c=128)
                    for h in range(2):
                        nc.vector.tensor_copy(
                            vt_v[h][:, chunks[0] : chunks[0] + len(chunks), 0:HD],
                            vps3[:, :, h * HD : (h + 1) * HD],
                        )

                # jobs woven between pairs: (ib, pp) -> thunk.  Each job is
                # needed 1-2 pairs after its slot (scores/PV deps noted inline).
                jobs = {
                    (0, 0): lambda: kjob(1), (0, 1): lambda: vjob(1),
                    (0, 2): lambda: kjob(2), (0, 3): lambda: vjob(2),
                    (0, 4): lambda: kjob(3), (0, 5): lambda: vjob(3),
                    (0, 6): lambda: kjob(4), (0, 7): lambda: vjob(4),
                    (0, 8): lambda: qjob(1),
                    (1, 0): lambda: qjob(2), (2, 0): lambda: qjob(3),
                    (3, 0): lambda: qjob(4),
                }
                # minimal pre-attention set; k/q first (iblk0's scores need
                # their copies), v's casts can trail into pair 0
                kjob(0)
                qjob(0)
                vjob(0)

                for ib, (i0, i1) in enumerate(IBLKS):
                    blk = i1 - i0
                    # u[h]: [65, blk] accumulator (64 channels + denominator row)
                    u = [
                        ps_att.tile([PVM, SALIGN], F32, tag=f"u{h}", name=f"u{h}", bufs=1)
                        for h in range(2)
                    ]

                    def emit_s(jc):
                        # S^T chunk for both heads, row-tiled (concurrent on PE).
                        # h0/h1 outputs land in different psum banks.
                        st = ps_att.tile([128, 2 * SALIGN], F32, tag="s", name="s", bufs=3)
                        for h in range(2):
                            nc.tensor.matmul(
                                st[:, h * SALIGN : h * SALIGN + blk],
                                k_sb[h * HD : (h + 1) * HD, jc * JC : (jc + 1) * JC],
                                q_t[ib][h * HD : (h + 1) * HD, :],
                                start=True, stop=True,
                            )
                        return st

                    def emit_exp(jc, st, pair_v, s):
                        # pt8[:, h, s, :] = exp(st[:, h, :]/16) as fp8e4m3.
                        # slot 0 on ACT, slot 1 on DVE: the two exps of every
                        # pair run concurrently on different engines.
                        src = st[:].rearrange("p (h x) -> p h x", h=2)[:, :, 0:blk]
                        dst = pair_v[:, s, :, 0:blk]
                        if s == 0:
                            nc.scalar.activation(dst, src, AF.Exp, scale=1.0 / 16.0)
                        else:
                            nc.vector.tensor_scalar(
                                dst.bitcast(U8), src, EXPA, EXPB,
                                op0=OP.mult, op1=OP.add,
                            )

                    def emit_pv(pp, pair_v):
                        # DoubleRow fp8: contracts both chunks of the pair (K=256)
                        for h in range(2):
                            nc.tensor.matmul(
                                u[h][:, 0:blk],
                                vt_v[h][:, 2 * pp : 2 * pp + 2, 0:PVM],
                                pair_v[:, :, h, 0:blk],
                                start=(pp == 0), stop=(pp == NJP - 1),
                                perf_mode=PM.DoubleRow,
                            )

                    packed = 2 * blk <= SALIGN  # tail iblk: one exp per pair
                    prev_pair = None
                    for pp in range(NJP):
                        pt = ptp.tile([128, 4 * blk], F8, tag="pt8", name="pt8")
                        # slot-major layout: exp writes are fully contiguous,
                        # PV reads [slot, x] per head (valid DoubleRow rhs)
                        pair_v = pt[:].rearrange("p (s h x) -> p s h x", s=2, h=2)
                        pair_hv = pt[:].rearrange("p (s h x) -> p h s x", s=2, h=2)
                        if packed:
                            # both chunks' scores into one st tile -> single exp
                            st = ps_att.tile([128, 2 * SALIGN], F32, tag="s", name="s", bufs=3)
                            for s in range(2):
                                jc = 2 * pp + s
                                for h in range(2):
                                    nc.tensor.matmul(
                                        st[:, h * SALIGN + s * blk : h * SALIGN + (s + 1) * blk],
                                        k_sb[h * HD : (h + 1) * HD, jc * JC : (jc + 1) * JC],
                                        q_t[ib][h * HD : (h + 1) * HD, :],
                                        start=True, stop=True,
                                    )
                            src4 = st[:].rearrange("p (h s x) -> p h s x", h=2, s=2)
                            dst4 = pair_hv[:, :, :, 0:blk]
                            if pp % 2 == 0:
                                nc.scalar.activation(dst4, src4, AF.Exp, scale=1.0 / 16.0)
                            else:
                                nc.vector.tensor_scalar(
                                    dst4.bitcast(U8), src4, EXPA, EXPB,
                                    op0=OP.mult, op1=OP.add,
                                )
                        else:
                            for s in range(2):
                                jc = 2 * pp + s
                                st = emit_s(jc)
                                emit_exp(jc, st, pair_v, s)
                        job = jobs.pop((ib, pp), None)
                        if job is not None:
                            job()
                        if pp > 0:
                            emit_pv(pp - 1, prev_pair)
                        prev_pair = pair_v
                    emit_pv(NJP - 1, prev_pair)
                    emit_epilogue(i0, i1, u)

            # ---- output projections (per head, unnormalized) + stores ----
            # copies land in the merged y_sb accumulators; four big DMAs at the
            # end (two per queue) replace 20 small serialized stores
            with tc.tile_pool(name="ps_fin", bufs=4, space=bass.MemorySpace.PSUM) as ps_fin:
                n = 0
                for i0, i1, hos in ho_saved:
                    blk = i1 - i0
                    for h in range(2):
                        for mt in range(2):
                            yp = ps_fin.tile([128, SALIGN], F32, tag="yp", name="yp")
                            nc.tensor.matmul(
                                yp[:, 0:blk],
                                wo_h[h][0:64, mt * 128 : (mt + 1) * 128],
                                hos[h][0:HD, :],
                                start=True, stop=True,
                            )
                            # 3 of 5 copies on DVE (0.5us) vs 2 on ACT (0.72us)
                            if n % 5 < 3:
                                nc.vector.tensor_copy(y_sb[h][mt][:, i0:i1], yp[:, 0:blk])
                            else:
                                nc.scalar.copy(y_sb[h][mt][:, i0:i1], yp[:, 0:blk])
                            n += 1
                    if n == 4:
                        # first-processed iblk (the 256-wide one) ships at once
                        a0, a1 = ho_saved[0][0], ho_saved[0][1]
                        for h in range(2):
                            for mt in range(2):
                                nc.sync.dma_start(
                                    y_d[h][mt * 128 : (mt + 1) * 128, a0:a1],
                                    y_sb[h][mt][:, a0:a1],
                                )
                    if i1 == 1024:
                        # first two iblks projected: ship y[:, 0:1024] now so the
                        # final drain only waits on the second wave
                        for h in range(2):
                            for mt in range(2):
                                eng = nc.sync if (h + mt) % 2 == 0 else nc.scalar
                                eng.dma_start(
                                    y_d[h][mt * 128 : (mt + 1) * 128, 0:1024],
                                    y_sb[h][mt][:, 0:1024],
                                )
                for h in range(2):
                    for mt in range(2):
                        eng = nc.sync if (h + mt) % 2 == 0 else nc.scalar
                        eng.dma_start(
                            y_d[h][mt * 128 : (mt + 1) * 128, 1024:2048],
                            y_sb[h][mt][:, 1024:2048],
                        )

    nc.compile()
    return nc


def _consts():
    # gind[:, 0:16]: tile-0 channel -> group one-hot; [:, 16:32]: tile-1 channel -> group
    gind = np.zeros((128, 32), np.float32)
    for c in range(128):
        gind[c, c // GC] = 1.0
        gind[c, 16 + 8 + c // GC] = 1.0
    gbc = np.zeros((16, C), np.float32)
    for c in range(C):
        gbc[c // GC, c] = 1.0
    return gind, gbc


def make_in_maps(x, gn_weight, gn_bias, qkv_w, out_w, out_b):
    import ml_dtypes
    x = np.asarray(x, np.float32)
    qkv_w = np.asarray(qkv_w, np.float32)
    out_w = np.asarray(out_w, np.float32)
    gn_weight = np.asarray(gn_weight, np.float32)
    gn_bias = np.asarray(gn_bias, np.float32)
    xr = np.ascontiguousarray(x.reshape(B, C, HW).astype(ml_dtypes.bfloat16))
    gind, gbc = _consts()
    gnp = np.ascontiguousarray(np.stack([gn_weight, gn_bias], axis=1))
    in_maps = []
    for core in range(NCORES):
        b, hp = divmod(core, 2)
        heads = (2 * hp, 2 * hp + 1)
        qs = np.concatenate([qkv_w[n * 192 : n * 192 + 64] for n in heads], 0)
        ks = np.concatenate([qkv_w[n * 192 + 64 : n * 192 + 128] for n in heads], 0)
        vs = np.concatenate([qkv_w[n * 192 + 128 : n * 192 + 192] for n in heads], 0)
        in_maps.append({
            "x": xr[b],
            "wq": np.ascontiguousarray(qs.T),
            "wk": np.ascontiguousarray(ks.T),
            "wv": np.ascontiguousarray(vs.T),
            "wo": np.ascontiguousarray(out_w[:, hp * 128 : (hp + 1) * 128].T),
            "gnp": gnp,
            "gind": gind,
            "gbc": gbc,
        })
    return in_maps


def gather(results, x, out_b):
    """Host-side: divide per-head partials by softmax denominators, sum, add
    residual + bias."""
    x = np.asarray(x, np.float32)
    out_b = np.asarray(out_b, np.float32)
    xr = x.reshape(B, C, HW)
    y = np.empty((B, C, HW), np.float32)
    for b in range(B):
        acc = xr[b] + out_b[:, None]
        for hp in range(2):
            r = results[2 * b + hp]
            dns = np.asarray(r["dns"], np.float32)
            acc = acc + np.asarray(r["y0"], np.float32) / dns[0][None, :]
            acc = acc + np.asarray(r["y1"], np.float32) / dns[1][None, :]
        y[b] = acc
    return y.reshape(B, C, H, W)


_NC_CACHE = {}


def get_nc(mm_dt=BF16):
    key = str(mm_dt)
    if key not in _NC_CACHE:
        _NC_CACHE[key] = _build(mm_dt)
    return _NC_CACHE[key]


def kernel(x, gn_weight, gn_bias, qkv_w, out_w, out_b):
    nc = get_nc(BF16)
    in_maps = make_in_maps(x, gn_weight, gn_bias, qkv_w, out_w, out_b)
    res = bass_utils.run_bass_kernel_spmd(nc, in_maps, core_ids=list(range(NCORES)))
    return gather(res.results, x, out_b)
